# revision 12
# baseline (speedup 1.0000x reference)
"""Mistral attention (B=2, S=2048, D=4096, H=32, KVH=8, HD=128) on 8 trn2 cores.

Sharding: core c -> (batch b = c//4, head-group g = c%4).
Each core computes q/k/v projections for its 8 Q heads + 2 KV heads of one
batch, RoPE, causal attention, and a row-parallel partial o_proj
[2048, 4096]. Host sums the 4 partials per batch. No collectives.

Causal fast path (v2):
- All matmul operands are bf16 (same 1 cycle/row PE rate as float32r, half
  the DMA bytes, no 4x small-free penalty on the causal diagonal strips).
- Weights are streamed per token block in bf16; hidden/cos/sin in bf16.
- Attention is transposed (scoresT[keys, qtok], keys on partitions).
  Softmax denominator: exp tiles are accumulated across key blocks on the
  vector engine, then one ones[128,128]-stationary matmul per (head,qblock)
  produces the denominator pre-broadcast across partitions; a fast
  approximate reciprocal replaces the slow DVE reciprocal.
- The causal mask is added only on the true-diagonal 128x128 tiles
  (in place, into the scores psum).
- o_proj for token block t is fused and its matmuls are interleaved as
  filler work into token block t+1's attention rounds, so the in-order PE
  queue always has independent work while waiting for exp results.
- Output partials are written in bf16 and summed on the host in fp64.
"""

import os
import sys

for _p in ("/opt/trn_rl_repo",):
    if _p not in sys.path:
        sys.path.insert(0, _p)

import numpy as np

import concourse.bass as bass
import concourse.tile as tile
from concourse import bacc, bass_isa, mybir
from concourse.bass_utils import run_bass_kernel_spmd

F32 = mybir.dt.float32
F32R = mybir.dt.float32r
BF16 = mybir.dt.bfloat16
EXP = mybir.ActivationFunctionType.Exp
BF16_NP = mybir.dt.np(mybir.dt.bfloat16)

B, S, D = 2, 2048, 4096
H, KVH, HD = 32, 8, 128
SCALE = HD ** -0.5
NCORES = 8

QH = H // 4              # 8 q heads per core
QCOLS = QH * HD          # 1024
KCOLS = (KVH // 4) * HD  # 256 (2 kv heads per core)
TOK = S

NEG = -1e9

_PROGRAMS = {}


def _build_causal():
    nc = bacc.Bacc("TRN2", target_bir_lowering=False, debug=False)

    hT = nc.dram_tensor("hT", [4, 2, 128, 16 * 512], BF16, kind="ExternalInput").ap()
    wq = nc.dram_tensor("wq", [8, 2, 128, 2048], BF16, kind="ExternalInput").ap()
    wk = nc.dram_tensor("wk", [2, 2, 128, 2048], BF16, kind="ExternalInput").ap()
    wvT = nc.dram_tensor("wvT", [128, 32 * 256], BF16, kind="ExternalInput").ap()
    wo = nc.dram_tensor("wo", [8, 128, 8 * 512], BF16, kind="ExternalInput").ap()
    cosT = nc.dram_tensor("cosT", [HD, TOK], BF16, kind="ExternalInput").ap()
    sinTr = nc.dram_tensor("sinTr", [HD, TOK], BF16, kind="ExternalInput").ap()
    mask_diag = nc.dram_tensor("mask_diag", [128, 128], F32, kind="ExternalInput").ap()
    out = nc.dram_tensor("out", [TOK, D], BF16, kind="ExternalOutput").ap()

    with tile.TileContext(nc) as tc:
        with tc.tile_pool(name="per", bufs=1) as per, \
             tc.tile_pool(name="wrk", bufs=2) as wrk, \
             tc.tile_pool(name="ps", bufs=2, space="PSUM") as psp:

            mask_sb = per.tile([128, 128], F32, tag="mask")
            cos_sb = per.tile([HD, TOK], BF16, tag="cos")
            sin_sb = per.tile([HD, TOK], BF16, tag="sin")
            kT_sb = per.tile([HD, 2 * TOK], BF16, tag="kT")
            V_sb = per.tile([128, 16 * 256], BF16, tag="V")  # tb-major: tb*256+kv*128
            wv_sb = per.tile([128, 32 * 256], BF16, tag="wv")

            # ---- o_proj for token block th, yielded one PE-op at a time ----
            def oproj_gen(th, at2_t):
                wo_ts = {}

                def load(nb):
                    t = wrk.tile([128, 4096], BF16, tag="wo", bufs=4,
                                 name=f"wo_{th}_{nb}")
                    nc.sync.dma_start(t[:, :2048], wo[nb, :, :2048])
                    nc.sync.dma_start(t[:, 2048:], wo[nb, :, 2048:])
                    wo_ts[nb] = t

                load(0)
                load(1)
                for nb in range(8):
                    wo_t = wo_ts.pop(nb)
                    if nb + 2 < 8:
                        load(nb + 2)
                    for j in range(4):
                        po = psp.tile([128, 512], F32, tag="pc",
                                      name=f"po_{th}_{nb}_{j}")
                        for hc in range(8):
                            nc.tensor.matmul(
                                po[:],
                                at2_t[:, hc * 512 + j * 128: hc * 512 + j * 128 + 128],
                                wo_t[:, hc * 512:(hc + 1) * 512],
                                start=(hc == 0), stop=(hc == 7))
                            yield
                        ot = wrk.tile([128, 512], BF16, tag="ot", bufs=4,
                                      name=f"ot_{th}_{nb}_{j}")
                        nc.scalar.copy(ot[:], po[:])
                        nc.scalar.dma_start(
                            out[th * 512 + j * 128: th * 512 + (j + 1) * 128,
                                nb * 512:(nb + 1) * 512],
                            ot[:])
                        yield

            filler = [iter(())]

            def fill(n):
                for _ in range(n):
                    try:
                        next(filler[0])
                    except StopIteration:
                        return

            def attention_group(th, hs, qT_t, at2_t, fin_q):
                """Transposed causal attention for q heads hs, query block th."""
                nkb = 4 * th + 4
                att_ps = [psp.tile([128, 512], F32, tag="att",
                                   name=f"att_{h}_{th}") for h in hs]
                accs = [wrk.tile([128, 512], F32R, tag="acc", bufs=4,
                                 name=f"acc_{h}_{th}") for h in hs]
                pend = [None, None]
                pend_acc = [None, None]

                def emit_av(i, kb, expT, co):
                    h = hs[i]
                    kv = h // 4
                    nc.tensor.matmul(
                        att_ps[i][:, co:],
                        V_sb[:, kb * 256 + kv * 128: kb * 256 + (kv + 1) * 128],
                        expT[:, co:],
                        start=(kb == 0), stop=(kb == nkb - 1))

                def emit_acc(i, kb, expT, co):
                    if kb == 0:
                        nc.vector.tensor_scalar_add(accs[i][:], expT[:], 0.0)
                    else:
                        nc.vector.tensor_add(
                            accs[i][:, co:], accs[i][:, co:], expT[:, co:])

                for kb in range(nkb):
                    o = kb - 4 * th
                    co = o * 128 if o > 0 else 0
                    exps = []
                    for i, h in enumerate(hs):
                        kv = h // 4
                        s_ps = psp.tile([128, 512], F32, tag="pb",
                                        name=f"s_{h}_{th}_{kb}")
                        nc.tensor.matmul(
                            s_ps[:, co:],
                            kT_sb[:, kv * TOK + kb * 128: kv * TOK + (kb + 1) * 128],
                            qT_t[:, h * 512 + co: (h + 1) * 512],
                            start=True, stop=True)
                        if o >= 0:
                            nc.vector.tensor_add(
                                s_ps[:, co:co + 128], s_ps[:, co:co + 128],
                                mask_sb[:])
                        expT = wrk.tile([128, 512], BF16, tag="expT", bufs=4,
                                        name=f"exp_{h}_{th}_{kb}")
                        nc.scalar.activation(
                            expT[:, co:], s_ps[:, co:], EXP, scale=float(SCALE))
                        exps.append(expT)
                    for i in range(2):
                        if pend[i] is not None:
                            emit_av(i, *pend[i])
                        pend[i] = (kb, exps[i], co)
                    for i in range(2):
                        if pend_acc[i] is not None:
                            emit_acc(i, *pend_acc[i])
                        pend_acc[i] = (kb, exps[i], co)
                    if kb == 2 and fin_q:
                        fin_q.pop(0)()
                    fill(8)
                for i in range(2):
                    emit_av(i, *pend[i])
                    emit_acc(i, *pend_acc[i])

                def finisher():
                    for i, h in enumerate(hs):
                        den_bc = wrk.tile([128, 512], F32, tag="den",
                                          name=f"den_{h}_{th}")
                        nc.gpsimd.partition_all_reduce(
                            den_bc[:], accs[i][:], 128, bass_isa.ReduceOp.add)
                        rb = wrk.tile([128, 512], F32, tag="rb",
                                      name=f"rb_{h}_{th}")
                        nc.vector.reciprocal_approx_fast(rb[:], den_bc[:])
                        nc.vector.tensor_mul(
                            at2_t[:, h * 512:(h + 1) * 512], att_ps[i][:], rb[:])
                return finisher

            for th in range(4):
                ts = th * 512
                hts = []
                for jj in range(8):
                    t = wrk.tile([128, 2048], BF16, tag="hT", bufs=11,
                                 name=f"hT_{th}_{jj}")
                    half, j4 = divmod(jj, 4)
                    nc.sync.dma_start(
                        t[:, :1024], hT[th, half, :, j4 * 2048: j4 * 2048 + 1024])
                    nc.sync.dma_start(
                        t[:, 1024:], hT[th, half, :, j4 * 2048 + 1024:(j4 + 1) * 2048])
                    hts.append(t)
                    if th == 0 and jj == 0:
                        # first weight block right behind the first hidden
                        # tile so the PE can start ~4us in, ahead of the
                        # bulk of the startup DMA traffic
                        w_first = wrk.tile([128, 4096], BF16, tag="w", bufs=3,
                                           name="w_k0_0")
                        for q4 in range(4):
                            nc.sync.dma_start(
                                w_first[:, q4 * 1024:(q4 + 1) * 1024],
                                wk[0, q4 // 2, :, (q4 % 2) * 1024:
                                   (q4 % 2 + 1) * 1024])
                if th == 0:
                    nc.sync.dma_start(cos_sb[:], cosT[:])
                    nc.sync.dma_start(sin_sb[:], sinTr[:])
                    nc.sync.dma_start(mask_sb[:], mask_diag[:])
                    for q4 in range(4):
                        nc.sync.dma_start(
                            wv_sb[:, q4 * 2048:(q4 + 1) * 2048],
                            wvT[:, q4 * 2048:(q4 + 1) * 2048])
                qT_t = wrk.tile([128, 8 * 512], BF16, tag="qT", name=f"qT_{th}")
                at2_t = wrk.tile([128, 8 * 512], BF16, tag="at2", name=f"at2_{th}")

                def proj_block(wsrc, idx, kind, pre=None):
                    if pre is not None:
                        w_t = pre
                    else:
                        w_t = wrk.tile([128, 4096], BF16, tag="w", bufs=3,
                                       name=f"w_{kind}{idx}_{th}")
                        nc.sync.dma_start(w_t[:, :2048], wsrc[idx, 0])
                        nc.sync.dma_start(w_t[:, 2048:], wsrc[idx, 1])
                    ps = psp.tile([128, 512], F32, tag="pa",
                                  name=f"ps_{kind}{idx}_{th}")
                    for ic in range(32):
                        half, i = divmod(ic, 16)
                        nc.tensor.matmul(
                            ps[:],
                            w_t[:, half * 2048 + i * 128: half * 2048 + (i + 1) * 128],
                            hts[ic // 4][:, (ic % 4) * 512: (ic % 4 + 1) * 512],
                            start=(ic == 0), stop=(ic == 31))
                    return ps

                def rope(ps, dst, kind, idx):
                    m1 = wrk.tile([128, 512], F32, tag="m1",
                                  name=f"m1_{kind}{idx}_{th}")
                    nc.vector.tensor_mul(m1[:], ps[:], cos_sb[:, ts:ts + 512])
                    m2 = wrk.tile([128, 512], F32, tag="m2",
                                  name=f"m2_{kind}{idx}_{th}")
                    nc.vector.tensor_mul(
                        m2[0:64, :], ps[64:128, :], sin_sb[0:64, ts:ts + 512])
                    nc.vector.tensor_mul(
                        m2[64:128, :], ps[0:64, :], sin_sb[64:128, ts:ts + 512])
                    nc.vector.tensor_add(dst, m1[:], m2[:])

                for kv in range(2):
                    ps = proj_block(wk, kv, "k",
                                    pre=w_first if (th == 0 and kv == 0) else None)
                    rope(ps, kT_sb[:, kv * TOK + ts: kv * TOK + ts + 512], "k", kv)
                    fill(4)
                # V^T directly: out[tok, vdim] with hidden chunks stationary
                for j in range(4):
                    tb = th * 4 + j
                    pv = psp.tile([128, 256], F32, tag="pa", name=f"pv_{th}_{j}")
                    for c in range(32):
                        nc.tensor.matmul(
                            pv[:],
                            hts[c // 4][:, (c % 4) * 512 + j * 128:
                                        (c % 4) * 512 + (j + 1) * 128],
                            wv_sb[:, c * 256:(c + 1) * 256],
                            start=(c == 0), stop=(c == 31))
                    nc.scalar.copy(V_sb[:, tb * 256:(tb + 1) * 256], pv[:])
                    fill(2)

                fin_q = []
                for hp in range(0, QH, 2):
                    for h in (hp, hp + 1):
                        ps = proj_block(wq, h, "q")
                        rope(ps, qT_t[:, h * 512:(h + 1) * 512], "q", h)
                        fill(4)
                    fin = attention_group(th, [hp, hp + 1], qT_t, at2_t, fin_q)
                    fin_q.append(fin)
                while fin_q:
                    fin_q.pop(0)()
                    fill(8)

                # drain previous block's o_proj, then queue this block's
                fill(1 << 30)
                filler[0] = oproj_gen(th, at2_t)
            fill(1 << 30)

    nc.compile()
    return nc


def _build_program(variant: str):
    """variant: 'zero' | 'general' (legacy fp32r path, kept as fallback)"""
    nc = bacc.Bacc("TRN2", target_bir_lowering=False, debug=False)

    hT = nc.dram_tensor("hT", [4, 2, 128, 16 * 512], F32R, kind="ExternalInput").ap()
    wq = nc.dram_tensor("wq", [8, 2, 128, 16 * 128], F32R, kind="ExternalInput").ap()
    wk = nc.dram_tensor("wk", [2, 2, 128, 16 * 128], F32R, kind="ExternalInput").ap()
    wv = nc.dram_tensor("wv", [2, 2, 128, 16 * 128], F32R, kind="ExternalInput").ap()
    wo = nc.dram_tensor("wo", [8, 8, 128, 512], F32R, kind="ExternalInput").ap()
    cosT = nc.dram_tensor("cosT", [HD, TOK], F32, kind="ExternalInput").ap()
    sinTr = nc.dram_tensor("sinTr", [HD, TOK], F32, kind="ExternalInput").ap()
    ident = nc.dram_tensor("ident", [128, 128], F32R, kind="ExternalInput").ap()
    ones = nc.dram_tensor("ones", [128, 1], F32R, kind="ExternalInput").ap()
    if variant == "general":
        maskT = nc.dram_tensor("maskT", [S, S], F32, kind="ExternalInput").ap()
    else:
        maskT = None
    out = nc.dram_tensor("out", [TOK, D], F32, kind="ExternalOutput").ap()

    attnT_spill = nc.dram_tensor("attnT_spill", [QCOLS, TOK], F32R).ap()
    qT_spill = nc.dram_tensor("qT_spill", [QCOLS, TOK], F32R).ap()

    NTH = 4
    THW = TOK // NTH         # 512
    NCH = D // 128           # 32 contraction chunks
    NCB = (QCOLS + 2 * KCOLS) // 128  # 12: 0-7 q, 8-9 k, 10-11 v

    with tile.TileContext(nc) as tc:
        with tc.tile_pool(name="per", bufs=1) as per, \
             tc.tile_pool(name="wrk", bufs=2) as wrk, \
             tc.tile_pool(name="one", bufs=1) as one, \
             tc.tile_pool(name="ps", bufs=2, space="PSUM") as psp:

            ident_sb = per.tile([128, 128], F32R, tag="ident")
            ones_sb = per.tile([128, 1], F32R, tag="ones")
            kT_sb = per.tile([HD, 2 * TOK], F32R, tag="kT")
            V_sb = per.tile([128, (TOK // 128) * KCOLS], F32R, tag="V")
            nc.sync.dma_start(ident_sb[:], ident[:])
            nc.sync.dma_start(ones_sb[:], ones[:])

            def attention_group(hs, qb, qT_aps):
                qs = qb * 512
                nkb = TOK // 128
                n = len(hs)
                att_ps = [psp.tile([128, 512], F32, tag="aux", name=f"att_{h}_{qb}")
                          for h in hs]
                sum_ps = [psp.tile([1, 512], F32, tag="sum", name=f"sum_{h}_{qb}")
                          for h in hs]

                def emit_av(i, kb, expT, co):
                    h = hs[i]
                    kv = h // (QH // 2)
                    nc.tensor.matmul(
                        att_ps[i][:, co:],
                        V_sb[:, kb * KCOLS + kv * 128: kb * KCOLS + (kv + 1) * 128],
                        expT[:, co:],
                        start=(kb == 0), stop=(kb == nkb - 1))
                    nc.tensor.matmul(
                        sum_ps[i][:, co:], ones_sb[:], expT[:, co:],
                        start=(kb == 0), stop=(kb == nkb - 1))

                pend = [None] * n
                for kb in range(nkb):
                    co = 0
                    exps = []
                    for i, h in enumerate(hs):
                        kv = h // (QH // 2)
                        s_ps = psp.tile([128, 512], F32, tag="pb",
                                        name=f"s_{h}_{qb}_{kb}")
                        nc.tensor.matmul(
                            s_ps[:, co:],
                            kT_sb[:, kv * TOK + kb * 128: kv * TOK + (kb + 1) * 128],
                            qT_aps[i][:, co:],
                            start=True, stop=True)
                        exp_in = s_ps
                        if variant == "general":
                            mt = wrk.tile([128, 512], F32, tag="mt",
                                          name=f"mt_{h}_{qb}_{kb}")
                            nc.sync.dma_start(
                                mt[:], maskT[kb * 128:(kb + 1) * 128, qs:qs + 512])
                            msk = wrk.tile([128, 512], F32, tag="m1",
                                           name=f"mskg_{h}_{qb}_{kb}")
                            nc.vector.tensor_add(msk[:], s_ps[:], mt[:])
                            exp_in = msk
                        expT = wrk.tile([128, 512], F32R, tag="expT", bufs=4,
                                        name=f"exp_{h}_{qb}_{kb}")
                        nc.scalar.activation(
                            expT[:, co:], exp_in[:, co:], EXP, scale=float(SCALE))
                        exps.append(expT)
                    for i in range(n):
                        if pend[i] is not None:
                            emit_av(i, *pend[i])
                        pend[i] = (kb, exps[i], co)
                for i in range(n):
                    emit_av(i, *pend[i])
                for i, h in enumerate(hs):
                    atu = wrk.tile([128, 512], F32, tag="atu",
                                   name=f"atu_{h}_{qb}")
                    nc.scalar.copy(atu[:], att_ps[i][:])
                    recip = wrk.tile([1, 512], F32, tag="rcp",
                                     name=f"rcp_{h}_{qb}")
                    nc.vector.reciprocal(recip[:], sum_ps[i][:])
                    rb = wrk.tile([128, 512], F32, tag="m2",
                                  name=f"rb_{h}_{qb}")
                    nc.gpsimd.partition_broadcast(rb[:], recip[:])
                    at2 = wrk.tile([128, 512], F32R, tag="vT",
                                   name=f"at2_{h}_{qb}")
                    nc.vector.tensor_mul(at2[:], atu[:], rb[:])
                    nc.scalar.dma_start(
                        attnT_spill[h * 128:(h + 1) * 128, qs:qs + 512], at2[:])

            # ============ Phase A ============
            for th in range(NTH):
                ts = th * THW
                hts = []
                for j in range(8):
                    t = one.tile([128, 4 * THW], F32R, tag=f"hT{j}")
                    half, jj = divmod(j, 4)
                    nc.sync.dma_start(
                        t[:, :1024], hT[th, half, :, jj * 2048:jj * 2048 + 1024])
                    nc.sync.dma_start(
                        t[:, 1024:], hT[th, half, :, jj * 2048 + 1024:(jj + 1) * 2048])
                    hts.append(t)
                cos_t = wrk.tile([HD, THW], F32, tag="cos")
                sin_t = wrk.tile([HD, THW], F32, tag="sin")
                nc.sync.dma_start(cos_t[:], cosT[:, ts:ts + THW])
                nc.sync.dma_start(sin_t[:], sinTr[:, ts:ts + THW])

                qT_lo = one.tile([128, 4 * 512], F32R, tag="qTbl")
                qT_hi = one.tile([128, 4 * 512], F32R, tag="qTbh")

                for cb in range(NCB):
                    if cb < 8:
                        wsrc, widx = wq, cb
                    elif cb < 10:
                        wsrc, widx = wk, cb - 8
                    else:
                        wsrc, widx = wv, cb - 10
                    ps = psp.tile([128, THW], F32, tag="pa")
                    for half in range(2):
                        w_sb = wrk.tile([128, (NCH // 2) * 128], F32R, tag="w")
                        nc.sync.dma_start(w_sb[:, :1024], wsrc[widx, half, :, :1024])
                        nc.sync.dma_start(w_sb[:, 1024:], wsrc[widx, half, :, 1024:])
                        for i in range(NCH // 2):
                            ic = half * (NCH // 2) + i
                            t = hts[ic // 4]
                            nc.tensor.matmul(
                                ps[:],
                                w_sb[:, i * 128:(i + 1) * 128],
                                t[:, (ic % 4) * THW:(ic % 4 + 1) * THW],
                                start=(half == 0 and i == 0),
                                stop=(half == 1 and i == NCH // 2 - 1),
                            )
                    if cb < 10:
                        m1 = wrk.tile([128, THW], F32, tag="m1")
                        nc.vector.tensor_mul(m1[:], ps[:], cos_t[:])
                        m2 = wrk.tile([128, THW], F32, tag="m2")
                        nc.vector.tensor_mul(m2[0:64, :], ps[64:128, :], sin_t[0:64, :])
                        nc.vector.tensor_mul(m2[64:128, :], ps[0:64, :], sin_t[64:128, :])
                        if cb < 8:
                            qdst = qT_lo if cb < 4 else qT_hi
                            nc.vector.tensor_add(
                                qdst[:, (cb % 4) * 512:(cb % 4 + 1) * 512],
                                m1[:], m2[:])
                        else:
                            kv = cb - 8
                            nc.vector.tensor_add(
                                kT_sb[:, kv * TOK + ts: kv * TOK + ts + THW],
                                m1[:], m2[:])
                    else:
                        kv = cb - 10
                        vT = wrk.tile([128, THW], F32R, tag="vT")
                        nc.scalar.copy(vT[:], ps[:])
                        for j in range(THW // 128):
                            tb = th * (THW // 128) + j
                            pt = psp.tile([128, 128], F32R, tag="aux")
                            nc.tensor.transpose(
                                pt[:], vT[:, j * 128:(j + 1) * 128], ident_sb[:])
                            nc.scalar.copy(
                                V_sb[:, tb * KCOLS + kv * 128:
                                     tb * KCOLS + (kv + 1) * 128],
                                pt[:])

                for qi, qt in ((0, qT_lo), (1, qT_hi)):
                    nc.scalar.dma_start(
                        qT_spill[qi * 512:(qi + 1) * 512, ts:ts + THW]
                        .rearrange("(i p) t -> p i t", p=128),
                        qt[:].rearrange("p (i t) -> p i t", i=4),
                    )

            for hp in range(0, QH, 2):
                for qb in range(4):
                    qts = []
                    for h in (hp, hp + 1):
                        qT_t = wrk.tile([128, 512], F32R, tag="qTs",
                                        name=f"qt_{h}_{qb}")
                        nc.sync.dma_start(
                            qT_t[:],
                            qT_spill[h * 128:(h + 1) * 128,
                                     qb * 512:(qb + 1) * 512])
                        qts.append(qT_t)
                    attention_group([hp, hp + 1], qb, qts)

            # ================= Phase C: o_proj partial =================
            ags = []
            for h in range(QH):
                a = one.tile([128, TOK], F32R, tag=f"hT{h}")
                nc.sync.dma_start(a[:], attnT_spill[h * 128:(h + 1) * 128, :])
                ags.append(a)
            for nb in range(D // 512):
                wo_sb = wrk.tile([128, QH * 512], F32R, tag="w")
                for hc in range(QH):
                    nc.sync.dma_start(
                        wo_sb[:, hc * 512:(hc + 1) * 512], wo[nb, hc])
                for qtb in range(TOK // 128):
                    o_ps = psp.tile([128, 512], F32, tag=["pa", "pb", "aux", "sum"][qtb % 4])
                    for hc in range(QH):
                        nc.tensor.matmul(
                            o_ps[:],
                            ags[hc][:, qtb * 128:(qtb + 1) * 128],
                            wo_sb[:, hc * 512:(hc + 1) * 512],
                            start=(hc == 0), stop=(hc == QH - 1))
                    ot = wrk.tile([128, 512], F32, tag="ot", bufs=4)
                    nc.scalar.copy(ot[:], o_ps[:])
                    nc.scalar.dma_start(
                        out[qtb * 128:(qtb + 1) * 128, nb * 512:(nb + 1) * 512],
                        ot[:])

    nc.compile()
    return nc


def _get_program(variant: str):
    if variant not in _PROGRAMS:
        if variant == "causal":
            _PROGRAMS[variant] = _build_causal()
        else:
            _PROGRAMS[variant] = _build_program(variant)
    return _PROGRAMS[variant]


def _detect_variant(mask: np.ndarray) -> str:
    m = mask.reshape(mask.shape[-2], mask.shape[-1])
    if not m.any():
        return "zero"
    causal = np.where(
        np.tril(np.ones((S, S), dtype=bool)), np.float32(0.0), np.float32(NEG))
    if np.array_equal(m, causal):
        return "causal"
    return "general"


def _tile_w(W, np_dt):  # [4096, C] -> [C//128, 2, 128, 2048]
    C = W.shape[1]
    return np.ascontiguousarray(
        W.reshape(2, 16, 128, C // 128, 128).transpose(3, 0, 2, 1, 4)
        .reshape(C // 128, 2, 128, 16 * 128).astype(np_dt))


def _kernel_causal(hidden_states, cos, sin, Wq, Wk, Wv, Wo, trace):
    nc = _get_program("causal")

    i = np.arange(128)[:, None]
    j = np.arange(128)[None, :]
    mask_diag = np.where(i <= j, np.float32(0.0),
                         np.float32(NEG / SCALE)).astype(np.float32)

    per_batch = {}
    for b in range(B):
        sT = np.ascontiguousarray(sin[b].T)
        sinTr = np.concatenate([-sT[:64], sT[64:]], axis=0)
        hid = hidden_states[b]  # [2048, 4096]
        hT_t = np.ascontiguousarray(
            hid.reshape(4, 512, 2, 16, 128).transpose(0, 2, 4, 3, 1)
            .reshape(4, 2, 128, 16 * 512).astype(BF16_NP))
        per_batch[b] = (hT_t,
                        np.ascontiguousarray(cos[b].T.astype(BF16_NP)),
                        np.ascontiguousarray(sinTr.astype(BF16_NP)))

    in_maps = []
    for c in range(NCORES):
        b, g = divmod(c, 4)
        hT_t, cosT, sinTr = per_batch[b]
        wo_c = Wo[g * QCOLS:(g + 1) * QCOLS, :]  # [1024, 4096]
        wo_t = np.ascontiguousarray(
            wo_c.reshape(8, 128, 8, 512).transpose(2, 1, 0, 3)
            .reshape(8, 128, 8 * 512).astype(BF16_NP))
        wv_c = Wv[:, g * KCOLS:(g + 1) * KCOLS]  # [4096, 256]
        wvT_t = np.ascontiguousarray(
            wv_c.reshape(32, 128, 256).transpose(1, 0, 2)
            .reshape(128, 32 * 256).astype(BF16_NP))
        im = {
            "hT": hT_t,
            "wq": _tile_w(Wq[:, g * QCOLS:(g + 1) * QCOLS], BF16_NP),
            "wk": _tile_w(Wk[:, g * KCOLS:(g + 1) * KCOLS], BF16_NP),
            "wvT": wvT_t,
            "wo": wo_t,
            "cosT": cosT,
            "sinTr": sinTr,
            "mask_diag": mask_diag,
        }
        in_maps.append(im)

    res = run_bass_kernel_spmd(nc, in_maps, core_ids=list(range(NCORES)),
                               trace=trace)
    if trace:
        print(f"HW exec time: {res.exec_time_ns} ns")

    out = np.empty((B, S, D), dtype=np.float32)
    for b in range(B):
        acc = np.zeros((S, D), dtype=np.float64)
        for g in range(4):
            acc += np.asarray(res.results[4 * b + g]["out"], dtype=np.float64)
        out[b] = acc.astype(np.float32)
    return out


def _kernel_legacy(variant, hidden_states, cos, sin, attention_mask,
                   Wq, Wk, Wv, Wo, trace):
    nc = _get_program(variant)

    ident = np.eye(128, dtype=np.float32)
    ones = np.ones((128, 1), dtype=np.float32)

    if variant == "general":
        m = attention_mask.reshape(S, S)
        maskT = np.ascontiguousarray(m.T / np.float32(SCALE))
    else:
        maskT = None

    per_batch = {}
    for b in range(B):
        sT = np.ascontiguousarray(sin[b].T)
        sinTr = np.concatenate([-sT[:64], sT[64:]], axis=0)
        hid = hidden_states[b]  # [2048, 4096]
        hT_t = np.ascontiguousarray(
            hid.reshape(4, 512, 2, 16, 128).transpose(0, 2, 4, 3, 1)
            .reshape(4, 2, 128, 16 * 512))
        per_batch[b] = (hT_t, np.ascontiguousarray(cos[b].T),
                        np.ascontiguousarray(sinTr))

    in_maps = []
    for c in range(NCORES):
        b, g = divmod(c, 4)
        hT_t, cosT, sinTr = per_batch[b]
        wo_c = Wo[g * QCOLS:(g + 1) * QCOLS, :]  # [1024, 4096]
        wo_t = np.ascontiguousarray(
            wo_c.reshape(8, 128, 8, 512).transpose(2, 0, 1, 3))
        im = {
            "hT": hT_t,
            "wq": _tile_w(Wq[:, g * QCOLS:(g + 1) * QCOLS], np.float32),
            "wk": _tile_w(Wk[:, g * KCOLS:(g + 1) * KCOLS], np.float32),
            "wv": _tile_w(Wv[:, g * KCOLS:(g + 1) * KCOLS], np.float32),
            "wo": wo_t,
            "cosT": cosT,
            "sinTr": sinTr,
            "ident": ident,
            "ones": ones,
        }
        if maskT is not None:
            im["maskT"] = maskT
        in_maps.append(im)

    res = run_bass_kernel_spmd(nc, in_maps, core_ids=list(range(NCORES)),
                               trace=trace)
    if trace:
        print(f"HW exec time: {res.exec_time_ns} ns")

    out = np.empty((B, S, D), dtype=np.float32)
    for b in range(B):
        acc = np.zeros((S, D), dtype=np.float64)
        for g in range(4):
            acc += res.results[4 * b + g]["out"]
        out[b] = acc.astype(np.float32)
    return out


def kernel(hidden_states, cos, sin, attention_mask, Wq, Wk, Wv, Wo):
    hidden_states = np.asarray(hidden_states, dtype=np.float32)
    cos = np.asarray(cos, dtype=np.float32)
    sin = np.asarray(sin, dtype=np.float32)
    attention_mask = np.asarray(attention_mask, dtype=np.float32)
    Wq = np.asarray(Wq, dtype=np.float32)
    Wk = np.asarray(Wk, dtype=np.float32)
    Wv = np.asarray(Wv, dtype=np.float32)
    Wo = np.asarray(Wo, dtype=np.float32)

    trace = bool(os.environ.get("KERNEL_TRACE"))
    variant = _detect_variant(attention_mask)
    if variant == "causal":
        return _kernel_causal(hidden_states, cos, sin, Wq, Wk, Wv, Wo, trace)
    return _kernel_legacy(variant, hidden_states, cos, sin, attention_mask,
                          Wq, Wk, Wv, Wo, trace)


# revision 19
# speedup vs baseline: 1.0265x; 1.0265x over previous
"""Mistral attention (B=2, S=2048, D=4096, H=32, KVH=8, HD=128) on 8 trn2 cores.

Sharding: core c -> (batch b = c//4, head-group g = c%4).
Each core computes q/k/v projections for its 8 Q heads + 2 KV heads of one
batch, RoPE, causal attention, and a row-parallel partial o_proj
[2048, 4096]. Host sums the 4 partials per batch. No collectives.

Causal fast path (v2):
- All matmul operands are bf16 (same 1 cycle/row PE rate as float32r, half
  the DMA bytes, no 4x small-free penalty on the causal diagonal strips).
- Weights are streamed per token block in bf16; hidden/cos/sin in bf16.
- Attention is transposed (scoresT[keys, qtok], keys on partitions).
  Softmax denominator: exp tiles are accumulated across key blocks on the
  vector engine, then one ones[128,128]-stationary matmul per (head,qblock)
  produces the denominator pre-broadcast across partitions; a fast
  approximate reciprocal replaces the slow DVE reciprocal.
- The causal mask is added only on the true-diagonal 128x128 tiles
  (in place, into the scores psum).
- o_proj for token block t is fused and its matmuls are interleaved as
  filler work into token block t+1's attention rounds, so the in-order PE
  queue always has independent work while waiting for exp results.
- Output partials are written in bf16 and summed on the host in fp64.
"""

import os
import sys

for _p in ("/opt/trn_rl_repo",):
    if _p not in sys.path:
        sys.path.insert(0, _p)

import numpy as np

import concourse.bass as bass
import concourse.tile as tile
from concourse import bacc, bass_isa, mybir
from concourse.bass_utils import run_bass_kernel_spmd

F32 = mybir.dt.float32
F32R = mybir.dt.float32r
BF16 = mybir.dt.bfloat16
EXP = mybir.ActivationFunctionType.Exp
BF16_NP = mybir.dt.np(mybir.dt.bfloat16)

B, S, D = 2, 2048, 4096
H, KVH, HD = 32, 8, 128
SCALE = HD ** -0.5
NCORES = 8

QH = H // 4              # 8 q heads per core
QCOLS = QH * HD          # 1024
KCOLS = (KVH // 4) * HD  # 256 (2 kv heads per core)
TOK = S

NEG = -1e9

_PROGRAMS = {}


def _build_causal():
    nc = bacc.Bacc("TRN2", target_bir_lowering=False, debug=False)

    hT = nc.dram_tensor("hT", [4, 2, 128, 16 * 512], BF16, kind="ExternalInput").ap()
    wq = nc.dram_tensor("wq", [8, 2, 128, 2048], BF16, kind="ExternalInput").ap()
    wk = nc.dram_tensor("wk", [2, 2, 128, 2048], BF16, kind="ExternalInput").ap()
    wvT = nc.dram_tensor("wvT", [128, 32 * 256], BF16, kind="ExternalInput").ap()
    wo = nc.dram_tensor("wo", [8, 128, 8 * 512], BF16, kind="ExternalInput").ap()
    cosT = nc.dram_tensor("cosT", [HD, TOK], BF16, kind="ExternalInput").ap()
    sinTr = nc.dram_tensor("sinTr", [HD, TOK], BF16, kind="ExternalInput").ap()
    ones_sq = nc.dram_tensor("ones_sq", [128, 128], F32R, kind="ExternalInput").ap()
    mask_diag = nc.dram_tensor("mask_diag", [128, 128], F32, kind="ExternalInput").ap()
    out = nc.dram_tensor("out", [TOK, D], BF16, kind="ExternalOutput").ap()

    with tile.TileContext(nc) as tc:
        with tc.tile_pool(name="per", bufs=1) as per, \
             tc.tile_pool(name="wrk", bufs=2) as wrk, \
             tc.tile_pool(name="ps", bufs=2, space="PSUM") as psp:

            mask_sb = per.tile([128, 128], F32, tag="mask")
            ones_sb = per.tile([128, 128], F32R, tag="ones")
            cos_sb = per.tile([HD, TOK], BF16, tag="cos")
            sin_sb = per.tile([HD, TOK], BF16, tag="sin")
            kT_sb = per.tile([HD, 2 * TOK], BF16, tag="kT")
            V_sb = per.tile([128, 16 * 256], BF16, tag="V")  # tb-major: tb*256+kv*128
            wv_sb = per.tile([128, 32 * 256], BF16, tag="wv")

            # ---- o_proj for token block th, yielded one PE-op at a time ----
            def oproj_gen(th, at2_t):
                wo_ts = {}

                def load(nb):
                    t = wrk.tile([128, 4096], BF16, tag="wo", bufs=4,
                                 name=f"wo_{th}_{nb}")
                    nc.sync.dma_start(t[:, :2048], wo[nb, :, :2048])
                    nc.sync.dma_start(t[:, 2048:], wo[nb, :, 2048:])
                    wo_ts[nb] = t

                load(0)
                load(1)
                for nb in range(8):
                    wo_t = wo_ts.pop(nb)
                    if nb + 2 < 8:
                        load(nb + 2)
                    for j in range(4):
                        po = psp.tile([128, 512], F32, tag="pc",
                                      name=f"po_{th}_{nb}_{j}")
                        for hc in range(8):
                            nc.tensor.matmul(
                                po[:],
                                at2_t[:, hc * 512 + j * 128: hc * 512 + j * 128 + 128],
                                wo_t[:, hc * 512:(hc + 1) * 512],
                                start=(hc == 0), stop=(hc == 7))
                            yield
                        ot = wrk.tile([128, 512], BF16, tag="ot", bufs=4,
                                      name=f"ot_{th}_{nb}_{j}")
                        nc.scalar.copy(ot[:], po[:])
                        nc.scalar.dma_start(
                            out[th * 512 + j * 128: th * 512 + (j + 1) * 128,
                                nb * 512:(nb + 1) * 512],
                            ot[:])
                        yield

            filler = [iter(())]

            def fill(n):
                for _ in range(n):
                    try:
                        next(filler[0])
                    except StopIteration:
                        return

            def attention_group(th, hs, qT_t, at2_t, fin_q):
                """Transposed causal attention for q heads hs, query block th."""
                nkb = 4 * th + 4
                att_ps = [psp.tile([128, 512], F32, tag="att",
                                   name=f"att_{h}_{th}") for h in hs]
                accs = [wrk.tile([128, 512], F32R, tag="acc", bufs=4,
                                 name=f"acc_{h}_{th}") for h in hs]
                pend = [None, None]
                pend_acc = [None, None]

                def emit_av(i, kb, expT, co):
                    h = hs[i]
                    kv = h // 4
                    nc.tensor.matmul(
                        att_ps[i][:, co:],
                        V_sb[:, kb * 256 + kv * 128: kb * 256 + (kv + 1) * 128],
                        expT[:, co:],
                        start=(kb == 0), stop=(kb == nkb - 1))

                def emit_acc(i, kb, expT, co):
                    if kb == 0:
                        nc.vector.tensor_scalar_add(accs[i][:], expT[:], 0.0)
                    else:
                        nc.vector.tensor_add(
                            accs[i][:, co:], accs[i][:, co:], expT[:, co:])

                for kb in range(nkb):
                    o = kb - 4 * th
                    co = o * 128 if o > 0 else 0
                    exps = []
                    for i, h in enumerate(hs):
                        kv = h // 4
                        s_ps = psp.tile([128, 512], F32, tag="pb",
                                        name=f"s_{h}_{th}_{kb}")
                        nc.tensor.matmul(
                            s_ps[:, co:],
                            kT_sb[:, kv * TOK + kb * 128: kv * TOK + (kb + 1) * 128],
                            qT_t[:, h * 512 + co: (h + 1) * 512],
                            start=True, stop=True)
                        if o >= 0:
                            nc.vector.tensor_add(
                                s_ps[:, co:co + 128], s_ps[:, co:co + 128],
                                mask_sb[:])
                        expT = wrk.tile([128, 512], BF16, tag="expT", bufs=4,
                                        name=f"exp_{h}_{th}_{kb}")
                        nc.scalar.activation(
                            expT[:, co:], s_ps[:, co:], EXP, scale=float(SCALE))
                        exps.append(expT)
                    for i in range(2):
                        if pend[i] is not None:
                            emit_av(i, *pend[i])
                        pend[i] = (kb, exps[i], co)
                    for i in range(2):
                        if pend_acc[i] is not None:
                            emit_acc(i, *pend_acc[i])
                        pend_acc[i] = (kb, exps[i], co)
                    if kb == 2 and fin_q:
                        fin_q.pop(0)()
                    fill(8)
                for i in range(2):
                    emit_av(i, *pend[i])
                    emit_acc(i, *pend_acc[i])

                def finisher():
                    for i, h in enumerate(hs):
                        den_ps = psp.tile([128, 512], F32, tag="pc",
                                          name=f"den_{h}_{th}")
                        nc.tensor.matmul(den_ps[:], ones_sb[:], accs[i][:],
                                         start=True, stop=True)
                        rb = wrk.tile([128, 512], F32, tag="rb",
                                      name=f"rb_{h}_{th}")
                        nc.vector.reciprocal_approx_fast(rb[:], den_ps[:])
                        nc.vector.tensor_mul(
                            at2_t[:, h * 512:(h + 1) * 512], att_ps[i][:], rb[:])
                return finisher

            for th in range(4):
                ts = th * 512
                hts = []
                for jj in range(8):
                    t = wrk.tile([128, 2048], BF16, tag="hT", bufs=11,
                                 name=f"hT_{th}_{jj}")
                    half, j4 = divmod(jj, 4)
                    nc.sync.dma_start(
                        t[:, :1024], hT[th, half, :, j4 * 2048: j4 * 2048 + 1024])
                    nc.sync.dma_start(
                        t[:, 1024:], hT[th, half, :, j4 * 2048 + 1024:(j4 + 1) * 2048])
                    hts.append(t)
                    if th == 0 and jj == 0:
                        # first weight block right behind the first hidden
                        # tile so the PE can start ~4us in, ahead of the
                        # bulk of the startup DMA traffic
                        w_first = wrk.tile([128, 4096], BF16, tag="w", bufs=3,
                                           name="w_k0_0")
                        for q4 in range(4):
                            nc.sync.dma_start(
                                w_first[:, q4 * 1024:(q4 + 1) * 1024],
                                wk[0, q4 // 2, :, (q4 % 2) * 1024:
                                   (q4 % 2 + 1) * 1024])
                if th == 0:
                    nc.sync.dma_start(cos_sb[:], cosT[:])
                    nc.sync.dma_start(sin_sb[:], sinTr[:])
                    nc.sync.dma_start(mask_sb[:], mask_diag[:])
                    nc.sync.dma_start(ones_sb[:], ones_sq[:])
                    for q4 in range(4):
                        nc.sync.dma_start(
                            wv_sb[:, q4 * 2048:(q4 + 1) * 2048],
                            wvT[:, q4 * 2048:(q4 + 1) * 2048])
                qT_t = wrk.tile([128, 8 * 512], BF16, tag="qT", name=f"qT_{th}")
                at2_t = wrk.tile([128, 8 * 512], BF16, tag="at2", name=f"at2_{th}")

                def proj_block(wsrc, idx, kind, pre=None):
                    if pre is not None:
                        w_t = pre
                    else:
                        w_t = wrk.tile([128, 4096], BF16, tag="w", bufs=3,
                                       name=f"w_{kind}{idx}_{th}")
                        nc.sync.dma_start(w_t[:, :2048], wsrc[idx, 0])
                        nc.sync.dma_start(w_t[:, 2048:], wsrc[idx, 1])
                    ps = psp.tile([128, 512], F32, tag="pa",
                                  name=f"ps_{kind}{idx}_{th}")
                    for ic in range(32):
                        half, i = divmod(ic, 16)
                        nc.tensor.matmul(
                            ps[:],
                            w_t[:, half * 2048 + i * 128: half * 2048 + (i + 1) * 128],
                            hts[ic // 4][:, (ic % 4) * 512: (ic % 4 + 1) * 512],
                            start=(ic == 0), stop=(ic == 31))
                    return ps

                def rope(ps, dst, kind, idx):
                    m1 = wrk.tile([128, 512], F32, tag="m1",
                                  name=f"m1_{kind}{idx}_{th}")
                    nc.vector.tensor_mul(m1[:], ps[:], cos_sb[:, ts:ts + 512])
                    m2 = wrk.tile([128, 512], F32, tag="m2",
                                  name=f"m2_{kind}{idx}_{th}")
                    nc.vector.tensor_mul(
                        m2[0:64, :], ps[64:128, :], sin_sb[0:64, ts:ts + 512])
                    nc.vector.tensor_mul(
                        m2[64:128, :], ps[0:64, :], sin_sb[64:128, ts:ts + 512])
                    nc.vector.tensor_add(dst, m1[:], m2[:])

                for kv in range(2):
                    ps = proj_block(wk, kv, "k",
                                    pre=w_first if (th == 0 and kv == 0) else None)
                    rope(ps, kT_sb[:, kv * TOK + ts: kv * TOK + ts + 512], "k", kv)
                    fill(4)
                fin_q = []
                for hp in range(0, QH, 2):
                    for h in (hp, hp + 1):
                        ps = proj_block(wq, h, "q")
                        rope(ps, qT_t[:, h * 512:(h + 1) * 512], "q", h)
                        fill(4)
                    if hp == 0:
                        # V^T: out[tok, vdim] with hidden chunks stationary
                        for j in range(4):
                            tb = th * 4 + j
                            pv = psp.tile([128, 256], F32, tag="pa",
                                          name=f"pv_{th}_{j}")
                            for c in range(32):
                                nc.tensor.matmul(
                                    pv[:],
                                    hts[c // 4][:, (c % 4) * 512 + j * 128:
                                                (c % 4) * 512 + (j + 1) * 128],
                                    wv_sb[:, c * 256:(c + 1) * 256],
                                    start=(c == 0), stop=(c == 31))
                            nc.scalar.copy(V_sb[:, tb * 256:(tb + 1) * 256], pv[:])
                            fill(2)
                    fin = attention_group(th, [hp, hp + 1], qT_t, at2_t, fin_q)
                    fin_q.append(fin)
                while fin_q:
                    fin_q.pop(0)()
                    fill(8)

                # drain previous block's o_proj, then queue this block's
                fill(1 << 30)
                filler[0] = oproj_gen(th, at2_t)
            fill(1 << 30)

    nc.compile()
    return nc


def _build_program(variant: str):
    """variant: 'zero' | 'general' (legacy fp32r path, kept as fallback)"""
    nc = bacc.Bacc("TRN2", target_bir_lowering=False, debug=False)

    hT = nc.dram_tensor("hT", [4, 2, 128, 16 * 512], F32R, kind="ExternalInput").ap()
    wq = nc.dram_tensor("wq", [8, 2, 128, 16 * 128], F32R, kind="ExternalInput").ap()
    wk = nc.dram_tensor("wk", [2, 2, 128, 16 * 128], F32R, kind="ExternalInput").ap()
    wv = nc.dram_tensor("wv", [2, 2, 128, 16 * 128], F32R, kind="ExternalInput").ap()
    wo = nc.dram_tensor("wo", [8, 8, 128, 512], F32R, kind="ExternalInput").ap()
    cosT = nc.dram_tensor("cosT", [HD, TOK], F32, kind="ExternalInput").ap()
    sinTr = nc.dram_tensor("sinTr", [HD, TOK], F32, kind="ExternalInput").ap()
    ident = nc.dram_tensor("ident", [128, 128], F32R, kind="ExternalInput").ap()
    ones = nc.dram_tensor("ones", [128, 1], F32R, kind="ExternalInput").ap()
    if variant == "general":
        maskT = nc.dram_tensor("maskT", [S, S], F32, kind="ExternalInput").ap()
    else:
        maskT = None
    out = nc.dram_tensor("out", [TOK, D], F32, kind="ExternalOutput").ap()

    attnT_spill = nc.dram_tensor("attnT_spill", [QCOLS, TOK], F32R).ap()
    qT_spill = nc.dram_tensor("qT_spill", [QCOLS, TOK], F32R).ap()

    NTH = 4
    THW = TOK // NTH         # 512
    NCH = D // 128           # 32 contraction chunks
    NCB = (QCOLS + 2 * KCOLS) // 128  # 12: 0-7 q, 8-9 k, 10-11 v

    with tile.TileContext(nc) as tc:
        with tc.tile_pool(name="per", bufs=1) as per, \
             tc.tile_pool(name="wrk", bufs=2) as wrk, \
             tc.tile_pool(name="one", bufs=1) as one, \
             tc.tile_pool(name="ps", bufs=2, space="PSUM") as psp:

            ident_sb = per.tile([128, 128], F32R, tag="ident")
            ones_sb = per.tile([128, 1], F32R, tag="ones")
            kT_sb = per.tile([HD, 2 * TOK], F32R, tag="kT")
            V_sb = per.tile([128, (TOK // 128) * KCOLS], F32R, tag="V")
            nc.sync.dma_start(ident_sb[:], ident[:])
            nc.sync.dma_start(ones_sb[:], ones[:])

            def attention_group(hs, qb, qT_aps):
                qs = qb * 512
                nkb = TOK // 128
                n = len(hs)
                att_ps = [psp.tile([128, 512], F32, tag="aux", name=f"att_{h}_{qb}")
                          for h in hs]
                sum_ps = [psp.tile([1, 512], F32, tag="sum", name=f"sum_{h}_{qb}")
                          for h in hs]

                def emit_av(i, kb, expT, co):
                    h = hs[i]
                    kv = h // (QH // 2)
                    nc.tensor.matmul(
                        att_ps[i][:, co:],
                        V_sb[:, kb * KCOLS + kv * 128: kb * KCOLS + (kv + 1) * 128],
                        expT[:, co:],
                        start=(kb == 0), stop=(kb == nkb - 1))
                    nc.tensor.matmul(
                        sum_ps[i][:, co:], ones_sb[:], expT[:, co:],
                        start=(kb == 0), stop=(kb == nkb - 1))

                pend = [None] * n
                for kb in range(nkb):
                    co = 0
                    exps = []
                    for i, h in enumerate(hs):
                        kv = h // (QH // 2)
                        s_ps = psp.tile([128, 512], F32, tag="pb",
                                        name=f"s_{h}_{qb}_{kb}")
                        nc.tensor.matmul(
                            s_ps[:, co:],
                            kT_sb[:, kv * TOK + kb * 128: kv * TOK + (kb + 1) * 128],
                            qT_aps[i][:, co:],
                            start=True, stop=True)
                        exp_in = s_ps
                        if variant == "general":
                            mt = wrk.tile([128, 512], F32, tag="mt",
                                          name=f"mt_{h}_{qb}_{kb}")
                            nc.sync.dma_start(
                                mt[:], maskT[kb * 128:(kb + 1) * 128, qs:qs + 512])
                            msk = wrk.tile([128, 512], F32, tag="m1",
                                           name=f"mskg_{h}_{qb}_{kb}")
                            nc.vector.tensor_add(msk[:], s_ps[:], mt[:])
                            exp_in = msk
                        expT = wrk.tile([128, 512], F32R, tag="expT", bufs=4,
                                        name=f"exp_{h}_{qb}_{kb}")
                        nc.scalar.activation(
                            expT[:, co:], exp_in[:, co:], EXP, scale=float(SCALE))
                        exps.append(expT)
                    for i in range(n):
                        if pend[i] is not None:
                            emit_av(i, *pend[i])
                        pend[i] = (kb, exps[i], co)
                for i in range(n):
                    emit_av(i, *pend[i])
                for i, h in enumerate(hs):
                    atu = wrk.tile([128, 512], F32, tag="atu",
                                   name=f"atu_{h}_{qb}")
                    nc.scalar.copy(atu[:], att_ps[i][:])
                    recip = wrk.tile([1, 512], F32, tag="rcp",
                                     name=f"rcp_{h}_{qb}")
                    nc.vector.reciprocal(recip[:], sum_ps[i][:])
                    rb = wrk.tile([128, 512], F32, tag="m2",
                                  name=f"rb_{h}_{qb}")
                    nc.gpsimd.partition_broadcast(rb[:], recip[:])
                    at2 = wrk.tile([128, 512], F32R, tag="vT",
                                   name=f"at2_{h}_{qb}")
                    nc.vector.tensor_mul(at2[:], atu[:], rb[:])
                    nc.scalar.dma_start(
                        attnT_spill[h * 128:(h + 1) * 128, qs:qs + 512], at2[:])

            # ============ Phase A ============
            for th in range(NTH):
                ts = th * THW
                hts = []
                for j in range(8):
                    t = one.tile([128, 4 * THW], F32R, tag=f"hT{j}")
                    half, jj = divmod(j, 4)
                    nc.sync.dma_start(
                        t[:, :1024], hT[th, half, :, jj * 2048:jj * 2048 + 1024])
                    nc.sync.dma_start(
                        t[:, 1024:], hT[th, half, :, jj * 2048 + 1024:(jj + 1) * 2048])
                    hts.append(t)
                cos_t = wrk.tile([HD, THW], F32, tag="cos")
                sin_t = wrk.tile([HD, THW], F32, tag="sin")
                nc.sync.dma_start(cos_t[:], cosT[:, ts:ts + THW])
                nc.sync.dma_start(sin_t[:], sinTr[:, ts:ts + THW])

                qT_lo = one.tile([128, 4 * 512], F32R, tag="qTbl")
                qT_hi = one.tile([128, 4 * 512], F32R, tag="qTbh")

                for cb in range(NCB):
                    if cb < 8:
                        wsrc, widx = wq, cb
                    elif cb < 10:
                        wsrc, widx = wk, cb - 8
                    else:
                        wsrc, widx = wv, cb - 10
                    ps = psp.tile([128, THW], F32, tag="pa")
                    for half in range(2):
                        w_sb = wrk.tile([128, (NCH // 2) * 128], F32R, tag="w")
                        nc.sync.dma_start(w_sb[:, :1024], wsrc[widx, half, :, :1024])
                        nc.sync.dma_start(w_sb[:, 1024:], wsrc[widx, half, :, 1024:])
                        for i in range(NCH // 2):
                            ic = half * (NCH // 2) + i
                            t = hts[ic // 4]
                            nc.tensor.matmul(
                                ps[:],
                                w_sb[:, i * 128:(i + 1) * 128],
                                t[:, (ic % 4) * THW:(ic % 4 + 1) * THW],
                                start=(half == 0 and i == 0),
                                stop=(half == 1 and i == NCH // 2 - 1),
                            )
                    if cb < 10:
                        m1 = wrk.tile([128, THW], F32, tag="m1")
                        nc.vector.tensor_mul(m1[:], ps[:], cos_t[:])
                        m2 = wrk.tile([128, THW], F32, tag="m2")
                        nc.vector.tensor_mul(m2[0:64, :], ps[64:128, :], sin_t[0:64, :])
                        nc.vector.tensor_mul(m2[64:128, :], ps[0:64, :], sin_t[64:128, :])
                        if cb < 8:
                            qdst = qT_lo if cb < 4 else qT_hi
                            nc.vector.tensor_add(
                                qdst[:, (cb % 4) * 512:(cb % 4 + 1) * 512],
                                m1[:], m2[:])
                        else:
                            kv = cb - 8
                            nc.vector.tensor_add(
                                kT_sb[:, kv * TOK + ts: kv * TOK + ts + THW],
                                m1[:], m2[:])
                    else:
                        kv = cb - 10
                        vT = wrk.tile([128, THW], F32R, tag="vT")
                        nc.scalar.copy(vT[:], ps[:])
                        for j in range(THW // 128):
                            tb = th * (THW // 128) + j
                            pt = psp.tile([128, 128], F32R, tag="aux")
                            nc.tensor.transpose(
                                pt[:], vT[:, j * 128:(j + 1) * 128], ident_sb[:])
                            nc.scalar.copy(
                                V_sb[:, tb * KCOLS + kv * 128:
                                     tb * KCOLS + (kv + 1) * 128],
                                pt[:])

                for qi, qt in ((0, qT_lo), (1, qT_hi)):
                    nc.scalar.dma_start(
                        qT_spill[qi * 512:(qi + 1) * 512, ts:ts + THW]
                        .rearrange("(i p) t -> p i t", p=128),
                        qt[:].rearrange("p (i t) -> p i t", i=4),
                    )

            for hp in range(0, QH, 2):
                for qb in range(4):
                    qts = []
                    for h in (hp, hp + 1):
                        qT_t = wrk.tile([128, 512], F32R, tag="qTs",
                                        name=f"qt_{h}_{qb}")
                        nc.sync.dma_start(
                            qT_t[:],
                            qT_spill[h * 128:(h + 1) * 128,
                                     qb * 512:(qb + 1) * 512])
                        qts.append(qT_t)
                    attention_group([hp, hp + 1], qb, qts)

            # ================= Phase C: o_proj partial =================
            ags = []
            for h in range(QH):
                a = one.tile([128, TOK], F32R, tag=f"hT{h}")
                nc.sync.dma_start(a[:], attnT_spill[h * 128:(h + 1) * 128, :])
                ags.append(a)
            for nb in range(D // 512):
                wo_sb = wrk.tile([128, QH * 512], F32R, tag="w")
                for hc in range(QH):
                    nc.sync.dma_start(
                        wo_sb[:, hc * 512:(hc + 1) * 512], wo[nb, hc])
                for qtb in range(TOK // 128):
                    o_ps = psp.tile([128, 512], F32, tag=["pa", "pb", "aux", "sum"][qtb % 4])
                    for hc in range(QH):
                        nc.tensor.matmul(
                            o_ps[:],
                            ags[hc][:, qtb * 128:(qtb + 1) * 128],
                            wo_sb[:, hc * 512:(hc + 1) * 512],
                            start=(hc == 0), stop=(hc == QH - 1))
                    ot = wrk.tile([128, 512], F32, tag="ot", bufs=4)
                    nc.scalar.copy(ot[:], o_ps[:])
                    nc.scalar.dma_start(
                        out[qtb * 128:(qtb + 1) * 128, nb * 512:(nb + 1) * 512],
                        ot[:])

    nc.compile()
    return nc


def _get_program(variant: str):
    if variant not in _PROGRAMS:
        if variant == "causal":
            _PROGRAMS[variant] = _build_causal()
        else:
            _PROGRAMS[variant] = _build_program(variant)
    return _PROGRAMS[variant]


def _detect_variant(mask: np.ndarray) -> str:
    m = mask.reshape(mask.shape[-2], mask.shape[-1])
    if not m.any():
        return "zero"
    causal = np.where(
        np.tril(np.ones((S, S), dtype=bool)), np.float32(0.0), np.float32(NEG))
    if np.array_equal(m, causal):
        return "causal"
    return "general"


def _tile_w(W, np_dt):  # [4096, C] -> [C//128, 2, 128, 2048]
    C = W.shape[1]
    return np.ascontiguousarray(
        W.reshape(2, 16, 128, C // 128, 128).transpose(3, 0, 2, 1, 4)
        .reshape(C // 128, 2, 128, 16 * 128).astype(np_dt))


def _kernel_causal(hidden_states, cos, sin, Wq, Wk, Wv, Wo, trace):
    nc = _get_program("causal")

    i = np.arange(128)[:, None]
    j = np.arange(128)[None, :]
    mask_diag = np.where(i <= j, np.float32(0.0),
                         np.float32(NEG / SCALE)).astype(np.float32)
    ones_sq = np.ones((128, 128), dtype=np.float32)

    per_batch = {}
    for b in range(B):
        sT = np.ascontiguousarray(sin[b].T)
        sinTr = np.concatenate([-sT[:64], sT[64:]], axis=0)
        hid = hidden_states[b]  # [2048, 4096]
        hT_t = np.ascontiguousarray(
            hid.reshape(4, 512, 2, 16, 128).transpose(0, 2, 4, 3, 1)
            .reshape(4, 2, 128, 16 * 512).astype(BF16_NP))
        per_batch[b] = (hT_t,
                        np.ascontiguousarray(cos[b].T.astype(BF16_NP)),
                        np.ascontiguousarray(sinTr.astype(BF16_NP)))

    in_maps = []
    for c in range(NCORES):
        b, g = divmod(c, 4)
        hT_t, cosT, sinTr = per_batch[b]
        wo_c = Wo[g * QCOLS:(g + 1) * QCOLS, :]  # [1024, 4096]
        wo_t = np.ascontiguousarray(
            wo_c.reshape(8, 128, 8, 512).transpose(2, 1, 0, 3)
            .reshape(8, 128, 8 * 512).astype(BF16_NP))
        wv_c = Wv[:, g * KCOLS:(g + 1) * KCOLS]  # [4096, 256]
        wvT_t = np.ascontiguousarray(
            wv_c.reshape(32, 128, 256).transpose(1, 0, 2)
            .reshape(128, 32 * 256).astype(BF16_NP))
        im = {
            "hT": hT_t,
            "wq": _tile_w(Wq[:, g * QCOLS:(g + 1) * QCOLS], BF16_NP),
            "wk": _tile_w(Wk[:, g * KCOLS:(g + 1) * KCOLS], BF16_NP),
            "wvT": wvT_t,
            "wo": wo_t,
            "cosT": cosT,
            "sinTr": sinTr,
            "ones_sq": ones_sq,
            "mask_diag": mask_diag,
        }
        in_maps.append(im)

    res = run_bass_kernel_spmd(nc, in_maps, core_ids=list(range(NCORES)),
                               trace=trace)
    if trace:
        print(f"HW exec time: {res.exec_time_ns} ns")

    out = np.empty((B, S, D), dtype=np.float32)
    for b in range(B):
        acc = np.zeros((S, D), dtype=np.float64)
        for g in range(4):
            acc += np.asarray(res.results[4 * b + g]["out"], dtype=np.float64)
        out[b] = acc.astype(np.float32)
    return out


def _kernel_legacy(variant, hidden_states, cos, sin, attention_mask,
                   Wq, Wk, Wv, Wo, trace):
    nc = _get_program(variant)

    ident = np.eye(128, dtype=np.float32)
    ones = np.ones((128, 1), dtype=np.float32)

    if variant == "general":
        m = attention_mask.reshape(S, S)
        maskT = np.ascontiguousarray(m.T / np.float32(SCALE))
    else:
        maskT = None

    per_batch = {}
    for b in range(B):
        sT = np.ascontiguousarray(sin[b].T)
        sinTr = np.concatenate([-sT[:64], sT[64:]], axis=0)
        hid = hidden_states[b]  # [2048, 4096]
        hT_t = np.ascontiguousarray(
            hid.reshape(4, 512, 2, 16, 128).transpose(0, 2, 4, 3, 1)
            .reshape(4, 2, 128, 16 * 512))
        per_batch[b] = (hT_t, np.ascontiguousarray(cos[b].T),
                        np.ascontiguousarray(sinTr))

    in_maps = []
    for c in range(NCORES):
        b, g = divmod(c, 4)
        hT_t, cosT, sinTr = per_batch[b]
        wo_c = Wo[g * QCOLS:(g + 1) * QCOLS, :]  # [1024, 4096]
        wo_t = np.ascontiguousarray(
            wo_c.reshape(8, 128, 8, 512).transpose(2, 0, 1, 3))
        im = {
            "hT": hT_t,
            "wq": _tile_w(Wq[:, g * QCOLS:(g + 1) * QCOLS], np.float32),
            "wk": _tile_w(Wk[:, g * KCOLS:(g + 1) * KCOLS], np.float32),
            "wv": _tile_w(Wv[:, g * KCOLS:(g + 1) * KCOLS], np.float32),
            "wo": wo_t,
            "cosT": cosT,
            "sinTr": sinTr,
            "ident": ident,
            "ones": ones,
        }
        if maskT is not None:
            im["maskT"] = maskT
        in_maps.append(im)

    res = run_bass_kernel_spmd(nc, in_maps, core_ids=list(range(NCORES)),
                               trace=trace)
    if trace:
        print(f"HW exec time: {res.exec_time_ns} ns")

    out = np.empty((B, S, D), dtype=np.float32)
    for b in range(B):
        acc = np.zeros((S, D), dtype=np.float64)
        for g in range(4):
            acc += res.results[4 * b + g]["out"]
        out[b] = acc.astype(np.float32)
    return out


def kernel(hidden_states, cos, sin, attention_mask, Wq, Wk, Wv, Wo):
    hidden_states = np.asarray(hidden_states, dtype=np.float32)
    cos = np.asarray(cos, dtype=np.float32)
    sin = np.asarray(sin, dtype=np.float32)
    attention_mask = np.asarray(attention_mask, dtype=np.float32)
    Wq = np.asarray(Wq, dtype=np.float32)
    Wk = np.asarray(Wk, dtype=np.float32)
    Wv = np.asarray(Wv, dtype=np.float32)
    Wo = np.asarray(Wo, dtype=np.float32)

    trace = bool(os.environ.get("KERNEL_TRACE"))
    variant = _detect_variant(attention_mask)
    if variant == "causal":
        return _kernel_causal(hidden_states, cos, sin, Wq, Wk, Wv, Wo, trace)
    return _kernel_legacy(variant, hidden_states, cos, sin, attention_mask,
                          Wq, Wk, Wv, Wo, trace)


# revision 20
# speedup vs baseline: 1.2199x; 1.1883x over previous
"""Mistral attention (B=2, S=2048, D=4096, H=32, KVH=8, HD=128) on 8 trn2 cores.

Sharding: core c -> (batch b = c//4, head-group g = c%4).
Each core computes q/k/v projections for its 8 Q heads + 2 KV heads of one
batch, RoPE, causal attention, and a row-parallel partial o_proj
[2048, 4096]. Host sums the 4 partials per batch. No collectives.

Causal fast path (v2):
- All matmul operands are bf16 (same 1 cycle/row PE rate as float32r, half
  the DMA bytes, no 4x small-free penalty on the causal diagonal strips).
- Weights are streamed per token block in bf16; hidden/cos/sin in bf16.
- Attention is transposed (scoresT[keys, qtok], keys on partitions).
  Softmax denominator: exp tiles are accumulated across key blocks on the
  vector engine, then one ones[128,128]-stationary matmul per (head,qblock)
  produces the denominator pre-broadcast across partitions; a fast
  approximate reciprocal replaces the slow DVE reciprocal.
- The causal mask is added only on the true-diagonal 128x128 tiles
  (in place, into the scores psum).
- o_proj for token block t is fused and its matmuls are interleaved as
  filler work into token block t+1's attention rounds, so the in-order PE
  queue always has independent work while waiting for exp results.
- Output partials are written in bf16 and summed on the host in fp64.
"""

import os
import sys

for _p in ("/opt/trn_rl_repo",):
    if _p not in sys.path:
        sys.path.insert(0, _p)

import numpy as np

import concourse.bass as bass
import concourse.tile as tile
from concourse import bacc, bass_isa, mybir
from concourse.bass_utils import run_bass_kernel_spmd

F32 = mybir.dt.float32
F32R = mybir.dt.float32r
BF16 = mybir.dt.bfloat16
EXP = mybir.ActivationFunctionType.Exp
BF16_NP = mybir.dt.np(mybir.dt.bfloat16)

B, S, D = 2, 2048, 4096
H, KVH, HD = 32, 8, 128
SCALE = HD ** -0.5
NCORES = 8

QH = H // 4              # 8 q heads per core
QCOLS = QH * HD          # 1024
KCOLS = (KVH // 4) * HD  # 256 (2 kv heads per core)
TOK = S

NEG = -1e9

_PROGRAMS = {}


def _build_causal():
    nc = bacc.Bacc("TRN2", target_bir_lowering=False, debug=False)

    hT = nc.dram_tensor("hT", [4, 2, 128, 16 * 512], BF16, kind="ExternalInput").ap()
    wq = nc.dram_tensor("wq", [8, 2, 128, 2048], BF16, kind="ExternalInput").ap()
    wk = nc.dram_tensor("wk", [2, 2, 128, 2048], BF16, kind="ExternalInput").ap()
    wvT = nc.dram_tensor("wvT", [128, 32 * 256], BF16, kind="ExternalInput").ap()
    wo = nc.dram_tensor("wo", [8, 128, 8 * 512], BF16, kind="ExternalInput").ap()
    cosT = nc.dram_tensor("cosT", [HD, TOK], BF16, kind="ExternalInput").ap()
    sinTr = nc.dram_tensor("sinTr", [HD, TOK], BF16, kind="ExternalInput").ap()
    ones_sq = nc.dram_tensor("ones_sq", [128, 128], F32R, kind="ExternalInput").ap()
    mask_diag = nc.dram_tensor("mask_diag", [128, 128], F32, kind="ExternalInput").ap()
    out = nc.dram_tensor("out", [TOK, D], BF16, kind="ExternalOutput").ap()

    with tile.TileContext(nc) as tc:
        with tc.tile_pool(name="per", bufs=1) as per, \
             tc.tile_pool(name="wrk", bufs=2) as wrk, \
             tc.tile_pool(name="ps", bufs=2, space="PSUM") as psp:

            mask_sb = per.tile([128, 128], F32, tag="mask")
            ones_sb = per.tile([128, 128], F32R, tag="ones")
            cos_sb = per.tile([HD, TOK], BF16, tag="cos")
            sin_sb = per.tile([HD, TOK], BF16, tag="sin")
            kT_sb = per.tile([HD, 2 * TOK], BF16, tag="kT")
            V_sb = per.tile([128, 16 * 256], BF16, tag="V")  # tb-major: tb*256+kv*128
            wv_sb = per.tile([128, 32 * 256], BF16, tag="wv")

            # ---- o_proj for token block th, yielded one PE-op at a time ----
            def oproj_gen(th, at2_t):
                wo_ts = {}

                def load(nb):
                    t = wrk.tile([128, 4096], BF16, tag="wo", bufs=4,
                                 name=f"wo_{th}_{nb}")
                    nc.sync.dma_start(t[:, :2048], wo[nb, :, :2048])
                    nc.sync.dma_start(t[:, 2048:], wo[nb, :, 2048:])
                    wo_ts[nb] = t

                load(0)
                load(1)
                for nb in range(8):
                    wo_t = wo_ts.pop(nb)
                    if nb + 2 < 8:
                        load(nb + 2)
                    for j in range(4):
                        po = psp.tile([128, 512], F32, tag="pc",
                                      name=f"po_{th}_{nb}_{j}")
                        for hc in range(8):
                            nc.tensor.matmul(
                                po[:],
                                at2_t[:, hc * 512 + j * 128: hc * 512 + j * 128 + 128],
                                wo_t[:, hc * 512:(hc + 1) * 512],
                                start=(hc == 0), stop=(hc == 7))
                            yield
                        ot = wrk.tile([128, 512], BF16, tag="ot", bufs=4,
                                      name=f"ot_{th}_{nb}_{j}")
                        nc.scalar.copy(ot[:], po[:])
                        nc.scalar.dma_start(
                            out[th * 512 + j * 128: th * 512 + (j + 1) * 128,
                                nb * 512:(nb + 1) * 512],
                            ot[:])
                        yield

            filler = [iter(())]

            def fill(n):
                for _ in range(n):
                    try:
                        next(filler[0])
                    except StopIteration:
                        return

            def attention_group(th, hs, qT_t, at2_t, fin_q):
                """Transposed causal attention for q heads hs, query block th."""
                nkb = 4 * th + 4
                att_ps = [psp.tile([128, 512], F32, tag="att",
                                   name=f"att_{h}_{th}") for h in hs]
                accs = [wrk.tile([128, 512], F32R, tag="acc", bufs=4,
                                 name=f"acc_{h}_{th}") for h in hs]
                pend = [None, None]
                pend_acc = [None, None]

                def emit_av(i, kb, expT, co):
                    h = hs[i]
                    kv = h // 4
                    nc.tensor.matmul(
                        att_ps[i][:, co:],
                        V_sb[:, kb * 256 + kv * 128: kb * 256 + (kv + 1) * 128],
                        expT[:, co:],
                        start=(kb == 0), stop=(kb == nkb - 1))

                def emit_acc(i, kb, expT, co):
                    if kb == 0:
                        nc.vector.tensor_scalar_add(accs[i][:], expT[:], 0.0)
                    else:
                        nc.vector.tensor_add(
                            accs[i][:, co:], accs[i][:, co:], expT[:, co:])

                for kb in range(nkb):
                    o = kb - 4 * th
                    co = o * 128 if o > 0 else 0
                    exps = []
                    for i, h in enumerate(hs):
                        kv = h // 4
                        s_ps = psp.tile([128, 512], F32, tag="pb",
                                        name=f"s_{h}_{th}_{kb}")
                        nc.tensor.matmul(
                            s_ps[:, co:],
                            kT_sb[:, kv * TOK + kb * 128: kv * TOK + (kb + 1) * 128],
                            qT_t[:, h * 512 + co: (h + 1) * 512],
                            start=True, stop=True)
                        if o >= 0:
                            nc.vector.tensor_add(
                                s_ps[:, co:co + 128], s_ps[:, co:co + 128],
                                mask_sb[:])
                        expT = wrk.tile([128, 512], BF16, tag="expT", bufs=4,
                                        name=f"exp_{h}_{th}_{kb}")
                        nc.scalar.activation(
                            expT[:, co:], s_ps[:, co:], EXP, scale=float(SCALE))
                        exps.append(expT)
                    for i in range(2):
                        if pend[i] is not None:
                            emit_av(i, *pend[i])
                        pend[i] = (kb, exps[i], co)
                    for i in range(2):
                        if pend_acc[i] is not None:
                            emit_acc(i, *pend_acc[i])
                        pend_acc[i] = (kb, exps[i], co)
                    if kb == 2 and fin_q:
                        fin_q.pop(0)()
                    fill(8)
                for i in range(2):
                    emit_av(i, *pend[i])
                    emit_acc(i, *pend_acc[i])

                def finisher():
                    for i, h in enumerate(hs):
                        den_ps = psp.tile([128, 512], F32, tag="pc",
                                          name=f"den_{h}_{th}")
                        nc.tensor.matmul(den_ps[:], ones_sb[:], accs[i][:],
                                         start=True, stop=True)
                        rb = wrk.tile([128, 512], F32, tag="rb",
                                      name=f"rb_{h}_{th}")
                        nc.vector.reciprocal_approx_fast(rb[:], den_ps[:])
                        nc.vector.tensor_mul(
                            at2_t[:, h * 512:(h + 1) * 512], att_ps[i][:], rb[:])
                return finisher

            for th in range(4):
                ts = th * 512
                hts = []
                for jj in range(8):
                    t = wrk.tile([128, 2048], BF16, tag="hT", bufs=11,
                                 name=f"hT_{th}_{jj}")
                    half, j4 = divmod(jj, 4)
                    nc.sync.dma_start(
                        t[:, :1024], hT[th, half, :, j4 * 2048: j4 * 2048 + 1024])
                    nc.sync.dma_start(
                        t[:, 1024:], hT[th, half, :, j4 * 2048 + 1024:(j4 + 1) * 2048])
                    hts.append(t)
                    if th == 0 and jj == 0:
                        # first weight block right behind the first hidden
                        # tile so the PE can start ~4us in, ahead of the
                        # bulk of the startup DMA traffic
                        w_first = wrk.tile([128, 4096], BF16, tag="w", bufs=3,
                                           name="w_k0_0")
                        for q4 in range(4):
                            nc.sync.dma_start(
                                w_first[:, q4 * 1024:(q4 + 1) * 1024],
                                wk[0, q4 // 2, :, (q4 % 2) * 1024:
                                   (q4 % 2 + 1) * 1024])
                    if th == 0 and jj == 3:
                        # cos/sin next: needed by rope(k0) well before the
                        # later hidden tiles are consumed
                        nc.sync.dma_start(cos_sb[:], cosT[:])
                        nc.sync.dma_start(sin_sb[:], sinTr[:])
                if th == 0:
                    nc.sync.dma_start(mask_sb[:], mask_diag[:])
                    nc.sync.dma_start(ones_sb[:], ones_sq[:])
                    for q4 in range(4):
                        nc.sync.dma_start(
                            wv_sb[:, q4 * 2048:(q4 + 1) * 2048],
                            wvT[:, q4 * 2048:(q4 + 1) * 2048])
                qT_t = wrk.tile([128, 8 * 512], BF16, tag="qT", name=f"qT_{th}")
                at2_t = wrk.tile([128, 8 * 512], BF16, tag="at2", name=f"at2_{th}")

                def proj_block(wsrc, idx, kind, pre=None):
                    if pre is not None:
                        w_t = pre
                    else:
                        w_t = wrk.tile([128, 4096], BF16, tag="w", bufs=3,
                                       name=f"w_{kind}{idx}_{th}")
                        nc.sync.dma_start(w_t[:, :2048], wsrc[idx, 0])
                        nc.sync.dma_start(w_t[:, 2048:], wsrc[idx, 1])
                    ps = psp.tile([128, 512], F32, tag="pa",
                                  name=f"ps_{kind}{idx}_{th}")
                    for ic in range(32):
                        half, i = divmod(ic, 16)
                        nc.tensor.matmul(
                            ps[:],
                            w_t[:, half * 2048 + i * 128: half * 2048 + (i + 1) * 128],
                            hts[ic // 4][:, (ic % 4) * 512: (ic % 4 + 1) * 512],
                            start=(ic == 0), stop=(ic == 31))
                    return ps

                def rope(ps, dst, kind, idx):
                    m1 = wrk.tile([128, 512], F32, tag="m1",
                                  name=f"m1_{kind}{idx}_{th}")
                    nc.vector.tensor_mul(m1[:], ps[:], cos_sb[:, ts:ts + 512])
                    m2 = wrk.tile([128, 512], F32, tag="m2",
                                  name=f"m2_{kind}{idx}_{th}")
                    nc.vector.tensor_mul(
                        m2[0:64, :], ps[64:128, :], sin_sb[0:64, ts:ts + 512])
                    nc.vector.tensor_mul(
                        m2[64:128, :], ps[0:64, :], sin_sb[64:128, ts:ts + 512])
                    nc.vector.tensor_add(dst, m1[:], m2[:])

                for kv in range(2):
                    ps = proj_block(wk, kv, "k",
                                    pre=w_first if (th == 0 and kv == 0) else None)
                    rope(ps, kT_sb[:, kv * TOK + ts: kv * TOK + ts + 512], "k", kv)
                    fill(4)
                fin_q = []
                for hp in range(0, QH, 2):
                    for h in (hp, hp + 1):
                        ps = proj_block(wq, h, "q")
                        rope(ps, qT_t[:, h * 512:(h + 1) * 512], "q", h)
                        fill(4)
                    if hp == 0:
                        # V^T: out[tok, vdim] with hidden chunks stationary
                        for j in range(4):
                            tb = th * 4 + j
                            pv = psp.tile([128, 256], F32, tag="pa",
                                          name=f"pv_{th}_{j}")
                            for c in range(32):
                                nc.tensor.matmul(
                                    pv[:],
                                    hts[c // 4][:, (c % 4) * 512 + j * 128:
                                                (c % 4) * 512 + (j + 1) * 128],
                                    wv_sb[:, c * 256:(c + 1) * 256],
                                    start=(c == 0), stop=(c == 31))
                            nc.scalar.copy(V_sb[:, tb * 256:(tb + 1) * 256], pv[:])
                            fill(2)
                    fin = attention_group(th, [hp, hp + 1], qT_t, at2_t, fin_q)
                    fin_q.append(fin)
                while fin_q:
                    fin_q.pop(0)()
                    fill(8)

                # drain previous block's o_proj, then queue this block's
                fill(1 << 30)
                filler[0] = oproj_gen(th, at2_t)
            fill(1 << 30)

    nc.compile()
    return nc


def _build_program(variant: str):
    """variant: 'zero' | 'general' (legacy fp32r path, kept as fallback)"""
    nc = bacc.Bacc("TRN2", target_bir_lowering=False, debug=False)

    hT = nc.dram_tensor("hT", [4, 2, 128, 16 * 512], F32R, kind="ExternalInput").ap()
    wq = nc.dram_tensor("wq", [8, 2, 128, 16 * 128], F32R, kind="ExternalInput").ap()
    wk = nc.dram_tensor("wk", [2, 2, 128, 16 * 128], F32R, kind="ExternalInput").ap()
    wv = nc.dram_tensor("wv", [2, 2, 128, 16 * 128], F32R, kind="ExternalInput").ap()
    wo = nc.dram_tensor("wo", [8, 8, 128, 512], F32R, kind="ExternalInput").ap()
    cosT = nc.dram_tensor("cosT", [HD, TOK], F32, kind="ExternalInput").ap()
    sinTr = nc.dram_tensor("sinTr", [HD, TOK], F32, kind="ExternalInput").ap()
    ident = nc.dram_tensor("ident", [128, 128], F32R, kind="ExternalInput").ap()
    ones = nc.dram_tensor("ones", [128, 1], F32R, kind="ExternalInput").ap()
    if variant == "general":
        maskT = nc.dram_tensor("maskT", [S, S], F32, kind="ExternalInput").ap()
    else:
        maskT = None
    out = nc.dram_tensor("out", [TOK, D], F32, kind="ExternalOutput").ap()

    attnT_spill = nc.dram_tensor("attnT_spill", [QCOLS, TOK], F32R).ap()
    qT_spill = nc.dram_tensor("qT_spill", [QCOLS, TOK], F32R).ap()

    NTH = 4
    THW = TOK // NTH         # 512
    NCH = D // 128           # 32 contraction chunks
    NCB = (QCOLS + 2 * KCOLS) // 128  # 12: 0-7 q, 8-9 k, 10-11 v

    with tile.TileContext(nc) as tc:
        with tc.tile_pool(name="per", bufs=1) as per, \
             tc.tile_pool(name="wrk", bufs=2) as wrk, \
             tc.tile_pool(name="one", bufs=1) as one, \
             tc.tile_pool(name="ps", bufs=2, space="PSUM") as psp:

            ident_sb = per.tile([128, 128], F32R, tag="ident")
            ones_sb = per.tile([128, 1], F32R, tag="ones")
            kT_sb = per.tile([HD, 2 * TOK], F32R, tag="kT")
            V_sb = per.tile([128, (TOK // 128) * KCOLS], F32R, tag="V")
            nc.sync.dma_start(ident_sb[:], ident[:])
            nc.sync.dma_start(ones_sb[:], ones[:])

            def attention_group(hs, qb, qT_aps):
                qs = qb * 512
                nkb = TOK // 128
                n = len(hs)
                att_ps = [psp.tile([128, 512], F32, tag="aux", name=f"att_{h}_{qb}")
                          for h in hs]
                sum_ps = [psp.tile([1, 512], F32, tag="sum", name=f"sum_{h}_{qb}")
                          for h in hs]

                def emit_av(i, kb, expT, co):
                    h = hs[i]
                    kv = h // (QH // 2)
                    nc.tensor.matmul(
                        att_ps[i][:, co:],
                        V_sb[:, kb * KCOLS + kv * 128: kb * KCOLS + (kv + 1) * 128],
                        expT[:, co:],
                        start=(kb == 0), stop=(kb == nkb - 1))
                    nc.tensor.matmul(
                        sum_ps[i][:, co:], ones_sb[:], expT[:, co:],
                        start=(kb == 0), stop=(kb == nkb - 1))

                pend = [None] * n
                for kb in range(nkb):
                    co = 0
                    exps = []
                    for i, h in enumerate(hs):
                        kv = h // (QH // 2)
                        s_ps = psp.tile([128, 512], F32, tag="pb",
                                        name=f"s_{h}_{qb}_{kb}")
                        nc.tensor.matmul(
                            s_ps[:, co:],
                            kT_sb[:, kv * TOK + kb * 128: kv * TOK + (kb + 1) * 128],
                            qT_aps[i][:, co:],
                            start=True, stop=True)
                        exp_in = s_ps
                        if variant == "general":
                            mt = wrk.tile([128, 512], F32, tag="mt",
                                          name=f"mt_{h}_{qb}_{kb}")
                            nc.sync.dma_start(
                                mt[:], maskT[kb * 128:(kb + 1) * 128, qs:qs + 512])
                            msk = wrk.tile([128, 512], F32, tag="m1",
                                           name=f"mskg_{h}_{qb}_{kb}")
                            nc.vector.tensor_add(msk[:], s_ps[:], mt[:])
                            exp_in = msk
                        expT = wrk.tile([128, 512], F32R, tag="expT", bufs=4,
                                        name=f"exp_{h}_{qb}_{kb}")
                        nc.scalar.activation(
                            expT[:, co:], exp_in[:, co:], EXP, scale=float(SCALE))
                        exps.append(expT)
                    for i in range(n):
                        if pend[i] is not None:
                            emit_av(i, *pend[i])
                        pend[i] = (kb, exps[i], co)
                for i in range(n):
                    emit_av(i, *pend[i])
                for i, h in enumerate(hs):
                    atu = wrk.tile([128, 512], F32, tag="atu",
                                   name=f"atu_{h}_{qb}")
                    nc.scalar.copy(atu[:], att_ps[i][:])
                    recip = wrk.tile([1, 512], F32, tag="rcp",
                                     name=f"rcp_{h}_{qb}")
                    nc.vector.reciprocal(recip[:], sum_ps[i][:])
                    rb = wrk.tile([128, 512], F32, tag="m2",
                                  name=f"rb_{h}_{qb}")
                    nc.gpsimd.partition_broadcast(rb[:], recip[:])
                    at2 = wrk.tile([128, 512], F32R, tag="vT",
                                   name=f"at2_{h}_{qb}")
                    nc.vector.tensor_mul(at2[:], atu[:], rb[:])
                    nc.scalar.dma_start(
                        attnT_spill[h * 128:(h + 1) * 128, qs:qs + 512], at2[:])

            # ============ Phase A ============
            for th in range(NTH):
                ts = th * THW
                hts = []
                for j in range(8):
                    t = one.tile([128, 4 * THW], F32R, tag=f"hT{j}")
                    half, jj = divmod(j, 4)
                    nc.sync.dma_start(
                        t[:, :1024], hT[th, half, :, jj * 2048:jj * 2048 + 1024])
                    nc.sync.dma_start(
                        t[:, 1024:], hT[th, half, :, jj * 2048 + 1024:(jj + 1) * 2048])
                    hts.append(t)
                cos_t = wrk.tile([HD, THW], F32, tag="cos")
                sin_t = wrk.tile([HD, THW], F32, tag="sin")
                nc.sync.dma_start(cos_t[:], cosT[:, ts:ts + THW])
                nc.sync.dma_start(sin_t[:], sinTr[:, ts:ts + THW])

                qT_lo = one.tile([128, 4 * 512], F32R, tag="qTbl")
                qT_hi = one.tile([128, 4 * 512], F32R, tag="qTbh")

                for cb in range(NCB):
                    if cb < 8:
                        wsrc, widx = wq, cb
                    elif cb < 10:
                        wsrc, widx = wk, cb - 8
                    else:
                        wsrc, widx = wv, cb - 10
                    ps = psp.tile([128, THW], F32, tag="pa")
                    for half in range(2):
                        w_sb = wrk.tile([128, (NCH // 2) * 128], F32R, tag="w")
                        nc.sync.dma_start(w_sb[:, :1024], wsrc[widx, half, :, :1024])
                        nc.sync.dma_start(w_sb[:, 1024:], wsrc[widx, half, :, 1024:])
                        for i in range(NCH // 2):
                            ic = half * (NCH // 2) + i
                            t = hts[ic // 4]
                            nc.tensor.matmul(
                                ps[:],
                                w_sb[:, i * 128:(i + 1) * 128],
                                t[:, (ic % 4) * THW:(ic % 4 + 1) * THW],
                                start=(half == 0 and i == 0),
                                stop=(half == 1 and i == NCH // 2 - 1),
                            )
                    if cb < 10:
                        m1 = wrk.tile([128, THW], F32, tag="m1")
                        nc.vector.tensor_mul(m1[:], ps[:], cos_t[:])
                        m2 = wrk.tile([128, THW], F32, tag="m2")
                        nc.vector.tensor_mul(m2[0:64, :], ps[64:128, :], sin_t[0:64, :])
                        nc.vector.tensor_mul(m2[64:128, :], ps[0:64, :], sin_t[64:128, :])
                        if cb < 8:
                            qdst = qT_lo if cb < 4 else qT_hi
                            nc.vector.tensor_add(
                                qdst[:, (cb % 4) * 512:(cb % 4 + 1) * 512],
                                m1[:], m2[:])
                        else:
                            kv = cb - 8
                            nc.vector.tensor_add(
                                kT_sb[:, kv * TOK + ts: kv * TOK + ts + THW],
                                m1[:], m2[:])
                    else:
                        kv = cb - 10
                        vT = wrk.tile([128, THW], F32R, tag="vT")
                        nc.scalar.copy(vT[:], ps[:])
                        for j in range(THW // 128):
                            tb = th * (THW // 128) + j
                            pt = psp.tile([128, 128], F32R, tag="aux")
                            nc.tensor.transpose(
                                pt[:], vT[:, j * 128:(j + 1) * 128], ident_sb[:])
                            nc.scalar.copy(
                                V_sb[:, tb * KCOLS + kv * 128:
                                     tb * KCOLS + (kv + 1) * 128],
                                pt[:])

                for qi, qt in ((0, qT_lo), (1, qT_hi)):
                    nc.scalar.dma_start(
                        qT_spill[qi * 512:(qi + 1) * 512, ts:ts + THW]
                        .rearrange("(i p) t -> p i t", p=128),
                        qt[:].rearrange("p (i t) -> p i t", i=4),
                    )

            for hp in range(0, QH, 2):
                for qb in range(4):
                    qts = []
                    for h in (hp, hp + 1):
                        qT_t = wrk.tile([128, 512], F32R, tag="qTs",
                                        name=f"qt_{h}_{qb}")
                        nc.sync.dma_start(
                            qT_t[:],
                            qT_spill[h * 128:(h + 1) * 128,
                                     qb * 512:(qb + 1) * 512])
                        qts.append(qT_t)
                    attention_group([hp, hp + 1], qb, qts)

            # ================= Phase C: o_proj partial =================
            ags = []
            for h in range(QH):
                a = one.tile([128, TOK], F32R, tag=f"hT{h}")
                nc.sync.dma_start(a[:], attnT_spill[h * 128:(h + 1) * 128, :])
                ags.append(a)
            for nb in range(D // 512):
                wo_sb = wrk.tile([128, QH * 512], F32R, tag="w")
                for hc in range(QH):
                    nc.sync.dma_start(
                        wo_sb[:, hc * 512:(hc + 1) * 512], wo[nb, hc])
                for qtb in range(TOK // 128):
                    o_ps = psp.tile([128, 512], F32, tag=["pa", "pb", "aux", "sum"][qtb % 4])
                    for hc in range(QH):
                        nc.tensor.matmul(
                            o_ps[:],
                            ags[hc][:, qtb * 128:(qtb + 1) * 128],
                            wo_sb[:, hc * 512:(hc + 1) * 512],
                            start=(hc == 0), stop=(hc == QH - 1))
                    ot = wrk.tile([128, 512], F32, tag="ot", bufs=4)
                    nc.scalar.copy(ot[:], o_ps[:])
                    nc.scalar.dma_start(
                        out[qtb * 128:(qtb + 1) * 128, nb * 512:(nb + 1) * 512],
                        ot[:])

    nc.compile()
    return nc


def _get_program(variant: str):
    if variant not in _PROGRAMS:
        if variant == "causal":
            _PROGRAMS[variant] = _build_causal()
        else:
            _PROGRAMS[variant] = _build_program(variant)
    return _PROGRAMS[variant]


def _detect_variant(mask: np.ndarray) -> str:
    m = mask.reshape(mask.shape[-2], mask.shape[-1])
    if not m.any():
        return "zero"
    causal = np.where(
        np.tril(np.ones((S, S), dtype=bool)), np.float32(0.0), np.float32(NEG))
    if np.array_equal(m, causal):
        return "causal"
    return "general"


def _tile_w(W, np_dt):  # [4096, C] -> [C//128, 2, 128, 2048]
    C = W.shape[1]
    return np.ascontiguousarray(
        W.reshape(2, 16, 128, C // 128, 128).transpose(3, 0, 2, 1, 4)
        .reshape(C // 128, 2, 128, 16 * 128).astype(np_dt))


def _kernel_causal(hidden_states, cos, sin, Wq, Wk, Wv, Wo, trace):
    nc = _get_program("causal")

    i = np.arange(128)[:, None]
    j = np.arange(128)[None, :]
    mask_diag = np.where(i <= j, np.float32(0.0),
                         np.float32(NEG / SCALE)).astype(np.float32)
    ones_sq = np.ones((128, 128), dtype=np.float32)

    per_batch = {}
    for b in range(B):
        sT = np.ascontiguousarray(sin[b].T)
        sinTr = np.concatenate([-sT[:64], sT[64:]], axis=0)
        hid = hidden_states[b]  # [2048, 4096]
        hT_t = np.ascontiguousarray(
            hid.reshape(4, 512, 2, 16, 128).transpose(0, 2, 4, 3, 1)
            .reshape(4, 2, 128, 16 * 512).astype(BF16_NP))
        per_batch[b] = (hT_t,
                        np.ascontiguousarray(cos[b].T.astype(BF16_NP)),
                        np.ascontiguousarray(sinTr.astype(BF16_NP)))

    in_maps = []
    for c in range(NCORES):
        b, g = divmod(c, 4)
        hT_t, cosT, sinTr = per_batch[b]
        wo_c = Wo[g * QCOLS:(g + 1) * QCOLS, :]  # [1024, 4096]
        wo_t = np.ascontiguousarray(
            wo_c.reshape(8, 128, 8, 512).transpose(2, 1, 0, 3)
            .reshape(8, 128, 8 * 512).astype(BF16_NP))
        wv_c = Wv[:, g * KCOLS:(g + 1) * KCOLS]  # [4096, 256]
        wvT_t = np.ascontiguousarray(
            wv_c.reshape(32, 128, 256).transpose(1, 0, 2)
            .reshape(128, 32 * 256).astype(BF16_NP))
        im = {
            "hT": hT_t,
            "wq": _tile_w(Wq[:, g * QCOLS:(g + 1) * QCOLS], BF16_NP),
            "wk": _tile_w(Wk[:, g * KCOLS:(g + 1) * KCOLS], BF16_NP),
            "wvT": wvT_t,
            "wo": wo_t,
            "cosT": cosT,
            "sinTr": sinTr,
            "ones_sq": ones_sq,
            "mask_diag": mask_diag,
        }
        in_maps.append(im)

    res = run_bass_kernel_spmd(nc, in_maps, core_ids=list(range(NCORES)),
                               trace=trace)
    if trace:
        print(f"HW exec time: {res.exec_time_ns} ns")

    out = np.empty((B, S, D), dtype=np.float32)
    for b in range(B):
        acc = np.zeros((S, D), dtype=np.float64)
        for g in range(4):
            acc += np.asarray(res.results[4 * b + g]["out"], dtype=np.float64)
        out[b] = acc.astype(np.float32)
    return out


def _kernel_legacy(variant, hidden_states, cos, sin, attention_mask,
                   Wq, Wk, Wv, Wo, trace):
    nc = _get_program(variant)

    ident = np.eye(128, dtype=np.float32)
    ones = np.ones((128, 1), dtype=np.float32)

    if variant == "general":
        m = attention_mask.reshape(S, S)
        maskT = np.ascontiguousarray(m.T / np.float32(SCALE))
    else:
        maskT = None

    per_batch = {}
    for b in range(B):
        sT = np.ascontiguousarray(sin[b].T)
        sinTr = np.concatenate([-sT[:64], sT[64:]], axis=0)
        hid = hidden_states[b]  # [2048, 4096]
        hT_t = np.ascontiguousarray(
            hid.reshape(4, 512, 2, 16, 128).transpose(0, 2, 4, 3, 1)
            .reshape(4, 2, 128, 16 * 512))
        per_batch[b] = (hT_t, np.ascontiguousarray(cos[b].T),
                        np.ascontiguousarray(sinTr))

    in_maps = []
    for c in range(NCORES):
        b, g = divmod(c, 4)
        hT_t, cosT, sinTr = per_batch[b]
        wo_c = Wo[g * QCOLS:(g + 1) * QCOLS, :]  # [1024, 4096]
        wo_t = np.ascontiguousarray(
            wo_c.reshape(8, 128, 8, 512).transpose(2, 0, 1, 3))
        im = {
            "hT": hT_t,
            "wq": _tile_w(Wq[:, g * QCOLS:(g + 1) * QCOLS], np.float32),
            "wk": _tile_w(Wk[:, g * KCOLS:(g + 1) * KCOLS], np.float32),
            "wv": _tile_w(Wv[:, g * KCOLS:(g + 1) * KCOLS], np.float32),
            "wo": wo_t,
            "cosT": cosT,
            "sinTr": sinTr,
            "ident": ident,
            "ones": ones,
        }
        if maskT is not None:
            im["maskT"] = maskT
        in_maps.append(im)

    res = run_bass_kernel_spmd(nc, in_maps, core_ids=list(range(NCORES)),
                               trace=trace)
    if trace:
        print(f"HW exec time: {res.exec_time_ns} ns")

    out = np.empty((B, S, D), dtype=np.float32)
    for b in range(B):
        acc = np.zeros((S, D), dtype=np.float64)
        for g in range(4):
            acc += res.results[4 * b + g]["out"]
        out[b] = acc.astype(np.float32)
    return out


def kernel(hidden_states, cos, sin, attention_mask, Wq, Wk, Wv, Wo):
    hidden_states = np.asarray(hidden_states, dtype=np.float32)
    cos = np.asarray(cos, dtype=np.float32)
    sin = np.asarray(sin, dtype=np.float32)
    attention_mask = np.asarray(attention_mask, dtype=np.float32)
    Wq = np.asarray(Wq, dtype=np.float32)
    Wk = np.asarray(Wk, dtype=np.float32)
    Wv = np.asarray(Wv, dtype=np.float32)
    Wo = np.asarray(Wo, dtype=np.float32)

    trace = bool(os.environ.get("KERNEL_TRACE"))
    variant = _detect_variant(attention_mask)
    if variant == "causal":
        return _kernel_causal(hidden_states, cos, sin, Wq, Wk, Wv, Wo, trace)
    return _kernel_legacy(variant, hidden_states, cos, sin, attention_mask,
                          Wq, Wk, Wv, Wo, trace)


# revision 25
# speedup vs baseline: 1.2254x; 1.0045x over previous
"""Mistral attention (B=2, S=2048, D=4096, H=32, KVH=8, HD=128) on 8 trn2 cores.

Sharding: core c -> (batch b = c//4, head-group g = c%4).
Each core computes q/k/v projections for its 8 Q heads + 2 KV heads of one
batch, RoPE, causal attention, and a row-parallel partial o_proj
[2048, 4096]. Host sums the 4 partials per batch. No collectives.

Causal fast path (v2):
- All matmul operands are bf16 (same 1 cycle/row PE rate as float32r, half
  the DMA bytes, no 4x small-free penalty on the causal diagonal strips).
- Weights are streamed per token block in bf16; hidden/cos/sin in bf16.
- Attention is transposed (scoresT[keys, qtok], keys on partitions).
  Softmax denominator: exp tiles are accumulated across key blocks on the
  vector engine, then one ones[128,128]-stationary matmul per (head,qblock)
  produces the denominator pre-broadcast across partitions; a fast
  approximate reciprocal replaces the slow DVE reciprocal.
- The causal mask is added only on the true-diagonal 128x128 tiles
  (in place, into the scores psum).
- o_proj for token block t is fused and its matmuls are interleaved as
  filler work into token block t+1's attention rounds, so the in-order PE
  queue always has independent work while waiting for exp results.
- Output partials are written in bf16 and summed on the host in fp64.
"""

import os
import sys

for _p in ("/opt/trn_rl_repo",):
    if _p not in sys.path:
        sys.path.insert(0, _p)

import numpy as np

import concourse.bass as bass
import concourse.tile as tile
from concourse import bacc, bass_isa, mybir
from concourse.bass_utils import run_bass_kernel_spmd

F32 = mybir.dt.float32
F32R = mybir.dt.float32r
BF16 = mybir.dt.bfloat16
EXP = mybir.ActivationFunctionType.Exp
BF16_NP = mybir.dt.np(mybir.dt.bfloat16)

B, S, D = 2, 2048, 4096
H, KVH, HD = 32, 8, 128
SCALE = HD ** -0.5
NCORES = 8

QH = H // 4              # 8 q heads per core
QCOLS = QH * HD          # 1024
KCOLS = (KVH // 4) * HD  # 256 (2 kv heads per core)
TOK = S

NEG = -1e9

_PROGRAMS = {}


def _build_causal():
    nc = bacc.Bacc("TRN2", target_bir_lowering=False, debug=False)

    hT = nc.dram_tensor("hT", [4, 2, 128, 16 * 512], BF16, kind="ExternalInput").ap()
    wq = nc.dram_tensor("wq", [8, 2, 128, 2048], BF16, kind="ExternalInput").ap()
    wk = nc.dram_tensor("wk", [2, 2, 128, 2048], BF16, kind="ExternalInput").ap()
    wvT = nc.dram_tensor("wvT", [128, 32 * 256], BF16, kind="ExternalInput").ap()
    wo = nc.dram_tensor("wo", [8, 128, 8 * 512], BF16, kind="ExternalInput").ap()
    cosT = nc.dram_tensor("cosT", [HD, TOK], BF16, kind="ExternalInput").ap()
    sinTr = nc.dram_tensor("sinTr", [HD, TOK], BF16, kind="ExternalInput").ap()
    ones_sq = nc.dram_tensor("ones_sq", [128, 128], F32R, kind="ExternalInput").ap()
    mask_diag = nc.dram_tensor("mask_diag", [128, 128], F32, kind="ExternalInput").ap()
    out = nc.dram_tensor("out", [TOK, D], BF16, kind="ExternalOutput").ap()
    gate_spill = nc.dram_tensor("gate_spill", [1, 8], BF16).ap()

    with tile.TileContext(nc) as tc:
        with tc.tile_pool(name="per", bufs=1) as per, \
             tc.tile_pool(name="wrk", bufs=2) as wrk, \
             tc.tile_pool(name="ps", bufs=2, space="PSUM") as psp:

            mask_sb = per.tile([128, 128], F32, tag="mask")
            ones_sb = per.tile([128, 128], F32R, tag="ones")
            cos_sb = per.tile([HD, TOK], BF16, tag="cos")
            sin_sb = per.tile([HD, TOK], BF16, tag="sin")
            kT_sb = per.tile([HD, 2 * TOK], BF16, tag="kT")
            V_sb = per.tile([128, 16 * 256], BF16, tag="V")  # tb-major: tb*256+kv*128
            wv_sb = per.tile([128, 32 * 256], BF16, tag="wv")

            # ---- o_proj for token block th, yielded one PE-op at a time ----
            def oproj_gen(th, at2_t):
                wo_ts = {}

                def load(nb):
                    t = wrk.tile([128, 4096], BF16, tag="wo", bufs=4,
                                 name=f"wo_{th}_{nb}")
                    nc.sync.dma_start(t[:, :2048], wo[nb, :, :2048])
                    nc.sync.dma_start(t[:, 2048:], wo[nb, :, 2048:])
                    wo_ts[nb] = t

                load(0)
                load(1)
                for nb in range(8):
                    wo_t = wo_ts.pop(nb)
                    if nb + 2 < 8:
                        load(nb + 2)
                    for j in range(4):
                        po = psp.tile([128, 512], F32, tag="pc",
                                      name=f"po_{th}_{nb}_{j}")
                        for hc in range(8):
                            nc.tensor.matmul(
                                po[:],
                                at2_t[:, hc * 512 + j * 128: hc * 512 + j * 128 + 128],
                                wo_t[:, hc * 512:(hc + 1) * 512],
                                start=(hc == 0), stop=(hc == 7))
                            yield
                        ot = wrk.tile([128, 512], BF16, tag="ot", bufs=4,
                                      name=f"ot_{th}_{nb}_{j}")
                        nc.scalar.copy(ot[:], po[:])
                        nc.scalar.dma_start(
                            out[th * 512 + j * 128: th * 512 + (j + 1) * 128,
                                nb * 512:(nb + 1) * 512],
                            ot[:])
                        yield

            filler = [iter(())]

            def fill(n):
                for _ in range(n):
                    try:
                        next(filler[0])
                    except StopIteration:
                        return

            def attention_group(th, hs, qT_t, at2_t, fin_q, fill_n=8):
                """Transposed causal attention for q heads hs, query block th."""
                nkb = 4 * th + 4
                att_ps = [psp.tile([128, 512], F32, tag="att",
                                   name=f"att_{h}_{th}") for h in hs]
                accs = [wrk.tile([128, 512], F32R, tag="acc", bufs=4,
                                 name=f"acc_{h}_{th}") for h in hs]
                pend = [None, None]
                pend_acc = [None, None]

                def emit_av(i, kb, expT, co):
                    h = hs[i]
                    kv = h // 4
                    nc.tensor.matmul(
                        att_ps[i][:, co:],
                        V_sb[:, kb * 256 + kv * 128: kb * 256 + (kv + 1) * 128],
                        expT[:, co:],
                        start=(kb == 0), stop=(kb == nkb - 1))

                def emit_acc(i, kb, expT, co):
                    if kb == 0:
                        nc.vector.tensor_scalar_add(accs[i][:], expT[:], 0.0)
                    else:
                        nc.vector.tensor_add(
                            accs[i][:, co:], accs[i][:, co:], expT[:, co:])

                for kb in range(nkb):
                    o = kb - 4 * th
                    co = o * 128 if o > 0 else 0
                    exps = []
                    for i, h in enumerate(hs):
                        kv = h // 4
                        s_ps = psp.tile([128, 512], F32, tag="pb",
                                        name=f"s_{h}_{th}_{kb}")
                        nc.tensor.matmul(
                            s_ps[:, co:],
                            kT_sb[:, kv * TOK + kb * 128: kv * TOK + (kb + 1) * 128],
                            qT_t[:, h * 512 + co: (h + 1) * 512],
                            start=True, stop=True)
                        if o >= 0:
                            nc.vector.tensor_add(
                                s_ps[:, co:co + 128], s_ps[:, co:co + 128],
                                mask_sb[:])
                        expT = wrk.tile([128, 512], BF16, tag="expT", bufs=4,
                                        name=f"exp_{h}_{th}_{kb}")
                        nc.scalar.activation(
                            expT[:, co:], s_ps[:, co:], EXP, scale=float(SCALE))
                        exps.append(expT)
                    for i in range(2):
                        if pend[i] is not None:
                            emit_av(i, *pend[i])
                        pend[i] = (kb, exps[i], co)
                    for i in range(2):
                        if pend_acc[i] is not None:
                            emit_acc(i, *pend_acc[i])
                        pend_acc[i] = (kb, exps[i], co)
                    if kb == 2 and fin_q:
                        fin_q.pop(0)()
                    fill(fill_n)
                for i in range(2):
                    emit_av(i, *pend[i])
                    emit_acc(i, *pend_acc[i])

                def finisher():
                    for i, h in enumerate(hs):
                        den_ps = psp.tile([128, 512], F32, tag="pc",
                                          name=f"den_{h}_{th}")
                        nc.tensor.matmul(den_ps[:], ones_sb[:], accs[i][:],
                                         start=True, stop=True)
                        rb = wrk.tile([128, 512], F32, tag="rb",
                                      name=f"rb_{h}_{th}")
                        nc.vector.reciprocal_approx_fast(rb[:], den_ps[:])
                        nc.vector.tensor_mul(
                            at2_t[:, h * 512:(h + 1) * 512], att_ps[i][:], rb[:])
                return finisher

            for th in range(4):
                ts = th * 512
                hts = []
                for jj in range(8):
                    t = wrk.tile([128, 2048], BF16, tag="hT", bufs=11,
                                 name=f"hT_{th}_{jj}")
                    half, j4 = divmod(jj, 4)
                    nc.sync.dma_start(
                        t[:, :1024], hT[th, half, :, j4 * 2048: j4 * 2048 + 1024])
                    nc.sync.dma_start(
                        t[:, 1024:], hT[th, half, :, j4 * 2048 + 1024:(j4 + 1) * 2048])
                    hts.append(t)
                    if th == 0 and jj == 0:
                        # first weight block right behind the first hidden
                        # tile so the PE can start ~4us in, ahead of the
                        # bulk of the startup DMA traffic
                        w_first = wrk.tile([128, 4096], BF16, tag="w", bufs=3,
                                           name="w_k0_0")
                        for q4 in range(4):
                            nc.sync.dma_start(
                                w_first[:, q4 * 1024:(q4 + 1) * 1024],
                                wk[0, q4 // 2, :, (q4 % 2) * 1024:
                                   (q4 % 2 + 1) * 1024])
                if th == 0:
                    # Gate: this dummy store's source depends on the last
                    # hidden tile, so the sync engine stalls here and the
                    # bulk DMAs below don't steal HBM bandwidth from the
                    # critical startup set (hidden tiles + first weights).
                    nc.sync.dma_start(gate_spill[:], hts[7][0:1, 0:8])
                    nc.sync.dma_start(cos_sb[:], cosT[:])
                    nc.sync.dma_start(sin_sb[:], sinTr[:])
                    nc.sync.dma_start(mask_sb[:], mask_diag[:])
                    nc.sync.dma_start(ones_sb[:], ones_sq[:])
                    for q4 in range(4):
                        nc.sync.dma_start(
                            wv_sb[:, q4 * 2048:(q4 + 1) * 2048],
                            wvT[:, q4 * 2048:(q4 + 1) * 2048])
                qT_t = wrk.tile([128, 8 * 512], BF16, tag="qT", name=f"qT_{th}")
                at2_t = wrk.tile([128, 8 * 512], BF16, tag="at2", name=f"at2_{th}")

                def proj_block(wsrc, idx, kind, pre=None):
                    if pre is not None:
                        w_t = pre
                    else:
                        w_t = wrk.tile([128, 4096], BF16, tag="w", bufs=3,
                                       name=f"w_{kind}{idx}_{th}")
                        nc.sync.dma_start(w_t[:, :2048], wsrc[idx, 0])
                        nc.sync.dma_start(w_t[:, 2048:], wsrc[idx, 1])
                    ps = psp.tile([128, 512], F32, tag="pa",
                                  name=f"ps_{kind}{idx}_{th}")
                    for ic in range(32):
                        half, i = divmod(ic, 16)
                        nc.tensor.matmul(
                            ps[:],
                            w_t[:, half * 2048 + i * 128: half * 2048 + (i + 1) * 128],
                            hts[ic // 4][:, (ic % 4) * 512: (ic % 4 + 1) * 512],
                            start=(ic == 0), stop=(ic == 31))
                    return ps

                def rope(ps, dst, kind, idx):
                    m1 = wrk.tile([128, 512], F32, tag="m1",
                                  name=f"m1_{kind}{idx}_{th}")
                    nc.vector.tensor_mul(m1[:], ps[:], cos_sb[:, ts:ts + 512])
                    m2 = wrk.tile([128, 512], F32, tag="m2",
                                  name=f"m2_{kind}{idx}_{th}")
                    nc.vector.tensor_mul(
                        m2[0:64, :], ps[64:128, :], sin_sb[0:64, ts:ts + 512])
                    nc.vector.tensor_mul(
                        m2[64:128, :], ps[0:64, :], sin_sb[64:128, ts:ts + 512])
                    nc.vector.tensor_add(dst, m1[:], m2[:])

                for kv in range(2):
                    ps = proj_block(wk, kv, "k",
                                    pre=w_first if (th == 0 and kv == 0) else None)
                    rope(ps, kT_sb[:, kv * TOK + ts: kv * TOK + ts + 512], "k", kv)
                    fill(4)
                fin_q = []
                for hp in range(0, QH, 2):
                    for h in (hp, hp + 1):
                        ps = proj_block(wq, h, "q")
                        rope(ps, qT_t[:, h * 512:(h + 1) * 512], "q", h)
                        fill(4)
                    if hp == 0:
                        # V^T: out[tok, vdim] with hidden chunks stationary
                        for j in range(4):
                            tb = th * 4 + j
                            pv = psp.tile([128, 256], F32, tag="pa",
                                          name=f"pv_{th}_{j}")
                            for c in range(32):
                                nc.tensor.matmul(
                                    pv[:],
                                    hts[c // 4][:, (c % 4) * 512 + j * 128:
                                                (c % 4) * 512 + (j + 1) * 128],
                                    wv_sb[:, c * 256:(c + 1) * 256],
                                    start=(c == 0), stop=(c == 31))
                            nc.scalar.copy(V_sb[:, tb * 256:(tb + 1) * 256], pv[:])
                            fill(2)
                    fin = attention_group(th, [hp, hp + 1], qT_t, at2_t, fin_q,
                                          fill_n=(8, 9, 6, 5)[th])
                    fin_q.append(fin)
                while fin_q:
                    fin_q.pop(0)()
                    fill(8)

                # drain previous block's o_proj, then queue this block's
                fill(1 << 30)
                filler[0] = oproj_gen(th, at2_t)
            fill(1 << 30)

    nc.compile()
    return nc


def _build_program(variant: str):
    """variant: 'zero' | 'general' (legacy fp32r path, kept as fallback)"""
    nc = bacc.Bacc("TRN2", target_bir_lowering=False, debug=False)

    hT = nc.dram_tensor("hT", [4, 2, 128, 16 * 512], F32R, kind="ExternalInput").ap()
    wq = nc.dram_tensor("wq", [8, 2, 128, 16 * 128], F32R, kind="ExternalInput").ap()
    wk = nc.dram_tensor("wk", [2, 2, 128, 16 * 128], F32R, kind="ExternalInput").ap()
    wv = nc.dram_tensor("wv", [2, 2, 128, 16 * 128], F32R, kind="ExternalInput").ap()
    wo = nc.dram_tensor("wo", [8, 8, 128, 512], F32R, kind="ExternalInput").ap()
    cosT = nc.dram_tensor("cosT", [HD, TOK], F32, kind="ExternalInput").ap()
    sinTr = nc.dram_tensor("sinTr", [HD, TOK], F32, kind="ExternalInput").ap()
    ident = nc.dram_tensor("ident", [128, 128], F32R, kind="ExternalInput").ap()
    ones = nc.dram_tensor("ones", [128, 1], F32R, kind="ExternalInput").ap()
    if variant == "general":
        maskT = nc.dram_tensor("maskT", [S, S], F32, kind="ExternalInput").ap()
    else:
        maskT = None
    out = nc.dram_tensor("out", [TOK, D], F32, kind="ExternalOutput").ap()

    attnT_spill = nc.dram_tensor("attnT_spill", [QCOLS, TOK], F32R).ap()
    qT_spill = nc.dram_tensor("qT_spill", [QCOLS, TOK], F32R).ap()

    NTH = 4
    THW = TOK // NTH         # 512
    NCH = D // 128           # 32 contraction chunks
    NCB = (QCOLS + 2 * KCOLS) // 128  # 12: 0-7 q, 8-9 k, 10-11 v

    with tile.TileContext(nc) as tc:
        with tc.tile_pool(name="per", bufs=1) as per, \
             tc.tile_pool(name="wrk", bufs=2) as wrk, \
             tc.tile_pool(name="one", bufs=1) as one, \
             tc.tile_pool(name="ps", bufs=2, space="PSUM") as psp:

            ident_sb = per.tile([128, 128], F32R, tag="ident")
            ones_sb = per.tile([128, 1], F32R, tag="ones")
            kT_sb = per.tile([HD, 2 * TOK], F32R, tag="kT")
            V_sb = per.tile([128, (TOK // 128) * KCOLS], F32R, tag="V")
            nc.sync.dma_start(ident_sb[:], ident[:])
            nc.sync.dma_start(ones_sb[:], ones[:])

            def attention_group(hs, qb, qT_aps):
                qs = qb * 512
                nkb = TOK // 128
                n = len(hs)
                att_ps = [psp.tile([128, 512], F32, tag="aux", name=f"att_{h}_{qb}")
                          for h in hs]
                sum_ps = [psp.tile([1, 512], F32, tag="sum", name=f"sum_{h}_{qb}")
                          for h in hs]

                def emit_av(i, kb, expT, co):
                    h = hs[i]
                    kv = h // (QH // 2)
                    nc.tensor.matmul(
                        att_ps[i][:, co:],
                        V_sb[:, kb * KCOLS + kv * 128: kb * KCOLS + (kv + 1) * 128],
                        expT[:, co:],
                        start=(kb == 0), stop=(kb == nkb - 1))
                    nc.tensor.matmul(
                        sum_ps[i][:, co:], ones_sb[:], expT[:, co:],
                        start=(kb == 0), stop=(kb == nkb - 1))

                pend = [None] * n
                for kb in range(nkb):
                    co = 0
                    exps = []
                    for i, h in enumerate(hs):
                        kv = h // (QH // 2)
                        s_ps = psp.tile([128, 512], F32, tag="pb",
                                        name=f"s_{h}_{qb}_{kb}")
                        nc.tensor.matmul(
                            s_ps[:, co:],
                            kT_sb[:, kv * TOK + kb * 128: kv * TOK + (kb + 1) * 128],
                            qT_aps[i][:, co:],
                            start=True, stop=True)
                        exp_in = s_ps
                        if variant == "general":
                            mt = wrk.tile([128, 512], F32, tag="mt",
                                          name=f"mt_{h}_{qb}_{kb}")
                            nc.sync.dma_start(
                                mt[:], maskT[kb * 128:(kb + 1) * 128, qs:qs + 512])
                            msk = wrk.tile([128, 512], F32, tag="m1",
                                           name=f"mskg_{h}_{qb}_{kb}")
                            nc.vector.tensor_add(msk[:], s_ps[:], mt[:])
                            exp_in = msk
                        expT = wrk.tile([128, 512], F32R, tag="expT", bufs=4,
                                        name=f"exp_{h}_{qb}_{kb}")
                        nc.scalar.activation(
                            expT[:, co:], exp_in[:, co:], EXP, scale=float(SCALE))
                        exps.append(expT)
                    for i in range(n):
                        if pend[i] is not None:
                            emit_av(i, *pend[i])
                        pend[i] = (kb, exps[i], co)
                for i in range(n):
                    emit_av(i, *pend[i])
                for i, h in enumerate(hs):
                    atu = wrk.tile([128, 512], F32, tag="atu",
                                   name=f"atu_{h}_{qb}")
                    nc.scalar.copy(atu[:], att_ps[i][:])
                    recip = wrk.tile([1, 512], F32, tag="rcp",
                                     name=f"rcp_{h}_{qb}")
                    nc.vector.reciprocal(recip[:], sum_ps[i][:])
                    rb = wrk.tile([128, 512], F32, tag="m2",
                                  name=f"rb_{h}_{qb}")
                    nc.gpsimd.partition_broadcast(rb[:], recip[:])
                    at2 = wrk.tile([128, 512], F32R, tag="vT",
                                   name=f"at2_{h}_{qb}")
                    nc.vector.tensor_mul(at2[:], atu[:], rb[:])
                    nc.scalar.dma_start(
                        attnT_spill[h * 128:(h + 1) * 128, qs:qs + 512], at2[:])

            # ============ Phase A ============
            for th in range(NTH):
                ts = th * THW
                hts = []
                for j in range(8):
                    t = one.tile([128, 4 * THW], F32R, tag=f"hT{j}")
                    half, jj = divmod(j, 4)
                    nc.sync.dma_start(
                        t[:, :1024], hT[th, half, :, jj * 2048:jj * 2048 + 1024])
                    nc.sync.dma_start(
                        t[:, 1024:], hT[th, half, :, jj * 2048 + 1024:(jj + 1) * 2048])
                    hts.append(t)
                cos_t = wrk.tile([HD, THW], F32, tag="cos")
                sin_t = wrk.tile([HD, THW], F32, tag="sin")
                nc.sync.dma_start(cos_t[:], cosT[:, ts:ts + THW])
                nc.sync.dma_start(sin_t[:], sinTr[:, ts:ts + THW])

                qT_lo = one.tile([128, 4 * 512], F32R, tag="qTbl")
                qT_hi = one.tile([128, 4 * 512], F32R, tag="qTbh")

                for cb in range(NCB):
                    if cb < 8:
                        wsrc, widx = wq, cb
                    elif cb < 10:
                        wsrc, widx = wk, cb - 8
                    else:
                        wsrc, widx = wv, cb - 10
                    ps = psp.tile([128, THW], F32, tag="pa")
                    for half in range(2):
                        w_sb = wrk.tile([128, (NCH // 2) * 128], F32R, tag="w")
                        nc.sync.dma_start(w_sb[:, :1024], wsrc[widx, half, :, :1024])
                        nc.sync.dma_start(w_sb[:, 1024:], wsrc[widx, half, :, 1024:])
                        for i in range(NCH // 2):
                            ic = half * (NCH // 2) + i
                            t = hts[ic // 4]
                            nc.tensor.matmul(
                                ps[:],
                                w_sb[:, i * 128:(i + 1) * 128],
                                t[:, (ic % 4) * THW:(ic % 4 + 1) * THW],
                                start=(half == 0 and i == 0),
                                stop=(half == 1 and i == NCH // 2 - 1),
                            )
                    if cb < 10:
                        m1 = wrk.tile([128, THW], F32, tag="m1")
                        nc.vector.tensor_mul(m1[:], ps[:], cos_t[:])
                        m2 = wrk.tile([128, THW], F32, tag="m2")
                        nc.vector.tensor_mul(m2[0:64, :], ps[64:128, :], sin_t[0:64, :])
                        nc.vector.tensor_mul(m2[64:128, :], ps[0:64, :], sin_t[64:128, :])
                        if cb < 8:
                            qdst = qT_lo if cb < 4 else qT_hi
                            nc.vector.tensor_add(
                                qdst[:, (cb % 4) * 512:(cb % 4 + 1) * 512],
                                m1[:], m2[:])
                        else:
                            kv = cb - 8
                            nc.vector.tensor_add(
                                kT_sb[:, kv * TOK + ts: kv * TOK + ts + THW],
                                m1[:], m2[:])
                    else:
                        kv = cb - 10
                        vT = wrk.tile([128, THW], F32R, tag="vT")
                        nc.scalar.copy(vT[:], ps[:])
                        for j in range(THW // 128):
                            tb = th * (THW // 128) + j
                            pt = psp.tile([128, 128], F32R, tag="aux")
                            nc.tensor.transpose(
                                pt[:], vT[:, j * 128:(j + 1) * 128], ident_sb[:])
                            nc.scalar.copy(
                                V_sb[:, tb * KCOLS + kv * 128:
                                     tb * KCOLS + (kv + 1) * 128],
                                pt[:])

                for qi, qt in ((0, qT_lo), (1, qT_hi)):
                    nc.scalar.dma_start(
                        qT_spill[qi * 512:(qi + 1) * 512, ts:ts + THW]
                        .rearrange("(i p) t -> p i t", p=128),
                        qt[:].rearrange("p (i t) -> p i t", i=4),
                    )

            for hp in range(0, QH, 2):
                for qb in range(4):
                    qts = []
                    for h in (hp, hp + 1):
                        qT_t = wrk.tile([128, 512], F32R, tag="qTs",
                                        name=f"qt_{h}_{qb}")
                        nc.sync.dma_start(
                            qT_t[:],
                            qT_spill[h * 128:(h + 1) * 128,
                                     qb * 512:(qb + 1) * 512])
                        qts.append(qT_t)
                    attention_group([hp, hp + 1], qb, qts)

            # ================= Phase C: o_proj partial =================
            ags = []
            for h in range(QH):
                a = one.tile([128, TOK], F32R, tag=f"hT{h}")
                nc.sync.dma_start(a[:], attnT_spill[h * 128:(h + 1) * 128, :])
                ags.append(a)
            for nb in range(D // 512):
                wo_sb = wrk.tile([128, QH * 512], F32R, tag="w")
                for hc in range(QH):
                    nc.sync.dma_start(
                        wo_sb[:, hc * 512:(hc + 1) * 512], wo[nb, hc])
                for qtb in range(TOK // 128):
                    o_ps = psp.tile([128, 512], F32, tag=["pa", "pb", "aux", "sum"][qtb % 4])
                    for hc in range(QH):
                        nc.tensor.matmul(
                            o_ps[:],
                            ags[hc][:, qtb * 128:(qtb + 1) * 128],
                            wo_sb[:, hc * 512:(hc + 1) * 512],
                            start=(hc == 0), stop=(hc == QH - 1))
                    ot = wrk.tile([128, 512], F32, tag="ot", bufs=4)
                    nc.scalar.copy(ot[:], o_ps[:])
                    nc.scalar.dma_start(
                        out[qtb * 128:(qtb + 1) * 128, nb * 512:(nb + 1) * 512],
                        ot[:])

    nc.compile()
    return nc


def _get_program(variant: str):
    if variant not in _PROGRAMS:
        if variant == "causal":
            _PROGRAMS[variant] = _build_causal()
        else:
            _PROGRAMS[variant] = _build_program(variant)
    return _PROGRAMS[variant]


def _detect_variant(mask: np.ndarray) -> str:
    m = mask.reshape(mask.shape[-2], mask.shape[-1])
    if not m.any():
        return "zero"
    causal = np.where(
        np.tril(np.ones((S, S), dtype=bool)), np.float32(0.0), np.float32(NEG))
    if np.array_equal(m, causal):
        return "causal"
    return "general"


def _tile_w(W, np_dt):  # [4096, C] -> [C//128, 2, 128, 2048]
    C = W.shape[1]
    return np.ascontiguousarray(
        W.reshape(2, 16, 128, C // 128, 128).transpose(3, 0, 2, 1, 4)
        .reshape(C // 128, 2, 128, 16 * 128).astype(np_dt))


def _kernel_causal(hidden_states, cos, sin, Wq, Wk, Wv, Wo, trace):
    nc = _get_program("causal")

    i = np.arange(128)[:, None]
    j = np.arange(128)[None, :]
    mask_diag = np.where(i <= j, np.float32(0.0),
                         np.float32(NEG / SCALE)).astype(np.float32)
    ones_sq = np.ones((128, 128), dtype=np.float32)

    per_batch = {}
    for b in range(B):
        sT = np.ascontiguousarray(sin[b].T)
        sinTr = np.concatenate([-sT[:64], sT[64:]], axis=0)
        hid = hidden_states[b]  # [2048, 4096]
        hT_t = np.ascontiguousarray(
            hid.reshape(4, 512, 2, 16, 128).transpose(0, 2, 4, 3, 1)
            .reshape(4, 2, 128, 16 * 512).astype(BF16_NP))
        per_batch[b] = (hT_t,
                        np.ascontiguousarray(cos[b].T.astype(BF16_NP)),
                        np.ascontiguousarray(sinTr.astype(BF16_NP)))

    in_maps = []
    for c in range(NCORES):
        b, g = divmod(c, 4)
        hT_t, cosT, sinTr = per_batch[b]
        wo_c = Wo[g * QCOLS:(g + 1) * QCOLS, :]  # [1024, 4096]
        wo_t = np.ascontiguousarray(
            wo_c.reshape(8, 128, 8, 512).transpose(2, 1, 0, 3)
            .reshape(8, 128, 8 * 512).astype(BF16_NP))
        wv_c = Wv[:, g * KCOLS:(g + 1) * KCOLS]  # [4096, 256]
        wvT_t = np.ascontiguousarray(
            wv_c.reshape(32, 128, 256).transpose(1, 0, 2)
            .reshape(128, 32 * 256).astype(BF16_NP))
        im = {
            "hT": hT_t,
            "wq": _tile_w(Wq[:, g * QCOLS:(g + 1) * QCOLS], BF16_NP),
            "wk": _tile_w(Wk[:, g * KCOLS:(g + 1) * KCOLS], BF16_NP),
            "wvT": wvT_t,
            "wo": wo_t,
            "cosT": cosT,
            "sinTr": sinTr,
            "ones_sq": ones_sq,
            "mask_diag": mask_diag,
        }
        in_maps.append(im)

    res = run_bass_kernel_spmd(nc, in_maps, core_ids=list(range(NCORES)),
                               trace=trace)
    if trace:
        print(f"HW exec time: {res.exec_time_ns} ns")

    out = np.empty((B, S, D), dtype=np.float32)
    for b in range(B):
        acc = np.zeros((S, D), dtype=np.float64)
        for g in range(4):
            acc += np.asarray(res.results[4 * b + g]["out"], dtype=np.float64)
        out[b] = acc.astype(np.float32)
    return out


def _kernel_legacy(variant, hidden_states, cos, sin, attention_mask,
                   Wq, Wk, Wv, Wo, trace):
    nc = _get_program(variant)

    ident = np.eye(128, dtype=np.float32)
    ones = np.ones((128, 1), dtype=np.float32)

    if variant == "general":
        m = attention_mask.reshape(S, S)
        maskT = np.ascontiguousarray(m.T / np.float32(SCALE))
    else:
        maskT = None

    per_batch = {}
    for b in range(B):
        sT = np.ascontiguousarray(sin[b].T)
        sinTr = np.concatenate([-sT[:64], sT[64:]], axis=0)
        hid = hidden_states[b]  # [2048, 4096]
        hT_t = np.ascontiguousarray(
            hid.reshape(4, 512, 2, 16, 128).transpose(0, 2, 4, 3, 1)
            .reshape(4, 2, 128, 16 * 512))
        per_batch[b] = (hT_t, np.ascontiguousarray(cos[b].T),
                        np.ascontiguousarray(sinTr))

    in_maps = []
    for c in range(NCORES):
        b, g = divmod(c, 4)
        hT_t, cosT, sinTr = per_batch[b]
        wo_c = Wo[g * QCOLS:(g + 1) * QCOLS, :]  # [1024, 4096]
        wo_t = np.ascontiguousarray(
            wo_c.reshape(8, 128, 8, 512).transpose(2, 0, 1, 3))
        im = {
            "hT": hT_t,
            "wq": _tile_w(Wq[:, g * QCOLS:(g + 1) * QCOLS], np.float32),
            "wk": _tile_w(Wk[:, g * KCOLS:(g + 1) * KCOLS], np.float32),
            "wv": _tile_w(Wv[:, g * KCOLS:(g + 1) * KCOLS], np.float32),
            "wo": wo_t,
            "cosT": cosT,
            "sinTr": sinTr,
            "ident": ident,
            "ones": ones,
        }
        if maskT is not None:
            im["maskT"] = maskT
        in_maps.append(im)

    res = run_bass_kernel_spmd(nc, in_maps, core_ids=list(range(NCORES)),
                               trace=trace)
    if trace:
        print(f"HW exec time: {res.exec_time_ns} ns")

    out = np.empty((B, S, D), dtype=np.float32)
    for b in range(B):
        acc = np.zeros((S, D), dtype=np.float64)
        for g in range(4):
            acc += res.results[4 * b + g]["out"]
        out[b] = acc.astype(np.float32)
    return out


def kernel(hidden_states, cos, sin, attention_mask, Wq, Wk, Wv, Wo):
    hidden_states = np.asarray(hidden_states, dtype=np.float32)
    cos = np.asarray(cos, dtype=np.float32)
    sin = np.asarray(sin, dtype=np.float32)
    attention_mask = np.asarray(attention_mask, dtype=np.float32)
    Wq = np.asarray(Wq, dtype=np.float32)
    Wk = np.asarray(Wk, dtype=np.float32)
    Wv = np.asarray(Wv, dtype=np.float32)
    Wo = np.asarray(Wo, dtype=np.float32)

    trace = bool(os.environ.get("KERNEL_TRACE"))
    variant = _detect_variant(attention_mask)
    if variant == "causal":
        return _kernel_causal(hidden_states, cos, sin, Wq, Wk, Wv, Wo, trace)
    return _kernel_legacy(variant, hidden_states, cos, sin, attention_mask,
                          Wq, Wk, Wv, Wo, trace)


# revision 26
# speedup vs baseline: 1.2320x; 1.0054x over previous
"""Mistral attention (B=2, S=2048, D=4096, H=32, KVH=8, HD=128) on 8 trn2 cores.

Sharding: core c -> (batch b = c//4, head-group g = c%4).
Each core computes q/k/v projections for its 8 Q heads + 2 KV heads of one
batch, RoPE, causal attention, and a row-parallel partial o_proj
[2048, 4096]. Host sums the 4 partials per batch. No collectives.

Causal fast path (v2):
- All matmul operands are bf16 (same 1 cycle/row PE rate as float32r, half
  the DMA bytes, no 4x small-free penalty on the causal diagonal strips).
- Weights are streamed per token block in bf16; hidden/cos/sin in bf16.
- Attention is transposed (scoresT[keys, qtok], keys on partitions).
  Softmax denominator: exp tiles are accumulated across key blocks on the
  vector engine, then one ones[128,128]-stationary matmul per (head,qblock)
  produces the denominator pre-broadcast across partitions; a fast
  approximate reciprocal replaces the slow DVE reciprocal.
- The causal mask is added only on the true-diagonal 128x128 tiles
  (in place, into the scores psum).
- o_proj for token block t is fused and its matmuls are interleaved as
  filler work into token block t+1's attention rounds, so the in-order PE
  queue always has independent work while waiting for exp results.
- Output partials are written in bf16 and summed on the host in fp64.
"""

import os
import sys

for _p in ("/opt/trn_rl_repo",):
    if _p not in sys.path:
        sys.path.insert(0, _p)

import numpy as np

import concourse.bass as bass
import concourse.tile as tile
from concourse import bacc, bass_isa, mybir
from concourse.bass_utils import run_bass_kernel_spmd

F32 = mybir.dt.float32
F32R = mybir.dt.float32r
BF16 = mybir.dt.bfloat16
EXP = mybir.ActivationFunctionType.Exp
BF16_NP = mybir.dt.np(mybir.dt.bfloat16)

B, S, D = 2, 2048, 4096
H, KVH, HD = 32, 8, 128
SCALE = HD ** -0.5
NCORES = 8

QH = H // 4              # 8 q heads per core
QCOLS = QH * HD          # 1024
KCOLS = (KVH // 4) * HD  # 256 (2 kv heads per core)
TOK = S

NEG = -1e9

_PROGRAMS = {}


def _build_causal():
    nc = bacc.Bacc("TRN2", target_bir_lowering=False, debug=False)

    hT = nc.dram_tensor("hT", [4, 2, 128, 16 * 512], BF16, kind="ExternalInput").ap()
    wq = nc.dram_tensor("wq", [8, 2, 128, 2048], BF16, kind="ExternalInput").ap()
    wk = nc.dram_tensor("wk", [2, 2, 128, 2048], BF16, kind="ExternalInput").ap()
    wvT = nc.dram_tensor("wvT", [128, 32 * 256], BF16, kind="ExternalInput").ap()
    wo = nc.dram_tensor("wo", [8, 128, 8 * 512], BF16, kind="ExternalInput").ap()
    cosT = nc.dram_tensor("cosT", [HD, TOK], BF16, kind="ExternalInput").ap()
    sinTr = nc.dram_tensor("sinTr", [HD, TOK], BF16, kind="ExternalInput").ap()
    ones_sq = nc.dram_tensor("ones_sq", [128, 128], F32R, kind="ExternalInput").ap()
    mask_diag = nc.dram_tensor("mask_diag", [128, 128], F32, kind="ExternalInput").ap()
    out = nc.dram_tensor("out", [TOK, D], BF16, kind="ExternalOutput").ap()
    gate_spill = nc.dram_tensor("gate_spill", [1, 8], BF16).ap()

    with tile.TileContext(nc) as tc:
        with tc.tile_pool(name="per", bufs=1) as per, \
             tc.tile_pool(name="wrk", bufs=2) as wrk, \
             tc.tile_pool(name="ps", bufs=2, space="PSUM") as psp:

            mask_sb = per.tile([128, 128], F32, tag="mask")
            ones_sb = per.tile([128, 128], F32R, tag="ones")
            cos_sb = per.tile([HD, TOK], BF16, tag="cos")
            sin_sb = per.tile([HD, TOK], BF16, tag="sin")
            kT_sb = per.tile([HD, 2 * TOK], BF16, tag="kT")
            V_sb = per.tile([128, 16 * 256], BF16, tag="V")  # tb-major: tb*256+kv*128
            wv_sb = per.tile([128, 32 * 256], BF16, tag="wv")

            # ---- o_proj for token block th, yielded one PE-op at a time ----
            def oproj_gen(th, at2_t):
                wo_ts = {}

                def load(nb):
                    t = wrk.tile([128, 4096], BF16, tag="wo", bufs=4,
                                 name=f"wo_{th}_{nb}")
                    nc.sync.dma_start(t[:, :2048], wo[nb, :, :2048])
                    nc.sync.dma_start(t[:, 2048:], wo[nb, :, 2048:])
                    wo_ts[nb] = t

                load(0)
                load(1)
                for nb in range(8):
                    wo_t = wo_ts.pop(nb)
                    if nb + 2 < 8:
                        load(nb + 2)
                    for j in range(4):
                        po = psp.tile([128, 512], F32, tag="pc",
                                      name=f"po_{th}_{nb}_{j}")
                        for hc in range(8):
                            nc.tensor.matmul(
                                po[:],
                                at2_t[:, hc * 512 + j * 128: hc * 512 + j * 128 + 128],
                                wo_t[:, hc * 512:(hc + 1) * 512],
                                start=(hc == 0), stop=(hc == 7))
                            yield
                        ot = wrk.tile([128, 512], BF16, tag="ot", bufs=4,
                                      name=f"ot_{th}_{nb}_{j}")
                        nc.scalar.copy(ot[:], po[:])
                        nc.scalar.dma_start(
                            out[th * 512 + j * 128: th * 512 + (j + 1) * 128,
                                nb * 512:(nb + 1) * 512],
                            ot[:])
                        yield

            filler = [iter(())]

            def fill(n):
                for _ in range(n):
                    try:
                        next(filler[0])
                    except StopIteration:
                        return

            def attention_group(th, hs, qT_t, at2_t, fin_q, fill_n=8):
                """Transposed causal attention for q heads hs, query block th."""
                nkb = 4 * th + 4
                att_ps = [psp.tile([128, 512], F32, tag="att",
                                   name=f"att_{h}_{th}") for h in hs]
                accs = [wrk.tile([128, 512], F32R, tag="acc", bufs=4,
                                 name=f"acc_{h}_{th}") for h in hs]
                pend = [None, None]
                pend_acc = [None, None]

                def emit_av(i, kb, expT, co):
                    h = hs[i]
                    kv = h // 4
                    nc.tensor.matmul(
                        att_ps[i][:, co:],
                        V_sb[:, kb * 256 + kv * 128: kb * 256 + (kv + 1) * 128],
                        expT[:, co:],
                        start=(kb == 0), stop=(kb == nkb - 1))

                def emit_acc(i, kb, expT, co):
                    if kb == 0:
                        nc.vector.tensor_scalar_add(accs[i][:], expT[:], 0.0)
                    else:
                        nc.vector.tensor_add(
                            accs[i][:, co:], accs[i][:, co:], expT[:, co:])

                for kb in range(nkb):
                    o = kb - 4 * th
                    co = o * 128 if o > 0 else 0
                    exps = []
                    for i, h in enumerate(hs):
                        kv = h // 4
                        s_ps = psp.tile([128, 512], F32, tag="pb",
                                        name=f"s_{h}_{th}_{kb}")
                        nc.tensor.matmul(
                            s_ps[:, co:],
                            kT_sb[:, kv * TOK + kb * 128: kv * TOK + (kb + 1) * 128],
                            qT_t[:, h * 512 + co: (h + 1) * 512],
                            start=True, stop=True)
                        if o >= 0:
                            nc.vector.tensor_add(
                                s_ps[:, co:co + 128], s_ps[:, co:co + 128],
                                mask_sb[:])
                        expT = wrk.tile([128, 512], BF16, tag="expT", bufs=4,
                                        name=f"exp_{h}_{th}_{kb}")
                        nc.scalar.activation(
                            expT[:, co:], s_ps[:, co:], EXP, scale=float(SCALE))
                        exps.append(expT)
                    for i in range(2):
                        if pend[i] is not None:
                            emit_av(i, *pend[i])
                        pend[i] = (kb, exps[i], co)
                    for i in range(2):
                        if pend_acc[i] is not None:
                            emit_acc(i, *pend_acc[i])
                        pend_acc[i] = (kb, exps[i], co)
                    if kb == 2 and fin_q:
                        fin_q.pop(0)()
                    fill(fill_n)
                for i in range(2):
                    emit_av(i, *pend[i])
                    emit_acc(i, *pend_acc[i])

                def finisher():
                    for i, h in enumerate(hs):
                        den_ps = psp.tile([128, 512], F32, tag="pc",
                                          name=f"den_{h}_{th}")
                        nc.tensor.matmul(den_ps[:], ones_sb[:], accs[i][:],
                                         start=True, stop=True)
                        rb = wrk.tile([128, 512], F32, tag="rb",
                                      name=f"rb_{h}_{th}")
                        nc.vector.reciprocal_approx_fast(rb[:], den_ps[:])
                        nc.vector.tensor_mul(
                            at2_t[:, h * 512:(h + 1) * 512], att_ps[i][:], rb[:])
                return finisher

            for th in range(4):
                ts = th * 512
                hts = []
                for jj in range(8):
                    t = wrk.tile([128, 2048], BF16, tag="hT", bufs=11,
                                 name=f"hT_{th}_{jj}")
                    half, j4 = divmod(jj, 4)
                    nc.sync.dma_start(
                        t[:, :1024], hT[th, half, :, j4 * 2048: j4 * 2048 + 1024])
                    nc.sync.dma_start(
                        t[:, 1024:], hT[th, half, :, j4 * 2048 + 1024:(j4 + 1) * 2048])
                    hts.append(t)
                    if th == 0 and jj == 0:
                        # first weight block right behind the first hidden
                        # tile so the PE can start ~4us in, ahead of the
                        # bulk of the startup DMA traffic
                        w_first = wrk.tile([128, 4096], BF16, tag="w", bufs=3,
                                           name="w_k0_0")
                        for q4 in range(4):
                            nc.sync.dma_start(
                                w_first[:, q4 * 1024:(q4 + 1) * 1024],
                                wk[0, q4 // 2, :, (q4 % 2) * 1024:
                                   (q4 % 2 + 1) * 1024])
                def w_fetch(wsrc, idx, kind):
                    w_t = wrk.tile([128, 4096], BF16, tag="w", bufs=3,
                                   name=f"w_{kind}{idx}_{th}")
                    nc.sync.dma_start(w_t[:, :2048], wsrc[idx, 0])
                    nc.sync.dma_start(w_t[:, 2048:], wsrc[idx, 1])
                    return w_t

                pre_w = {}
                if th == 0:
                    # Gate: this dummy store's source depends on the last
                    # hidden tile, so the sync engine stalls here and the
                    # bulk DMAs below don't steal HBM bandwidth from the
                    # critical startup set (hidden tiles + first weights).
                    nc.sync.dma_start(gate_spill[:], hts[7][0:1, 0:8])
                    nc.sync.dma_start(cos_sb[:], cosT[:])
                    nc.sync.dma_start(sin_sb[:], sinTr[:])
                    pre_w[("k", 1)] = w_fetch(wk, 1, "k")
                    pre_w[("q", 0)] = w_fetch(wq, 0, "q")
                    pre_w[("q", 1)] = w_fetch(wq, 1, "q")
                    nc.sync.dma_start(mask_sb[:], mask_diag[:])
                    nc.sync.dma_start(ones_sb[:], ones_sq[:])
                    for q4 in range(4):
                        nc.sync.dma_start(
                            wv_sb[:, q4 * 2048:(q4 + 1) * 2048],
                            wvT[:, q4 * 2048:(q4 + 1) * 2048])
                qT_t = wrk.tile([128, 8 * 512], BF16, tag="qT", name=f"qT_{th}")
                at2_t = wrk.tile([128, 8 * 512], BF16, tag="at2", name=f"at2_{th}")

                def proj_block(wsrc, idx, kind, pre=None):
                    if pre is None:
                        pre = pre_w.pop((kind, idx), None)
                    if pre is not None:
                        w_t = pre
                    else:
                        w_t = w_fetch(wsrc, idx, kind)
                    ps = psp.tile([128, 512], F32, tag="pa",
                                  name=f"ps_{kind}{idx}_{th}")
                    for ic in range(32):
                        half, i = divmod(ic, 16)
                        nc.tensor.matmul(
                            ps[:],
                            w_t[:, half * 2048 + i * 128: half * 2048 + (i + 1) * 128],
                            hts[ic // 4][:, (ic % 4) * 512: (ic % 4 + 1) * 512],
                            start=(ic == 0), stop=(ic == 31))
                    return ps

                def rope(ps, dst, kind, idx):
                    m1 = wrk.tile([128, 512], F32, tag="m1",
                                  name=f"m1_{kind}{idx}_{th}")
                    nc.vector.tensor_mul(m1[:], ps[:], cos_sb[:, ts:ts + 512])
                    m2 = wrk.tile([128, 512], F32, tag="m2",
                                  name=f"m2_{kind}{idx}_{th}")
                    nc.vector.tensor_mul(
                        m2[0:64, :], ps[64:128, :], sin_sb[0:64, ts:ts + 512])
                    nc.vector.tensor_mul(
                        m2[64:128, :], ps[0:64, :], sin_sb[64:128, ts:ts + 512])
                    nc.vector.tensor_add(dst, m1[:], m2[:])

                for kv in range(2):
                    ps = proj_block(wk, kv, "k",
                                    pre=w_first if (th == 0 and kv == 0) else None)
                    rope(ps, kT_sb[:, kv * TOK + ts: kv * TOK + ts + 512], "k", kv)
                    fill(4)
                fin_q = []
                for hp in range(0, QH, 2):
                    for h in (hp, hp + 1):
                        ps = proj_block(wq, h, "q")
                        rope(ps, qT_t[:, h * 512:(h + 1) * 512], "q", h)
                        fill(4)
                    if hp == 0:
                        # V^T: out[tok, vdim] with hidden chunks stationary
                        for j in range(4):
                            tb = th * 4 + j
                            pv = psp.tile([128, 256], F32, tag="pa",
                                          name=f"pv_{th}_{j}")
                            for c in range(32):
                                nc.tensor.matmul(
                                    pv[:],
                                    hts[c // 4][:, (c % 4) * 512 + j * 128:
                                                (c % 4) * 512 + (j + 1) * 128],
                                    wv_sb[:, c * 256:(c + 1) * 256],
                                    start=(c == 0), stop=(c == 31))
                            nc.scalar.copy(V_sb[:, tb * 256:(tb + 1) * 256], pv[:])
                            fill(2)
                    fin = attention_group(th, [hp, hp + 1], qT_t, at2_t, fin_q,
                                          fill_n=(8, 9, 6, 5)[th])
                    fin_q.append(fin)
                while fin_q:
                    fin_q.pop(0)()
                    fill(8)

                # drain previous block's o_proj, then queue this block's
                fill(1 << 30)
                filler[0] = oproj_gen(th, at2_t)
            fill(1 << 30)

    nc.compile()
    return nc


def _build_program(variant: str):
    """variant: 'zero' | 'general' (legacy fp32r path, kept as fallback)"""
    nc = bacc.Bacc("TRN2", target_bir_lowering=False, debug=False)

    hT = nc.dram_tensor("hT", [4, 2, 128, 16 * 512], F32R, kind="ExternalInput").ap()
    wq = nc.dram_tensor("wq", [8, 2, 128, 16 * 128], F32R, kind="ExternalInput").ap()
    wk = nc.dram_tensor("wk", [2, 2, 128, 16 * 128], F32R, kind="ExternalInput").ap()
    wv = nc.dram_tensor("wv", [2, 2, 128, 16 * 128], F32R, kind="ExternalInput").ap()
    wo = nc.dram_tensor("wo", [8, 8, 128, 512], F32R, kind="ExternalInput").ap()
    cosT = nc.dram_tensor("cosT", [HD, TOK], F32, kind="ExternalInput").ap()
    sinTr = nc.dram_tensor("sinTr", [HD, TOK], F32, kind="ExternalInput").ap()
    ident = nc.dram_tensor("ident", [128, 128], F32R, kind="ExternalInput").ap()
    ones = nc.dram_tensor("ones", [128, 1], F32R, kind="ExternalInput").ap()
    if variant == "general":
        maskT = nc.dram_tensor("maskT", [S, S], F32, kind="ExternalInput").ap()
    else:
        maskT = None
    out = nc.dram_tensor("out", [TOK, D], F32, kind="ExternalOutput").ap()

    attnT_spill = nc.dram_tensor("attnT_spill", [QCOLS, TOK], F32R).ap()
    qT_spill = nc.dram_tensor("qT_spill", [QCOLS, TOK], F32R).ap()

    NTH = 4
    THW = TOK // NTH         # 512
    NCH = D // 128           # 32 contraction chunks
    NCB = (QCOLS + 2 * KCOLS) // 128  # 12: 0-7 q, 8-9 k, 10-11 v

    with tile.TileContext(nc) as tc:
        with tc.tile_pool(name="per", bufs=1) as per, \
             tc.tile_pool(name="wrk", bufs=2) as wrk, \
             tc.tile_pool(name="one", bufs=1) as one, \
             tc.tile_pool(name="ps", bufs=2, space="PSUM") as psp:

            ident_sb = per.tile([128, 128], F32R, tag="ident")
            ones_sb = per.tile([128, 1], F32R, tag="ones")
            kT_sb = per.tile([HD, 2 * TOK], F32R, tag="kT")
            V_sb = per.tile([128, (TOK // 128) * KCOLS], F32R, tag="V")
            nc.sync.dma_start(ident_sb[:], ident[:])
            nc.sync.dma_start(ones_sb[:], ones[:])

            def attention_group(hs, qb, qT_aps):
                qs = qb * 512
                nkb = TOK // 128
                n = len(hs)
                att_ps = [psp.tile([128, 512], F32, tag="aux", name=f"att_{h}_{qb}")
                          for h in hs]
                sum_ps = [psp.tile([1, 512], F32, tag="sum", name=f"sum_{h}_{qb}")
                          for h in hs]

                def emit_av(i, kb, expT, co):
                    h = hs[i]
                    kv = h // (QH // 2)
                    nc.tensor.matmul(
                        att_ps[i][:, co:],
                        V_sb[:, kb * KCOLS + kv * 128: kb * KCOLS + (kv + 1) * 128],
                        expT[:, co:],
                        start=(kb == 0), stop=(kb == nkb - 1))
                    nc.tensor.matmul(
                        sum_ps[i][:, co:], ones_sb[:], expT[:, co:],
                        start=(kb == 0), stop=(kb == nkb - 1))

                pend = [None] * n
                for kb in range(nkb):
                    co = 0
                    exps = []
                    for i, h in enumerate(hs):
                        kv = h // (QH // 2)
                        s_ps = psp.tile([128, 512], F32, tag="pb",
                                        name=f"s_{h}_{qb}_{kb}")
                        nc.tensor.matmul(
                            s_ps[:, co:],
                            kT_sb[:, kv * TOK + kb * 128: kv * TOK + (kb + 1) * 128],
                            qT_aps[i][:, co:],
                            start=True, stop=True)
                        exp_in = s_ps
                        if variant == "general":
                            mt = wrk.tile([128, 512], F32, tag="mt",
                                          name=f"mt_{h}_{qb}_{kb}")
                            nc.sync.dma_start(
                                mt[:], maskT[kb * 128:(kb + 1) * 128, qs:qs + 512])
                            msk = wrk.tile([128, 512], F32, tag="m1",
                                           name=f"mskg_{h}_{qb}_{kb}")
                            nc.vector.tensor_add(msk[:], s_ps[:], mt[:])
                            exp_in = msk
                        expT = wrk.tile([128, 512], F32R, tag="expT", bufs=4,
                                        name=f"exp_{h}_{qb}_{kb}")
                        nc.scalar.activation(
                            expT[:, co:], exp_in[:, co:], EXP, scale=float(SCALE))
                        exps.append(expT)
                    for i in range(n):
                        if pend[i] is not None:
                            emit_av(i, *pend[i])
                        pend[i] = (kb, exps[i], co)
                for i in range(n):
                    emit_av(i, *pend[i])
                for i, h in enumerate(hs):
                    atu = wrk.tile([128, 512], F32, tag="atu",
                                   name=f"atu_{h}_{qb}")
                    nc.scalar.copy(atu[:], att_ps[i][:])
                    recip = wrk.tile([1, 512], F32, tag="rcp",
                                     name=f"rcp_{h}_{qb}")
                    nc.vector.reciprocal(recip[:], sum_ps[i][:])
                    rb = wrk.tile([128, 512], F32, tag="m2",
                                  name=f"rb_{h}_{qb}")
                    nc.gpsimd.partition_broadcast(rb[:], recip[:])
                    at2 = wrk.tile([128, 512], F32R, tag="vT",
                                   name=f"at2_{h}_{qb}")
                    nc.vector.tensor_mul(at2[:], atu[:], rb[:])
                    nc.scalar.dma_start(
                        attnT_spill[h * 128:(h + 1) * 128, qs:qs + 512], at2[:])

            # ============ Phase A ============
            for th in range(NTH):
                ts = th * THW
                hts = []
                for j in range(8):
                    t = one.tile([128, 4 * THW], F32R, tag=f"hT{j}")
                    half, jj = divmod(j, 4)
                    nc.sync.dma_start(
                        t[:, :1024], hT[th, half, :, jj * 2048:jj * 2048 + 1024])
                    nc.sync.dma_start(
                        t[:, 1024:], hT[th, half, :, jj * 2048 + 1024:(jj + 1) * 2048])
                    hts.append(t)
                cos_t = wrk.tile([HD, THW], F32, tag="cos")
                sin_t = wrk.tile([HD, THW], F32, tag="sin")
                nc.sync.dma_start(cos_t[:], cosT[:, ts:ts + THW])
                nc.sync.dma_start(sin_t[:], sinTr[:, ts:ts + THW])

                qT_lo = one.tile([128, 4 * 512], F32R, tag="qTbl")
                qT_hi = one.tile([128, 4 * 512], F32R, tag="qTbh")

                for cb in range(NCB):
                    if cb < 8:
                        wsrc, widx = wq, cb
                    elif cb < 10:
                        wsrc, widx = wk, cb - 8
                    else:
                        wsrc, widx = wv, cb - 10
                    ps = psp.tile([128, THW], F32, tag="pa")
                    for half in range(2):
                        w_sb = wrk.tile([128, (NCH // 2) * 128], F32R, tag="w")
                        nc.sync.dma_start(w_sb[:, :1024], wsrc[widx, half, :, :1024])
                        nc.sync.dma_start(w_sb[:, 1024:], wsrc[widx, half, :, 1024:])
                        for i in range(NCH // 2):
                            ic = half * (NCH // 2) + i
                            t = hts[ic // 4]
                            nc.tensor.matmul(
                                ps[:],
                                w_sb[:, i * 128:(i + 1) * 128],
                                t[:, (ic % 4) * THW:(ic % 4 + 1) * THW],
                                start=(half == 0 and i == 0),
                                stop=(half == 1 and i == NCH // 2 - 1),
                            )
                    if cb < 10:
                        m1 = wrk.tile([128, THW], F32, tag="m1")
                        nc.vector.tensor_mul(m1[:], ps[:], cos_t[:])
                        m2 = wrk.tile([128, THW], F32, tag="m2")
                        nc.vector.tensor_mul(m2[0:64, :], ps[64:128, :], sin_t[0:64, :])
                        nc.vector.tensor_mul(m2[64:128, :], ps[0:64, :], sin_t[64:128, :])
                        if cb < 8:
                            qdst = qT_lo if cb < 4 else qT_hi
                            nc.vector.tensor_add(
                                qdst[:, (cb % 4) * 512:(cb % 4 + 1) * 512],
                                m1[:], m2[:])
                        else:
                            kv = cb - 8
                            nc.vector.tensor_add(
                                kT_sb[:, kv * TOK + ts: kv * TOK + ts + THW],
                                m1[:], m2[:])
                    else:
                        kv = cb - 10
                        vT = wrk.tile([128, THW], F32R, tag="vT")
                        nc.scalar.copy(vT[:], ps[:])
                        for j in range(THW // 128):
                            tb = th * (THW // 128) + j
                            pt = psp.tile([128, 128], F32R, tag="aux")
                            nc.tensor.transpose(
                                pt[:], vT[:, j * 128:(j + 1) * 128], ident_sb[:])
                            nc.scalar.copy(
                                V_sb[:, tb * KCOLS + kv * 128:
                                     tb * KCOLS + (kv + 1) * 128],
                                pt[:])

                for qi, qt in ((0, qT_lo), (1, qT_hi)):
                    nc.scalar.dma_start(
                        qT_spill[qi * 512:(qi + 1) * 512, ts:ts + THW]
                        .rearrange("(i p) t -> p i t", p=128),
                        qt[:].rearrange("p (i t) -> p i t", i=4),
                    )

            for hp in range(0, QH, 2):
                for qb in range(4):
                    qts = []
                    for h in (hp, hp + 1):
                        qT_t = wrk.tile([128, 512], F32R, tag="qTs",
                                        name=f"qt_{h}_{qb}")
                        nc.sync.dma_start(
                            qT_t[:],
                            qT_spill[h * 128:(h + 1) * 128,
                                     qb * 512:(qb + 1) * 512])
                        qts.append(qT_t)
                    attention_group([hp, hp + 1], qb, qts)

            # ================= Phase C: o_proj partial =================
            ags = []
            for h in range(QH):
                a = one.tile([128, TOK], F32R, tag=f"hT{h}")
                nc.sync.dma_start(a[:], attnT_spill[h * 128:(h + 1) * 128, :])
                ags.append(a)
            for nb in range(D // 512):
                wo_sb = wrk.tile([128, QH * 512], F32R, tag="w")
                for hc in range(QH):
                    nc.sync.dma_start(
                        wo_sb[:, hc * 512:(hc + 1) * 512], wo[nb, hc])
                for qtb in range(TOK // 128):
                    o_ps = psp.tile([128, 512], F32, tag=["pa", "pb", "aux", "sum"][qtb % 4])
                    for hc in range(QH):
                        nc.tensor.matmul(
                            o_ps[:],
                            ags[hc][:, qtb * 128:(qtb + 1) * 128],
                            wo_sb[:, hc * 512:(hc + 1) * 512],
                            start=(hc == 0), stop=(hc == QH - 1))
                    ot = wrk.tile([128, 512], F32, tag="ot", bufs=4)
                    nc.scalar.copy(ot[:], o_ps[:])
                    nc.scalar.dma_start(
                        out[qtb * 128:(qtb + 1) * 128, nb * 512:(nb + 1) * 512],
                        ot[:])

    nc.compile()
    return nc


def _get_program(variant: str):
    if variant not in _PROGRAMS:
        if variant == "causal":
            _PROGRAMS[variant] = _build_causal()
        else:
            _PROGRAMS[variant] = _build_program(variant)
    return _PROGRAMS[variant]


def _detect_variant(mask: np.ndarray) -> str:
    m = mask.reshape(mask.shape[-2], mask.shape[-1])
    if not m.any():
        return "zero"
    causal = np.where(
        np.tril(np.ones((S, S), dtype=bool)), np.float32(0.0), np.float32(NEG))
    if np.array_equal(m, causal):
        return "causal"
    return "general"


def _tile_w(W, np_dt):  # [4096, C] -> [C//128, 2, 128, 2048]
    C = W.shape[1]
    return np.ascontiguousarray(
        W.reshape(2, 16, 128, C // 128, 128).transpose(3, 0, 2, 1, 4)
        .reshape(C // 128, 2, 128, 16 * 128).astype(np_dt))


def _kernel_causal(hidden_states, cos, sin, Wq, Wk, Wv, Wo, trace):
    nc = _get_program("causal")

    i = np.arange(128)[:, None]
    j = np.arange(128)[None, :]
    mask_diag = np.where(i <= j, np.float32(0.0),
                         np.float32(NEG / SCALE)).astype(np.float32)
    ones_sq = np.ones((128, 128), dtype=np.float32)

    per_batch = {}
    for b in range(B):
        sT = np.ascontiguousarray(sin[b].T)
        sinTr = np.concatenate([-sT[:64], sT[64:]], axis=0)
        hid = hidden_states[b]  # [2048, 4096]
        hT_t = np.ascontiguousarray(
            hid.reshape(4, 512, 2, 16, 128).transpose(0, 2, 4, 3, 1)
            .reshape(4, 2, 128, 16 * 512).astype(BF16_NP))
        per_batch[b] = (hT_t,
                        np.ascontiguousarray(cos[b].T.astype(BF16_NP)),
                        np.ascontiguousarray(sinTr.astype(BF16_NP)))

    in_maps = []
    for c in range(NCORES):
        b, g = divmod(c, 4)
        hT_t, cosT, sinTr = per_batch[b]
        wo_c = Wo[g * QCOLS:(g + 1) * QCOLS, :]  # [1024, 4096]
        wo_t = np.ascontiguousarray(
            wo_c.reshape(8, 128, 8, 512).transpose(2, 1, 0, 3)
            .reshape(8, 128, 8 * 512).astype(BF16_NP))
        wv_c = Wv[:, g * KCOLS:(g + 1) * KCOLS]  # [4096, 256]
        wvT_t = np.ascontiguousarray(
            wv_c.reshape(32, 128, 256).transpose(1, 0, 2)
            .reshape(128, 32 * 256).astype(BF16_NP))
        im = {
            "hT": hT_t,
            "wq": _tile_w(Wq[:, g * QCOLS:(g + 1) * QCOLS], BF16_NP),
            "wk": _tile_w(Wk[:, g * KCOLS:(g + 1) * KCOLS], BF16_NP),
            "wvT": wvT_t,
            "wo": wo_t,
            "cosT": cosT,
            "sinTr": sinTr,
            "ones_sq": ones_sq,
            "mask_diag": mask_diag,
        }
        in_maps.append(im)

    res = run_bass_kernel_spmd(nc, in_maps, core_ids=list(range(NCORES)),
                               trace=trace)
    if trace:
        print(f"HW exec time: {res.exec_time_ns} ns")

    out = np.empty((B, S, D), dtype=np.float32)
    for b in range(B):
        acc = np.zeros((S, D), dtype=np.float64)
        for g in range(4):
            acc += np.asarray(res.results[4 * b + g]["out"], dtype=np.float64)
        out[b] = acc.astype(np.float32)
    return out


def _kernel_legacy(variant, hidden_states, cos, sin, attention_mask,
                   Wq, Wk, Wv, Wo, trace):
    nc = _get_program(variant)

    ident = np.eye(128, dtype=np.float32)
    ones = np.ones((128, 1), dtype=np.float32)

    if variant == "general":
        m = attention_mask.reshape(S, S)
        maskT = np.ascontiguousarray(m.T / np.float32(SCALE))
    else:
        maskT = None

    per_batch = {}
    for b in range(B):
        sT = np.ascontiguousarray(sin[b].T)
        sinTr = np.concatenate([-sT[:64], sT[64:]], axis=0)
        hid = hidden_states[b]  # [2048, 4096]
        hT_t = np.ascontiguousarray(
            hid.reshape(4, 512, 2, 16, 128).transpose(0, 2, 4, 3, 1)
            .reshape(4, 2, 128, 16 * 512))
        per_batch[b] = (hT_t, np.ascontiguousarray(cos[b].T),
                        np.ascontiguousarray(sinTr))

    in_maps = []
    for c in range(NCORES):
        b, g = divmod(c, 4)
        hT_t, cosT, sinTr = per_batch[b]
        wo_c = Wo[g * QCOLS:(g + 1) * QCOLS, :]  # [1024, 4096]
        wo_t = np.ascontiguousarray(
            wo_c.reshape(8, 128, 8, 512).transpose(2, 0, 1, 3))
        im = {
            "hT": hT_t,
            "wq": _tile_w(Wq[:, g * QCOLS:(g + 1) * QCOLS], np.float32),
            "wk": _tile_w(Wk[:, g * KCOLS:(g + 1) * KCOLS], np.float32),
            "wv": _tile_w(Wv[:, g * KCOLS:(g + 1) * KCOLS], np.float32),
            "wo": wo_t,
            "cosT": cosT,
            "sinTr": sinTr,
            "ident": ident,
            "ones": ones,
        }
        if maskT is not None:
            im["maskT"] = maskT
        in_maps.append(im)

    res = run_bass_kernel_spmd(nc, in_maps, core_ids=list(range(NCORES)),
                               trace=trace)
    if trace:
        print(f"HW exec time: {res.exec_time_ns} ns")

    out = np.empty((B, S, D), dtype=np.float32)
    for b in range(B):
        acc = np.zeros((S, D), dtype=np.float64)
        for g in range(4):
            acc += res.results[4 * b + g]["out"]
        out[b] = acc.astype(np.float32)
    return out


def kernel(hidden_states, cos, sin, attention_mask, Wq, Wk, Wv, Wo):
    hidden_states = np.asarray(hidden_states, dtype=np.float32)
    cos = np.asarray(cos, dtype=np.float32)
    sin = np.asarray(sin, dtype=np.float32)
    attention_mask = np.asarray(attention_mask, dtype=np.float32)
    Wq = np.asarray(Wq, dtype=np.float32)
    Wk = np.asarray(Wk, dtype=np.float32)
    Wv = np.asarray(Wv, dtype=np.float32)
    Wo = np.asarray(Wo, dtype=np.float32)

    trace = bool(os.environ.get("KERNEL_TRACE"))
    variant = _detect_variant(attention_mask)
    if variant == "causal":
        return _kernel_causal(hidden_states, cos, sin, Wq, Wk, Wv, Wo, trace)
    return _kernel_legacy(variant, hidden_states, cos, sin, attention_mask,
                          Wq, Wk, Wv, Wo, trace)


# revision 33
# speedup vs baseline: 1.2347x; 1.0021x over previous
"""Mistral attention (B=2, S=2048, D=4096, H=32, KVH=8, HD=128) on 8 trn2 cores.

Sharding: core c -> (batch b = c//4, head-group g = c%4).
Each core computes q/k/v projections for its 8 Q heads + 2 KV heads of one
batch, RoPE, causal attention, and a row-parallel partial o_proj
[2048, 4096]. Host sums the 4 partials per batch. No collectives.

Causal fast path (v2):
- All matmul operands are bf16 (same 1 cycle/row PE rate as float32r, half
  the DMA bytes, no 4x small-free penalty on the causal diagonal strips).
- Weights are streamed per token block in bf16; hidden/cos/sin in bf16.
- Attention is transposed (scoresT[keys, qtok], keys on partitions).
  Softmax denominator: exp tiles are accumulated across key blocks on the
  vector engine, then one ones[128,128]-stationary matmul per (head,qblock)
  produces the denominator pre-broadcast across partitions; a fast
  approximate reciprocal replaces the slow DVE reciprocal.
- The causal mask is added only on the true-diagonal 128x128 tiles
  (in place, into the scores psum).
- o_proj for token block t is fused and its matmuls are interleaved as
  filler work into token block t+1's attention rounds, so the in-order PE
  queue always has independent work while waiting for exp results.
- Output partials are written in bf16 and summed on the host in fp64.
"""

import os
import sys

for _p in ("/opt/trn_rl_repo",):
    if _p not in sys.path:
        sys.path.insert(0, _p)

import numpy as np

import concourse.bass as bass
import concourse.tile as tile
from concourse import bacc, bass_isa, mybir
from concourse.bass_utils import run_bass_kernel_spmd

F32 = mybir.dt.float32
F32R = mybir.dt.float32r
BF16 = mybir.dt.bfloat16
EXP = mybir.ActivationFunctionType.Exp
BF16_NP = mybir.dt.np(mybir.dt.bfloat16)

B, S, D = 2, 2048, 4096
H, KVH, HD = 32, 8, 128
SCALE = HD ** -0.5
NCORES = 8

QH = H // 4              # 8 q heads per core
QCOLS = QH * HD          # 1024
KCOLS = (KVH // 4) * HD  # 256 (2 kv heads per core)
TOK = S

NEG = -1e9

_PROGRAMS = {}


def _build_causal():
    nc = bacc.Bacc("TRN2", target_bir_lowering=False, debug=False)

    hT = nc.dram_tensor("hT", [4, 2, 128, 16 * 512], BF16, kind="ExternalInput").ap()
    wq = nc.dram_tensor("wq", [8, 128, 4096], BF16, kind="ExternalInput").ap()
    wk = nc.dram_tensor("wk", [2, 128, 4096], BF16, kind="ExternalInput").ap()
    wvT = nc.dram_tensor("wvT", [128, 32 * 256], BF16, kind="ExternalInput").ap()
    wo = nc.dram_tensor("wo", [8, 128, 8 * 512], BF16, kind="ExternalInput").ap()
    cosT = nc.dram_tensor("cosT", [HD, TOK], BF16, kind="ExternalInput").ap()
    sinTr = nc.dram_tensor("sinTr", [HD, TOK], BF16, kind="ExternalInput").ap()
    ones_sq = nc.dram_tensor("ones_sq", [128, 128], F32R, kind="ExternalInput").ap()
    mask_diag = nc.dram_tensor("mask_diag", [128, 128], F32, kind="ExternalInput").ap()
    out = nc.dram_tensor("out", [TOK, D], BF16, kind="ExternalOutput").ap()
    gate_spill = nc.dram_tensor("gate_spill", [1, 8], BF16).ap()

    with tile.TileContext(nc) as tc:
        with tc.tile_pool(name="per", bufs=1) as per, \
             tc.tile_pool(name="wrk", bufs=2) as wrk, \
             tc.tile_pool(name="ps", bufs=2, space="PSUM") as psp:

            mask_sb = per.tile([128, 128], F32, tag="mask")
            ones_sb = per.tile([128, 128], F32R, tag="ones")
            cos_sb = per.tile([HD, TOK], BF16, tag="cos")
            sin_sb = per.tile([HD, TOK], BF16, tag="sin")
            kT_sb = per.tile([HD, 2 * TOK], BF16, tag="kT")
            V_sb = per.tile([128, 16 * 256], BF16, tag="V")  # tb-major: tb*256+kv*128
            wv_sb = per.tile([128, 32 * 256], BF16, tag="wv")

            # ---- o_proj for token block th, yielded one PE-op at a time ----
            def oproj_gen(th, at2_t):
                wo_ts = {}

                def load(nb):
                    t = wrk.tile([128, 4096], BF16, tag="wo", bufs=4,
                                 name=f"wo_{th}_{nb}")
                    nc.sync.dma_start(t[:], wo[nb])
                    wo_ts[nb] = t

                load(0)
                load(1)
                for nb in range(8):
                    wo_t = wo_ts.pop(nb)
                    if nb + 2 < 8:
                        load(nb + 2)
                    for j in range(4):
                        po = psp.tile([128, 512], F32, tag="pc",
                                      name=f"po_{th}_{nb}_{j}")
                        for hc in range(8):
                            nc.tensor.matmul(
                                po[:],
                                at2_t[:, hc * 512 + j * 128: hc * 512 + j * 128 + 128],
                                wo_t[:, hc * 512:(hc + 1) * 512],
                                start=(hc == 0), stop=(hc == 7))
                            yield
                        ot = wrk.tile([128, 512], BF16, tag="ot", bufs=4,
                                      name=f"ot_{th}_{nb}_{j}")
                        nc.scalar.copy(ot[:], po[:])
                        nc.scalar.dma_start(
                            out[th * 512 + j * 128: th * 512 + (j + 1) * 128,
                                nb * 512:(nb + 1) * 512],
                            ot[:])
                        yield

            filler = [iter(())]

            def fill(n):
                for _ in range(n):
                    try:
                        next(filler[0])
                    except StopIteration:
                        return

            def attention_group(th, hs, qT_t, at2_t, fin_q, fill_n=8):
                """Transposed causal attention for q heads hs, query block th."""
                nkb = 4 * th + 4
                att_ps = [psp.tile([128, 512], F32, tag="att",
                                   name=f"att_{h}_{th}") for h in hs]
                accs = [wrk.tile([128, 512], F32R, tag="acc", bufs=4,
                                 name=f"acc_{h}_{th}") for h in hs]
                pend = [None, None]
                pend_acc = [None, None]

                def emit_av(i, kb, expT, co):
                    h = hs[i]
                    kv = h // 4
                    nc.tensor.matmul(
                        att_ps[i][:, co:],
                        V_sb[:, kb * 256 + kv * 128: kb * 256 + (kv + 1) * 128],
                        expT[:, co:],
                        start=(kb == 0), stop=(kb == nkb - 1))

                def emit_acc(i, kb, expT, co):
                    if kb == 0:
                        nc.vector.tensor_scalar_add(accs[i][:], expT[:], 0.0)
                    else:
                        nc.vector.tensor_add(
                            accs[i][:, co:], accs[i][:, co:], expT[:, co:])

                for kb in range(nkb):
                    o = kb - 4 * th
                    co = o * 128 if o > 0 else 0
                    exps = []
                    for i, h in enumerate(hs):
                        kv = h // 4
                        s_ps = psp.tile([128, 512], F32, tag="pb",
                                        name=f"s_{h}_{th}_{kb}")
                        nc.tensor.matmul(
                            s_ps[:, co:],
                            kT_sb[:, kv * TOK + kb * 128: kv * TOK + (kb + 1) * 128],
                            qT_t[:, h * 512 + co: (h + 1) * 512],
                            start=True, stop=True)
                        if o >= 0:
                            nc.vector.tensor_add(
                                s_ps[:, co:co + 128], s_ps[:, co:co + 128],
                                mask_sb[:])
                        expT = wrk.tile([128, 512], BF16, tag="expT", bufs=4,
                                        name=f"exp_{h}_{th}_{kb}")
                        nc.scalar.activation(
                            expT[:, co:], s_ps[:, co:], EXP, scale=float(SCALE))
                        exps.append(expT)
                    for i in range(2):
                        if pend[i] is not None:
                            emit_av(i, *pend[i])
                        pend[i] = (kb, exps[i], co)
                    for i in range(2):
                        if pend_acc[i] is not None:
                            emit_acc(i, *pend_acc[i])
                        pend_acc[i] = (kb, exps[i], co)
                    if kb == 2 and fin_q:
                        fin_q.pop(0)()
                    fill(fill_n)
                for i in range(2):
                    emit_av(i, *pend[i])
                    emit_acc(i, *pend_acc[i])

                def finisher():
                    for i, h in enumerate(hs):
                        den_ps = psp.tile([128, 512], F32, tag="pc",
                                          name=f"den_{h}_{th}")
                        nc.tensor.matmul(den_ps[:], ones_sb[:], accs[i][:],
                                         start=True, stop=True)
                        rb = wrk.tile([128, 512], F32, tag="rb",
                                      name=f"rb_{h}_{th}")
                        nc.vector.reciprocal_approx_fast(rb[:], den_ps[:])
                        nc.vector.tensor_mul(
                            at2_t[:, h * 512:(h + 1) * 512], att_ps[i][:], rb[:])
                return finisher

            for th in range(4):
                ts = th * 512
                hts = []
                for jj in range(8):
                    t = wrk.tile([128, 2048], BF16, tag="hT", bufs=11,
                                 name=f"hT_{th}_{jj}")
                    half, j4 = divmod(jj, 4)
                    nc.sync.dma_start(
                        t[:], hT[th, half, :, j4 * 2048:(j4 + 1) * 2048])
                    hts.append(t)
                    if th == 0 and jj == 0:
                        # first weight block right behind the first hidden
                        # tile so the PE can start ~6us in, ahead of the
                        # bulk of the startup DMA traffic
                        w_first = wrk.tile([128, 4096], BF16, tag="w", bufs=3,
                                           name="w_k0_0")
                        nc.sync.dma_start(w_first[:], wk[0])
                def w_fetch(wsrc, idx, kind):
                    w_t = wrk.tile([128, 4096], BF16, tag="w", bufs=3,
                                   name=f"w_{kind}{idx}_{th}")
                    nc.sync.dma_start(w_t[:], wsrc[idx])
                    return w_t

                pre_w = {}
                if th == 0:
                    # Gate: this dummy store's source depends on the last
                    # hidden tile, so the sync engine stalls here and the
                    # bulk DMAs below don't steal HBM bandwidth from the
                    # critical startup set (hidden tiles + first weights).
                    nc.sync.dma_start(gate_spill[:], hts[7][0:1, 0:8])
                    nc.sync.dma_start(cos_sb[:], cosT[:])
                    nc.sync.dma_start(sin_sb[:], sinTr[:])
                    pre_w[("k", 1)] = w_fetch(wk, 1, "k")
                    pre_w[("q", 0)] = w_fetch(wq, 0, "q")
                    pre_w[("q", 1)] = w_fetch(wq, 1, "q")
                    nc.sync.dma_start(mask_sb[:], mask_diag[:])
                    nc.sync.dma_start(ones_sb[:], ones_sq[:])
                    nc.sync.dma_start(wv_sb[:], wvT[:])
                qT_t = wrk.tile([128, 8 * 512], BF16, tag="qT", name=f"qT_{th}")
                at2_t = wrk.tile([128, 8 * 512], BF16, tag="at2", name=f"at2_{th}")

                def proj_block(wsrc, idx, kind, pre=None):
                    if pre is None:
                        pre = pre_w.pop((kind, idx), None)
                    if pre is not None:
                        w_t = pre
                    else:
                        w_t = w_fetch(wsrc, idx, kind)
                    ps = psp.tile([128, 512], F32, tag="pa",
                                  name=f"ps_{kind}{idx}_{th}")
                    for ic in range(32):
                        half, i = divmod(ic, 16)
                        nc.tensor.matmul(
                            ps[:],
                            w_t[:, half * 2048 + i * 128: half * 2048 + (i + 1) * 128],
                            hts[ic // 4][:, (ic % 4) * 512: (ic % 4 + 1) * 512],
                            start=(ic == 0), stop=(ic == 31))
                    return ps

                def rope(ps, dst, kind, idx):
                    m1 = wrk.tile([128, 512], F32, tag="m1",
                                  name=f"m1_{kind}{idx}_{th}")
                    nc.vector.tensor_mul(m1[:], ps[:], cos_sb[:, ts:ts + 512])
                    m2 = wrk.tile([128, 512], F32, tag="m2",
                                  name=f"m2_{kind}{idx}_{th}")
                    nc.vector.tensor_mul(
                        m2[0:64, :], ps[64:128, :], sin_sb[0:64, ts:ts + 512])
                    nc.vector.tensor_mul(
                        m2[64:128, :], ps[0:64, :], sin_sb[64:128, ts:ts + 512])
                    nc.vector.tensor_add(dst, m1[:], m2[:])

                for kv in range(2):
                    ps = proj_block(wk, kv, "k",
                                    pre=w_first if (th == 0 and kv == 0) else None)
                    rope(ps, kT_sb[:, kv * TOK + ts: kv * TOK + ts + 512], "k", kv)
                    fill(4)
                fin_q = []
                for hp in range(0, QH, 2):
                    for h in (hp, hp + 1):
                        ps = proj_block(wq, h, "q")
                        rope(ps, qT_t[:, h * 512:(h + 1) * 512], "q", h)
                        fill(4)
                    if hp == 0:
                        # V^T: out[tok, vdim] with hidden chunks stationary
                        for j in range(4):
                            tb = th * 4 + j
                            pv = psp.tile([128, 256], F32, tag="pa",
                                          name=f"pv_{th}_{j}")
                            for c in range(32):
                                nc.tensor.matmul(
                                    pv[:],
                                    hts[c // 4][:, (c % 4) * 512 + j * 128:
                                                (c % 4) * 512 + (j + 1) * 128],
                                    wv_sb[:, c * 256:(c + 1) * 256],
                                    start=(c == 0), stop=(c == 31))
                            nc.scalar.copy(V_sb[:, tb * 256:(tb + 1) * 256], pv[:])
                            fill(2)
                    fin = attention_group(th, [hp, hp + 1], qT_t, at2_t, fin_q,
                                          fill_n=(8, 9, 6, 5)[th])
                    fin_q.append(fin)
                while fin_q:
                    fin_q.pop(0)()
                    fill(8)

                # drain previous block's o_proj, then queue this block's
                fill(1 << 30)
                filler[0] = oproj_gen(th, at2_t)
            fill(1 << 30)

    nc.compile()
    return nc


def _build_program(variant: str):
    """variant: 'zero' | 'general' (legacy fp32r path, kept as fallback)"""
    nc = bacc.Bacc("TRN2", target_bir_lowering=False, debug=False)

    hT = nc.dram_tensor("hT", [4, 2, 128, 16 * 512], F32R, kind="ExternalInput").ap()
    wq = nc.dram_tensor("wq", [8, 2, 128, 16 * 128], F32R, kind="ExternalInput").ap()
    wk = nc.dram_tensor("wk", [2, 2, 128, 16 * 128], F32R, kind="ExternalInput").ap()
    wv = nc.dram_tensor("wv", [2, 2, 128, 16 * 128], F32R, kind="ExternalInput").ap()
    wo = nc.dram_tensor("wo", [8, 8, 128, 512], F32R, kind="ExternalInput").ap()
    cosT = nc.dram_tensor("cosT", [HD, TOK], F32, kind="ExternalInput").ap()
    sinTr = nc.dram_tensor("sinTr", [HD, TOK], F32, kind="ExternalInput").ap()
    ident = nc.dram_tensor("ident", [128, 128], F32R, kind="ExternalInput").ap()
    ones = nc.dram_tensor("ones", [128, 1], F32R, kind="ExternalInput").ap()
    if variant == "general":
        maskT = nc.dram_tensor("maskT", [S, S], F32, kind="ExternalInput").ap()
    else:
        maskT = None
    out = nc.dram_tensor("out", [TOK, D], F32, kind="ExternalOutput").ap()

    attnT_spill = nc.dram_tensor("attnT_spill", [QCOLS, TOK], F32R).ap()
    qT_spill = nc.dram_tensor("qT_spill", [QCOLS, TOK], F32R).ap()

    NTH = 4
    THW = TOK // NTH         # 512
    NCH = D // 128           # 32 contraction chunks
    NCB = (QCOLS + 2 * KCOLS) // 128  # 12: 0-7 q, 8-9 k, 10-11 v

    with tile.TileContext(nc) as tc:
        with tc.tile_pool(name="per", bufs=1) as per, \
             tc.tile_pool(name="wrk", bufs=2) as wrk, \
             tc.tile_pool(name="one", bufs=1) as one, \
             tc.tile_pool(name="ps", bufs=2, space="PSUM") as psp:

            ident_sb = per.tile([128, 128], F32R, tag="ident")
            ones_sb = per.tile([128, 1], F32R, tag="ones")
            kT_sb = per.tile([HD, 2 * TOK], F32R, tag="kT")
            V_sb = per.tile([128, (TOK // 128) * KCOLS], F32R, tag="V")
            nc.sync.dma_start(ident_sb[:], ident[:])
            nc.sync.dma_start(ones_sb[:], ones[:])

            def attention_group(hs, qb, qT_aps):
                qs = qb * 512
                nkb = TOK // 128
                n = len(hs)
                att_ps = [psp.tile([128, 512], F32, tag="aux", name=f"att_{h}_{qb}")
                          for h in hs]
                sum_ps = [psp.tile([1, 512], F32, tag="sum", name=f"sum_{h}_{qb}")
                          for h in hs]

                def emit_av(i, kb, expT, co):
                    h = hs[i]
                    kv = h // (QH // 2)
                    nc.tensor.matmul(
                        att_ps[i][:, co:],
                        V_sb[:, kb * KCOLS + kv * 128: kb * KCOLS + (kv + 1) * 128],
                        expT[:, co:],
                        start=(kb == 0), stop=(kb == nkb - 1))
                    nc.tensor.matmul(
                        sum_ps[i][:, co:], ones_sb[:], expT[:, co:],
                        start=(kb == 0), stop=(kb == nkb - 1))

                pend = [None] * n
                for kb in range(nkb):
                    co = 0
                    exps = []
                    for i, h in enumerate(hs):
                        kv = h // (QH // 2)
                        s_ps = psp.tile([128, 512], F32, tag="pb",
                                        name=f"s_{h}_{qb}_{kb}")
                        nc.tensor.matmul(
                            s_ps[:, co:],
                            kT_sb[:, kv * TOK + kb * 128: kv * TOK + (kb + 1) * 128],
                            qT_aps[i][:, co:],
                            start=True, stop=True)
                        exp_in = s_ps
                        if variant == "general":
                            mt = wrk.tile([128, 512], F32, tag="mt",
                                          name=f"mt_{h}_{qb}_{kb}")
                            nc.sync.dma_start(
                                mt[:], maskT[kb * 128:(kb + 1) * 128, qs:qs + 512])
                            msk = wrk.tile([128, 512], F32, tag="m1",
                                           name=f"mskg_{h}_{qb}_{kb}")
                            nc.vector.tensor_add(msk[:], s_ps[:], mt[:])
                            exp_in = msk
                        expT = wrk.tile([128, 512], F32R, tag="expT", bufs=4,
                                        name=f"exp_{h}_{qb}_{kb}")
                        nc.scalar.activation(
                            expT[:, co:], exp_in[:, co:], EXP, scale=float(SCALE))
                        exps.append(expT)
                    for i in range(n):
                        if pend[i] is not None:
                            emit_av(i, *pend[i])
                        pend[i] = (kb, exps[i], co)
                for i in range(n):
                    emit_av(i, *pend[i])
                for i, h in enumerate(hs):
                    atu = wrk.tile([128, 512], F32, tag="atu",
                                   name=f"atu_{h}_{qb}")
                    nc.scalar.copy(atu[:], att_ps[i][:])
                    recip = wrk.tile([1, 512], F32, tag="rcp",
                                     name=f"rcp_{h}_{qb}")
                    nc.vector.reciprocal(recip[:], sum_ps[i][:])
                    rb = wrk.tile([128, 512], F32, tag="m2",
                                  name=f"rb_{h}_{qb}")
                    nc.gpsimd.partition_broadcast(rb[:], recip[:])
                    at2 = wrk.tile([128, 512], F32R, tag="vT",
                                   name=f"at2_{h}_{qb}")
                    nc.vector.tensor_mul(at2[:], atu[:], rb[:])
                    nc.scalar.dma_start(
                        attnT_spill[h * 128:(h + 1) * 128, qs:qs + 512], at2[:])

            # ============ Phase A ============
            for th in range(NTH):
                ts = th * THW
                hts = []
                for j in range(8):
                    t = one.tile([128, 4 * THW], F32R, tag=f"hT{j}")
                    half, jj = divmod(j, 4)
                    nc.sync.dma_start(
                        t[:, :1024], hT[th, half, :, jj * 2048:jj * 2048 + 1024])
                    nc.sync.dma_start(
                        t[:, 1024:], hT[th, half, :, jj * 2048 + 1024:(jj + 1) * 2048])
                    hts.append(t)
                cos_t = wrk.tile([HD, THW], F32, tag="cos")
                sin_t = wrk.tile([HD, THW], F32, tag="sin")
                nc.sync.dma_start(cos_t[:], cosT[:, ts:ts + THW])
                nc.sync.dma_start(sin_t[:], sinTr[:, ts:ts + THW])

                qT_lo = one.tile([128, 4 * 512], F32R, tag="qTbl")
                qT_hi = one.tile([128, 4 * 512], F32R, tag="qTbh")

                for cb in range(NCB):
                    if cb < 8:
                        wsrc, widx = wq, cb
                    elif cb < 10:
                        wsrc, widx = wk, cb - 8
                    else:
                        wsrc, widx = wv, cb - 10
                    ps = psp.tile([128, THW], F32, tag="pa")
                    for half in range(2):
                        w_sb = wrk.tile([128, (NCH // 2) * 128], F32R, tag="w")
                        nc.sync.dma_start(w_sb[:, :1024], wsrc[widx, half, :, :1024])
                        nc.sync.dma_start(w_sb[:, 1024:], wsrc[widx, half, :, 1024:])
                        for i in range(NCH // 2):
                            ic = half * (NCH // 2) + i
                            t = hts[ic // 4]
                            nc.tensor.matmul(
                                ps[:],
                                w_sb[:, i * 128:(i + 1) * 128],
                                t[:, (ic % 4) * THW:(ic % 4 + 1) * THW],
                                start=(half == 0 and i == 0),
                                stop=(half == 1 and i == NCH // 2 - 1),
                            )
                    if cb < 10:
                        m1 = wrk.tile([128, THW], F32, tag="m1")
                        nc.vector.tensor_mul(m1[:], ps[:], cos_t[:])
                        m2 = wrk.tile([128, THW], F32, tag="m2")
                        nc.vector.tensor_mul(m2[0:64, :], ps[64:128, :], sin_t[0:64, :])
                        nc.vector.tensor_mul(m2[64:128, :], ps[0:64, :], sin_t[64:128, :])
                        if cb < 8:
                            qdst = qT_lo if cb < 4 else qT_hi
                            nc.vector.tensor_add(
                                qdst[:, (cb % 4) * 512:(cb % 4 + 1) * 512],
                                m1[:], m2[:])
                        else:
                            kv = cb - 8
                            nc.vector.tensor_add(
                                kT_sb[:, kv * TOK + ts: kv * TOK + ts + THW],
                                m1[:], m2[:])
                    else:
                        kv = cb - 10
                        vT = wrk.tile([128, THW], F32R, tag="vT")
                        nc.scalar.copy(vT[:], ps[:])
                        for j in range(THW // 128):
                            tb = th * (THW // 128) + j
                            pt = psp.tile([128, 128], F32R, tag="aux")
                            nc.tensor.transpose(
                                pt[:], vT[:, j * 128:(j + 1) * 128], ident_sb[:])
                            nc.scalar.copy(
                                V_sb[:, tb * KCOLS + kv * 128:
                                     tb * KCOLS + (kv + 1) * 128],
                                pt[:])

                for qi, qt in ((0, qT_lo), (1, qT_hi)):
                    nc.scalar.dma_start(
                        qT_spill[qi * 512:(qi + 1) * 512, ts:ts + THW]
                        .rearrange("(i p) t -> p i t", p=128),
                        qt[:].rearrange("p (i t) -> p i t", i=4),
                    )

            for hp in range(0, QH, 2):
                for qb in range(4):
                    qts = []
                    for h in (hp, hp + 1):
                        qT_t = wrk.tile([128, 512], F32R, tag="qTs",
                                        name=f"qt_{h}_{qb}")
                        nc.sync.dma_start(
                            qT_t[:],
                            qT_spill[h * 128:(h + 1) * 128,
                                     qb * 512:(qb + 1) * 512])
                        qts.append(qT_t)
                    attention_group([hp, hp + 1], qb, qts)

            # ================= Phase C: o_proj partial =================
            ags = []
            for h in range(QH):
                a = one.tile([128, TOK], F32R, tag=f"hT{h}")
                nc.sync.dma_start(a[:], attnT_spill[h * 128:(h + 1) * 128, :])
                ags.append(a)
            for nb in range(D // 512):
                wo_sb = wrk.tile([128, QH * 512], F32R, tag="w")
                for hc in range(QH):
                    nc.sync.dma_start(
                        wo_sb[:, hc * 512:(hc + 1) * 512], wo[nb, hc])
                for qtb in range(TOK // 128):
                    o_ps = psp.tile([128, 512], F32, tag=["pa", "pb", "aux", "sum"][qtb % 4])
                    for hc in range(QH):
                        nc.tensor.matmul(
                            o_ps[:],
                            ags[hc][:, qtb * 128:(qtb + 1) * 128],
                            wo_sb[:, hc * 512:(hc + 1) * 512],
                            start=(hc == 0), stop=(hc == QH - 1))
                    ot = wrk.tile([128, 512], F32, tag="ot", bufs=4)
                    nc.scalar.copy(ot[:], o_ps[:])
                    nc.scalar.dma_start(
                        out[qtb * 128:(qtb + 1) * 128, nb * 512:(nb + 1) * 512],
                        ot[:])

    nc.compile()
    return nc


def _get_program(variant: str):
    if variant not in _PROGRAMS:
        if variant == "causal":
            _PROGRAMS[variant] = _build_causal()
        else:
            _PROGRAMS[variant] = _build_program(variant)
    return _PROGRAMS[variant]


def _detect_variant(mask: np.ndarray) -> str:
    m = mask.reshape(mask.shape[-2], mask.shape[-1])
    if not m.any():
        return "zero"
    causal = np.where(
        np.tril(np.ones((S, S), dtype=bool)), np.float32(0.0), np.float32(NEG))
    if np.array_equal(m, causal):
        return "causal"
    return "general"


def _tile_w(W, np_dt):  # [4096, C] -> [C//128, 2, 128, 2048]
    C = W.shape[1]
    return np.ascontiguousarray(
        W.reshape(2, 16, 128, C // 128, 128).transpose(3, 0, 2, 1, 4)
        .reshape(C // 128, 2, 128, 16 * 128).astype(np_dt))


def _tile_w2(W, np_dt):  # [4096, C] -> [C//128, 128, 4096] (half-major cols)
    C = W.shape[1]
    return np.ascontiguousarray(
        W.reshape(2, 16, 128, C // 128, 128).transpose(3, 2, 0, 1, 4)
        .reshape(C // 128, 128, 4096).astype(np_dt))


def _kernel_causal(hidden_states, cos, sin, Wq, Wk, Wv, Wo, trace):
    nc = _get_program("causal")

    i = np.arange(128)[:, None]
    j = np.arange(128)[None, :]
    mask_diag = np.where(i <= j, np.float32(0.0),
                         np.float32(NEG / SCALE)).astype(np.float32)
    ones_sq = np.ones((128, 128), dtype=np.float32)

    per_batch = {}
    for b in range(B):
        sT = np.ascontiguousarray(sin[b].T)
        sinTr = np.concatenate([-sT[:64], sT[64:]], axis=0)
        hid = hidden_states[b]  # [2048, 4096]
        hT_t = np.ascontiguousarray(
            hid.reshape(4, 512, 2, 16, 128).transpose(0, 2, 4, 3, 1)
            .reshape(4, 2, 128, 16 * 512).astype(BF16_NP))
        per_batch[b] = (hT_t,
                        np.ascontiguousarray(cos[b].T.astype(BF16_NP)),
                        np.ascontiguousarray(sinTr.astype(BF16_NP)))

    in_maps = []
    for c in range(NCORES):
        b, g = divmod(c, 4)
        hT_t, cosT, sinTr = per_batch[b]
        wo_c = Wo[g * QCOLS:(g + 1) * QCOLS, :]  # [1024, 4096]
        wo_t = np.ascontiguousarray(
            wo_c.reshape(8, 128, 8, 512).transpose(2, 1, 0, 3)
            .reshape(8, 128, 8 * 512).astype(BF16_NP))
        wv_c = Wv[:, g * KCOLS:(g + 1) * KCOLS]  # [4096, 256]
        wvT_t = np.ascontiguousarray(
            wv_c.reshape(32, 128, 256).transpose(1, 0, 2)
            .reshape(128, 32 * 256).astype(BF16_NP))
        im = {
            "hT": hT_t,
            "wq": _tile_w2(Wq[:, g * QCOLS:(g + 1) * QCOLS], BF16_NP),
            "wk": _tile_w2(Wk[:, g * KCOLS:(g + 1) * KCOLS], BF16_NP),
            "wvT": wvT_t,
            "wo": wo_t,
            "cosT": cosT,
            "sinTr": sinTr,
            "ones_sq": ones_sq,
            "mask_diag": mask_diag,
        }
        in_maps.append(im)

    res = run_bass_kernel_spmd(nc, in_maps, core_ids=list(range(NCORES)),
                               trace=trace)
    if trace:
        print(f"HW exec time: {res.exec_time_ns} ns")

    out = np.empty((B, S, D), dtype=np.float32)
    for b in range(B):
        acc = np.zeros((S, D), dtype=np.float64)
        for g in range(4):
            acc += np.asarray(res.results[4 * b + g]["out"], dtype=np.float64)
        out[b] = acc.astype(np.float32)
    return out


def _kernel_legacy(variant, hidden_states, cos, sin, attention_mask,
                   Wq, Wk, Wv, Wo, trace):
    nc = _get_program(variant)

    ident = np.eye(128, dtype=np.float32)
    ones = np.ones((128, 1), dtype=np.float32)

    if variant == "general":
        m = attention_mask.reshape(S, S)
        maskT = np.ascontiguousarray(m.T / np.float32(SCALE))
    else:
        maskT = None

    per_batch = {}
    for b in range(B):
        sT = np.ascontiguousarray(sin[b].T)
        sinTr = np.concatenate([-sT[:64], sT[64:]], axis=0)
        hid = hidden_states[b]  # [2048, 4096]
        hT_t = np.ascontiguousarray(
            hid.reshape(4, 512, 2, 16, 128).transpose(0, 2, 4, 3, 1)
            .reshape(4, 2, 128, 16 * 512))
        per_batch[b] = (hT_t, np.ascontiguousarray(cos[b].T),
                        np.ascontiguousarray(sinTr))

    in_maps = []
    for c in range(NCORES):
        b, g = divmod(c, 4)
        hT_t, cosT, sinTr = per_batch[b]
        wo_c = Wo[g * QCOLS:(g + 1) * QCOLS, :]  # [1024, 4096]
        wo_t = np.ascontiguousarray(
            wo_c.reshape(8, 128, 8, 512).transpose(2, 0, 1, 3))
        im = {
            "hT": hT_t,
            "wq": _tile_w(Wq[:, g * QCOLS:(g + 1) * QCOLS], np.float32),
            "wk": _tile_w(Wk[:, g * KCOLS:(g + 1) * KCOLS], np.float32),
            "wv": _tile_w(Wv[:, g * KCOLS:(g + 1) * KCOLS], np.float32),
            "wo": wo_t,
            "cosT": cosT,
            "sinTr": sinTr,
            "ident": ident,
            "ones": ones,
        }
        if maskT is not None:
            im["maskT"] = maskT
        in_maps.append(im)

    res = run_bass_kernel_spmd(nc, in_maps, core_ids=list(range(NCORES)),
                               trace=trace)
    if trace:
        print(f"HW exec time: {res.exec_time_ns} ns")

    out = np.empty((B, S, D), dtype=np.float32)
    for b in range(B):
        acc = np.zeros((S, D), dtype=np.float64)
        for g in range(4):
            acc += res.results[4 * b + g]["out"]
        out[b] = acc.astype(np.float32)
    return out


def kernel(hidden_states, cos, sin, attention_mask, Wq, Wk, Wv, Wo):
    hidden_states = np.asarray(hidden_states, dtype=np.float32)
    cos = np.asarray(cos, dtype=np.float32)
    sin = np.asarray(sin, dtype=np.float32)
    attention_mask = np.asarray(attention_mask, dtype=np.float32)
    Wq = np.asarray(Wq, dtype=np.float32)
    Wk = np.asarray(Wk, dtype=np.float32)
    Wv = np.asarray(Wv, dtype=np.float32)
    Wo = np.asarray(Wo, dtype=np.float32)

    trace = bool(os.environ.get("KERNEL_TRACE"))
    variant = _detect_variant(attention_mask)
    if variant == "causal":
        return _kernel_causal(hidden_states, cos, sin, Wq, Wk, Wv, Wo, trace)
    return _kernel_legacy(variant, hidden_states, cos, sin, attention_mask,
                          Wq, Wk, Wv, Wo, trace)


# revision 35
# speedup vs baseline: 1.2380x; 1.0027x over previous
"""Mistral attention (B=2, S=2048, D=4096, H=32, KVH=8, HD=128) on 8 trn2 cores.

Sharding: core c -> (batch b = c//4, head-group g = c%4).
Each core computes q/k/v projections for its 8 Q heads + 2 KV heads of one
batch, RoPE, causal attention, and a row-parallel partial o_proj
[2048, 4096]. Host sums the 4 partials per batch. No collectives.

Causal fast path (v2):
- All matmul operands are bf16 (same 1 cycle/row PE rate as float32r, half
  the DMA bytes, no 4x small-free penalty on the causal diagonal strips).
- Weights are streamed per token block in bf16; hidden/cos/sin in bf16.
- Attention is transposed (scoresT[keys, qtok], keys on partitions).
  Softmax denominator: exp tiles are accumulated across key blocks on the
  vector engine, then one ones[128,128]-stationary matmul per (head,qblock)
  produces the denominator pre-broadcast across partitions; a fast
  approximate reciprocal replaces the slow DVE reciprocal.
- The causal mask is added only on the true-diagonal 128x128 tiles
  (in place, into the scores psum).
- o_proj for token block t is fused and its matmuls are interleaved as
  filler work into token block t+1's attention rounds, so the in-order PE
  queue always has independent work while waiting for exp results.
- Output partials are written in bf16 and summed on the host in fp64.
"""

import os
import sys

for _p in ("/opt/trn_rl_repo",):
    if _p not in sys.path:
        sys.path.insert(0, _p)

import numpy as np

import concourse.bass as bass
import concourse.tile as tile
from concourse import bacc, bass_isa, mybir
from concourse.bass_utils import run_bass_kernel_spmd

F32 = mybir.dt.float32
F32R = mybir.dt.float32r
BF16 = mybir.dt.bfloat16
EXP = mybir.ActivationFunctionType.Exp
BF16_NP = mybir.dt.np(mybir.dt.bfloat16)

B, S, D = 2, 2048, 4096
H, KVH, HD = 32, 8, 128
SCALE = HD ** -0.5
NCORES = 8

QH = H // 4              # 8 q heads per core
QCOLS = QH * HD          # 1024
KCOLS = (KVH // 4) * HD  # 256 (2 kv heads per core)
TOK = S

NEG = -1e9

_PROGRAMS = {}


def _build_causal():
    nc = bacc.Bacc("TRN2", target_bir_lowering=False, debug=False)

    hT = nc.dram_tensor("hT", [4, 2, 128, 16 * 512], BF16, kind="ExternalInput").ap()
    wq = nc.dram_tensor("wq", [8, 128, 4096], BF16, kind="ExternalInput").ap()
    wk = nc.dram_tensor("wk", [2, 128, 4096], BF16, kind="ExternalInput").ap()
    wvT = nc.dram_tensor("wvT", [128, 32 * 256], BF16, kind="ExternalInput").ap()
    wo = nc.dram_tensor("wo", [8, 128, 8 * 512], BF16, kind="ExternalInput").ap()
    cosT = nc.dram_tensor("cosT", [HD, TOK], BF16, kind="ExternalInput").ap()
    sinTr = nc.dram_tensor("sinTr", [HD, TOK], BF16, kind="ExternalInput").ap()
    ones_sq = nc.dram_tensor("ones_sq", [128, 128], F32R, kind="ExternalInput").ap()
    mask_diag = nc.dram_tensor("mask_diag", [128, 128], F32, kind="ExternalInput").ap()
    out = nc.dram_tensor("out", [TOK, D], BF16, kind="ExternalOutput").ap()
    gate_spill = nc.dram_tensor("gate_spill", [1, 8], BF16).ap()

    with tile.TileContext(nc) as tc:
        with tc.tile_pool(name="per", bufs=1) as per, \
             tc.tile_pool(name="wrk", bufs=2) as wrk, \
             tc.tile_pool(name="ps", bufs=2, space="PSUM") as psp:

            mask_sb = per.tile([128, 128], F32, tag="mask")
            ones_sb = per.tile([128, 128], F32R, tag="ones")
            cos_sb = per.tile([HD, TOK], BF16, tag="cos")
            sin_sb = per.tile([HD, TOK], BF16, tag="sin")
            kT_sb = per.tile([HD, 2 * TOK], BF16, tag="kT")
            V_sb = per.tile([128, 16 * 256], BF16, tag="V")  # tb-major: tb*256+kv*128
            wv_sb = per.tile([128, 32 * 256], BF16, tag="wv")

            # ---- o_proj for token block th, yielded one PE-op at a time ----
            def oproj_gen(th, at2_t):
                wo_ts = {}

                def load(nb):
                    t = wrk.tile([128, 4096], BF16, tag="wo", bufs=4,
                                 name=f"wo_{th}_{nb}")
                    nc.sync.dma_start(t[:], wo[nb])
                    wo_ts[nb] = t

                load(0)
                load(1)
                for nb in range(8):
                    wo_t = wo_ts.pop(nb)
                    if nb + 2 < 8:
                        load(nb + 2)
                    for j in range(4):
                        po = psp.tile([128, 512], F32, tag="pc",
                                      name=f"po_{th}_{nb}_{j}")
                        for hc in range(8):
                            nc.tensor.matmul(
                                po[:],
                                at2_t[:, hc * 512 + j * 128: hc * 512 + j * 128 + 128],
                                wo_t[:, hc * 512:(hc + 1) * 512],
                                start=(hc == 0), stop=(hc == 7))
                            yield
                        ot = wrk.tile([128, 512], BF16, tag="ot", bufs=4,
                                      name=f"ot_{th}_{nb}_{j}")
                        nc.scalar.copy(ot[:], po[:])
                        nc.scalar.dma_start(
                            out[th * 512 + j * 128: th * 512 + (j + 1) * 128,
                                nb * 512:(nb + 1) * 512],
                            ot[:])
                        yield

            filler = [iter(())]

            def fill(n):
                for _ in range(n):
                    try:
                        next(filler[0])
                    except StopIteration:
                        return

            def attention_group(th, hs, qT_t, at2_t, fin_q, fill_n=8):
                """Transposed causal attention for q heads hs, query block th."""
                nkb = 4 * th + 4
                att_ps = [psp.tile([128, 512], F32, tag="att",
                                   name=f"att_{h}_{th}") for h in hs]
                accs = [wrk.tile([128, 512], F32R, tag="acc", bufs=4,
                                 name=f"acc_{h}_{th}") for h in hs]
                pend = [None, None]
                pend_acc = [None, None]

                def emit_av(i, kb, expT, co):
                    h = hs[i]
                    kv = h // 4
                    nc.tensor.matmul(
                        att_ps[i][:, co:],
                        V_sb[:, kb * 256 + kv * 128: kb * 256 + (kv + 1) * 128],
                        expT[:, co:],
                        start=(kb == 0), stop=(kb == nkb - 1))

                def emit_acc(i, kb, expT, co):
                    if kb == 0:
                        nc.vector.tensor_scalar_add(accs[i][:], expT[:], 0.0)
                    else:
                        nc.vector.tensor_add(
                            accs[i][:, co:], accs[i][:, co:], expT[:, co:])

                for kb in range(nkb):
                    o = kb - 4 * th
                    co = o * 128 if o > 0 else 0
                    exps = []
                    for i, h in enumerate(hs):
                        kv = h // 4
                        s_ps = psp.tile([128, 512], F32, tag="pb",
                                        name=f"s_{h}_{th}_{kb}")
                        nc.tensor.matmul(
                            s_ps[:, co:],
                            kT_sb[:, kv * TOK + kb * 128: kv * TOK + (kb + 1) * 128],
                            qT_t[:, h * 512 + co: (h + 1) * 512],
                            start=True, stop=True)
                        if o >= 0:
                            nc.vector.tensor_add(
                                s_ps[:, co:co + 128], s_ps[:, co:co + 128],
                                mask_sb[:])
                        expT = wrk.tile([128, 512], BF16, tag="expT", bufs=4,
                                        name=f"exp_{h}_{th}_{kb}")
                        nc.scalar.activation(
                            expT[:, co:], s_ps[:, co:], EXP, scale=float(SCALE))
                        exps.append(expT)
                    for i in range(2):
                        if pend[i] is not None:
                            emit_av(i, *pend[i])
                        pend[i] = (kb, exps[i], co)
                    for i in range(2):
                        if pend_acc[i] is not None:
                            emit_acc(i, *pend_acc[i])
                        pend_acc[i] = (kb, exps[i], co)
                    if kb == 2 and fin_q:
                        fin_q.pop(0)()
                    fill(fill_n)
                for i in range(2):
                    emit_av(i, *pend[i])
                    emit_acc(i, *pend_acc[i])

                def finisher():
                    for i, h in enumerate(hs):
                        den_ps = psp.tile([128, 512], F32, tag="pc",
                                          name=f"den_{h}_{th}")
                        nc.tensor.matmul(den_ps[:], ones_sb[:], accs[i][:],
                                         start=True, stop=True)
                        rb = wrk.tile([128, 512], F32, tag="rb",
                                      name=f"rb_{h}_{th}")
                        nc.vector.reciprocal_approx_fast(rb[:], den_ps[:])
                        nc.vector.tensor_mul(
                            at2_t[:, h * 512:(h + 1) * 512], att_ps[i][:], rb[:])
                return finisher

            for th in range(4):
                ts = th * 512
                hts = []
                for jj in range(8):
                    t = wrk.tile([128, 2048], BF16, tag="hT", bufs=11,
                                 name=f"hT_{th}_{jj}")
                    half, j4 = divmod(jj, 4)
                    if th == 0 and jj == 0:
                        # split the startup-critical first tiles so the very
                        # first matmuls have fine-grained completion to wait on
                        nc.sync.dma_start(
                            t[:, :512], hT[0, 0, :, :512])
                        w_first = wrk.tile([128, 4096], BF16, tag="w", bufs=3,
                                           name="w_k0_0")
                        nc.sync.dma_start(w_first[:, :512], wk[0, :, :512])
                        nc.sync.dma_start(
                            t[:, 512:], hT[0, 0, :, 512:2048])
                        nc.sync.dma_start(w_first[:, 512:], wk[0, :, 512:])
                    else:
                        nc.sync.dma_start(
                            t[:], hT[th, half, :, j4 * 2048:(j4 + 1) * 2048])
                    hts.append(t)
                def w_fetch(wsrc, idx, kind):
                    w_t = wrk.tile([128, 4096], BF16, tag="w", bufs=3,
                                   name=f"w_{kind}{idx}_{th}")
                    nc.sync.dma_start(w_t[:], wsrc[idx])
                    return w_t

                pre_w = {}
                if th == 0:
                    # Gate: this dummy store's source depends on the last
                    # hidden tile, so the sync engine stalls here and the
                    # bulk DMAs below don't steal HBM bandwidth from the
                    # critical startup set (hidden tiles + first weights).
                    nc.sync.dma_start(gate_spill[:], hts[5][0:1, 0:8])
                    nc.sync.dma_start(cos_sb[:], cosT[:])
                    nc.sync.dma_start(sin_sb[:], sinTr[:])
                    pre_w[("k", 1)] = w_fetch(wk, 1, "k")
                    pre_w[("q", 0)] = w_fetch(wq, 0, "q")
                    pre_w[("q", 1)] = w_fetch(wq, 1, "q")
                    nc.sync.dma_start(mask_sb[:], mask_diag[:])
                    nc.sync.dma_start(ones_sb[:], ones_sq[:])
                    nc.sync.dma_start(wv_sb[:], wvT[:])
                qT_t = wrk.tile([128, 8 * 512], BF16, tag="qT", name=f"qT_{th}")
                at2_t = wrk.tile([128, 8 * 512], BF16, tag="at2", name=f"at2_{th}")

                def proj_block(wsrc, idx, kind, pre=None):
                    if pre is None:
                        pre = pre_w.pop((kind, idx), None)
                    if pre is not None:
                        w_t = pre
                    else:
                        w_t = w_fetch(wsrc, idx, kind)
                    ps = psp.tile([128, 512], F32, tag="pa",
                                  name=f"ps_{kind}{idx}_{th}")
                    for ic in range(32):
                        half, i = divmod(ic, 16)
                        nc.tensor.matmul(
                            ps[:],
                            w_t[:, half * 2048 + i * 128: half * 2048 + (i + 1) * 128],
                            hts[ic // 4][:, (ic % 4) * 512: (ic % 4 + 1) * 512],
                            start=(ic == 0), stop=(ic == 31))
                    return ps

                def rope(ps, dst, kind, idx):
                    m1 = wrk.tile([128, 512], F32, tag="m1",
                                  name=f"m1_{kind}{idx}_{th}")
                    nc.vector.tensor_mul(m1[:], ps[:], cos_sb[:, ts:ts + 512])
                    m2 = wrk.tile([128, 512], F32, tag="m2",
                                  name=f"m2_{kind}{idx}_{th}")
                    nc.vector.tensor_mul(
                        m2[0:64, :], ps[64:128, :], sin_sb[0:64, ts:ts + 512])
                    nc.vector.tensor_mul(
                        m2[64:128, :], ps[0:64, :], sin_sb[64:128, ts:ts + 512])
                    nc.vector.tensor_add(dst, m1[:], m2[:])

                for kv in range(2):
                    ps = proj_block(wk, kv, "k",
                                    pre=w_first if (th == 0 and kv == 0) else None)
                    rope(ps, kT_sb[:, kv * TOK + ts: kv * TOK + ts + 512], "k", kv)
                    fill(4)
                fin_q = []
                for hp in range(0, QH, 2):
                    for h in (hp, hp + 1):
                        ps = proj_block(wq, h, "q")
                        rope(ps, qT_t[:, h * 512:(h + 1) * 512], "q", h)
                        fill(4)
                    if hp == 0:
                        # V^T: out[tok, vdim] with hidden chunks stationary
                        for j in range(4):
                            tb = th * 4 + j
                            pv = psp.tile([128, 256], F32, tag="pa",
                                          name=f"pv_{th}_{j}")
                            for c in range(32):
                                nc.tensor.matmul(
                                    pv[:],
                                    hts[c // 4][:, (c % 4) * 512 + j * 128:
                                                (c % 4) * 512 + (j + 1) * 128],
                                    wv_sb[:, c * 256:(c + 1) * 256],
                                    start=(c == 0), stop=(c == 31))
                            nc.scalar.copy(V_sb[:, tb * 256:(tb + 1) * 256], pv[:])
                            fill(2)
                    fin = attention_group(th, [hp, hp + 1], qT_t, at2_t, fin_q,
                                          fill_n=(8, 9, 6, 5)[th])
                    fin_q.append(fin)
                while fin_q:
                    fin_q.pop(0)()
                    fill(8)

                # drain previous block's o_proj, then queue this block's
                fill(1 << 30)
                filler[0] = oproj_gen(th, at2_t)
            fill(1 << 30)

    nc.compile()
    return nc


def _build_program(variant: str):
    """variant: 'zero' | 'general' (legacy fp32r path, kept as fallback)"""
    nc = bacc.Bacc("TRN2", target_bir_lowering=False, debug=False)

    hT = nc.dram_tensor("hT", [4, 2, 128, 16 * 512], F32R, kind="ExternalInput").ap()
    wq = nc.dram_tensor("wq", [8, 2, 128, 16 * 128], F32R, kind="ExternalInput").ap()
    wk = nc.dram_tensor("wk", [2, 2, 128, 16 * 128], F32R, kind="ExternalInput").ap()
    wv = nc.dram_tensor("wv", [2, 2, 128, 16 * 128], F32R, kind="ExternalInput").ap()
    wo = nc.dram_tensor("wo", [8, 8, 128, 512], F32R, kind="ExternalInput").ap()
    cosT = nc.dram_tensor("cosT", [HD, TOK], F32, kind="ExternalInput").ap()
    sinTr = nc.dram_tensor("sinTr", [HD, TOK], F32, kind="ExternalInput").ap()
    ident = nc.dram_tensor("ident", [128, 128], F32R, kind="ExternalInput").ap()
    ones = nc.dram_tensor("ones", [128, 1], F32R, kind="ExternalInput").ap()
    if variant == "general":
        maskT = nc.dram_tensor("maskT", [S, S], F32, kind="ExternalInput").ap()
    else:
        maskT = None
    out = nc.dram_tensor("out", [TOK, D], F32, kind="ExternalOutput").ap()

    attnT_spill = nc.dram_tensor("attnT_spill", [QCOLS, TOK], F32R).ap()
    qT_spill = nc.dram_tensor("qT_spill", [QCOLS, TOK], F32R).ap()

    NTH = 4
    THW = TOK // NTH         # 512
    NCH = D // 128           # 32 contraction chunks
    NCB = (QCOLS + 2 * KCOLS) // 128  # 12: 0-7 q, 8-9 k, 10-11 v

    with tile.TileContext(nc) as tc:
        with tc.tile_pool(name="per", bufs=1) as per, \
             tc.tile_pool(name="wrk", bufs=2) as wrk, \
             tc.tile_pool(name="one", bufs=1) as one, \
             tc.tile_pool(name="ps", bufs=2, space="PSUM") as psp:

            ident_sb = per.tile([128, 128], F32R, tag="ident")
            ones_sb = per.tile([128, 1], F32R, tag="ones")
            kT_sb = per.tile([HD, 2 * TOK], F32R, tag="kT")
            V_sb = per.tile([128, (TOK // 128) * KCOLS], F32R, tag="V")
            nc.sync.dma_start(ident_sb[:], ident[:])
            nc.sync.dma_start(ones_sb[:], ones[:])

            def attention_group(hs, qb, qT_aps):
                qs = qb * 512
                nkb = TOK // 128
                n = len(hs)
                att_ps = [psp.tile([128, 512], F32, tag="aux", name=f"att_{h}_{qb}")
                          for h in hs]
                sum_ps = [psp.tile([1, 512], F32, tag="sum", name=f"sum_{h}_{qb}")
                          for h in hs]

                def emit_av(i, kb, expT, co):
                    h = hs[i]
                    kv = h // (QH // 2)
                    nc.tensor.matmul(
                        att_ps[i][:, co:],
                        V_sb[:, kb * KCOLS + kv * 128: kb * KCOLS + (kv + 1) * 128],
                        expT[:, co:],
                        start=(kb == 0), stop=(kb == nkb - 1))
                    nc.tensor.matmul(
                        sum_ps[i][:, co:], ones_sb[:], expT[:, co:],
                        start=(kb == 0), stop=(kb == nkb - 1))

                pend = [None] * n
                for kb in range(nkb):
                    co = 0
                    exps = []
                    for i, h in enumerate(hs):
                        kv = h // (QH // 2)
                        s_ps = psp.tile([128, 512], F32, tag="pb",
                                        name=f"s_{h}_{qb}_{kb}")
                        nc.tensor.matmul(
                            s_ps[:, co:],
                            kT_sb[:, kv * TOK + kb * 128: kv * TOK + (kb + 1) * 128],
                            qT_aps[i][:, co:],
                            start=True, stop=True)
                        exp_in = s_ps
                        if variant == "general":
                            mt = wrk.tile([128, 512], F32, tag="mt",
                                          name=f"mt_{h}_{qb}_{kb}")
                            nc.sync.dma_start(
                                mt[:], maskT[kb * 128:(kb + 1) * 128, qs:qs + 512])
                            msk = wrk.tile([128, 512], F32, tag="m1",
                                           name=f"mskg_{h}_{qb}_{kb}")
                            nc.vector.tensor_add(msk[:], s_ps[:], mt[:])
                            exp_in = msk
                        expT = wrk.tile([128, 512], F32R, tag="expT", bufs=4,
                                        name=f"exp_{h}_{qb}_{kb}")
                        nc.scalar.activation(
                            expT[:, co:], exp_in[:, co:], EXP, scale=float(SCALE))
                        exps.append(expT)
                    for i in range(n):
                        if pend[i] is not None:
                            emit_av(i, *pend[i])
                        pend[i] = (kb, exps[i], co)
                for i in range(n):
                    emit_av(i, *pend[i])
                for i, h in enumerate(hs):
                    atu = wrk.tile([128, 512], F32, tag="atu",
                                   name=f"atu_{h}_{qb}")
                    nc.scalar.copy(atu[:], att_ps[i][:])
                    recip = wrk.tile([1, 512], F32, tag="rcp",
                                     name=f"rcp_{h}_{qb}")
                    nc.vector.reciprocal(recip[:], sum_ps[i][:])
                    rb = wrk.tile([128, 512], F32, tag="m2",
                                  name=f"rb_{h}_{qb}")
                    nc.gpsimd.partition_broadcast(rb[:], recip[:])
                    at2 = wrk.tile([128, 512], F32R, tag="vT",
                                   name=f"at2_{h}_{qb}")
                    nc.vector.tensor_mul(at2[:], atu[:], rb[:])
                    nc.scalar.dma_start(
                        attnT_spill[h * 128:(h + 1) * 128, qs:qs + 512], at2[:])

            # ============ Phase A ============
            for th in range(NTH):
                ts = th * THW
                hts = []
                for j in range(8):
                    t = one.tile([128, 4 * THW], F32R, tag=f"hT{j}")
                    half, jj = divmod(j, 4)
                    nc.sync.dma_start(
                        t[:, :1024], hT[th, half, :, jj * 2048:jj * 2048 + 1024])
                    nc.sync.dma_start(
                        t[:, 1024:], hT[th, half, :, jj * 2048 + 1024:(jj + 1) * 2048])
                    hts.append(t)
                cos_t = wrk.tile([HD, THW], F32, tag="cos")
                sin_t = wrk.tile([HD, THW], F32, tag="sin")
                nc.sync.dma_start(cos_t[:], cosT[:, ts:ts + THW])
                nc.sync.dma_start(sin_t[:], sinTr[:, ts:ts + THW])

                qT_lo = one.tile([128, 4 * 512], F32R, tag="qTbl")
                qT_hi = one.tile([128, 4 * 512], F32R, tag="qTbh")

                for cb in range(NCB):
                    if cb < 8:
                        wsrc, widx = wq, cb
                    elif cb < 10:
                        wsrc, widx = wk, cb - 8
                    else:
                        wsrc, widx = wv, cb - 10
                    ps = psp.tile([128, THW], F32, tag="pa")
                    for half in range(2):
                        w_sb = wrk.tile([128, (NCH // 2) * 128], F32R, tag="w")
                        nc.sync.dma_start(w_sb[:, :1024], wsrc[widx, half, :, :1024])
                        nc.sync.dma_start(w_sb[:, 1024:], wsrc[widx, half, :, 1024:])
                        for i in range(NCH // 2):
                            ic = half * (NCH // 2) + i
                            t = hts[ic // 4]
                            nc.tensor.matmul(
                                ps[:],
                                w_sb[:, i * 128:(i + 1) * 128],
                                t[:, (ic % 4) * THW:(ic % 4 + 1) * THW],
                                start=(half == 0 and i == 0),
                                stop=(half == 1 and i == NCH // 2 - 1),
                            )
                    if cb < 10:
                        m1 = wrk.tile([128, THW], F32, tag="m1")
                        nc.vector.tensor_mul(m1[:], ps[:], cos_t[:])
                        m2 = wrk.tile([128, THW], F32, tag="m2")
                        nc.vector.tensor_mul(m2[0:64, :], ps[64:128, :], sin_t[0:64, :])
                        nc.vector.tensor_mul(m2[64:128, :], ps[0:64, :], sin_t[64:128, :])
                        if cb < 8:
                            qdst = qT_lo if cb < 4 else qT_hi
                            nc.vector.tensor_add(
                                qdst[:, (cb % 4) * 512:(cb % 4 + 1) * 512],
                                m1[:], m2[:])
                        else:
                            kv = cb - 8
                            nc.vector.tensor_add(
                                kT_sb[:, kv * TOK + ts: kv * TOK + ts + THW],
                                m1[:], m2[:])
                    else:
                        kv = cb - 10
                        vT = wrk.tile([128, THW], F32R, tag="vT")
                        nc.scalar.copy(vT[:], ps[:])
                        for j in range(THW // 128):
                            tb = th * (THW // 128) + j
                            pt = psp.tile([128, 128], F32R, tag="aux")
                            nc.tensor.transpose(
                                pt[:], vT[:, j * 128:(j + 1) * 128], ident_sb[:])
                            nc.scalar.copy(
                                V_sb[:, tb * KCOLS + kv * 128:
                                     tb * KCOLS + (kv + 1) * 128],
                                pt[:])

                for qi, qt in ((0, qT_lo), (1, qT_hi)):
                    nc.scalar.dma_start(
                        qT_spill[qi * 512:(qi + 1) * 512, ts:ts + THW]
                        .rearrange("(i p) t -> p i t", p=128),
                        qt[:].rearrange("p (i t) -> p i t", i=4),
                    )

            for hp in range(0, QH, 2):
                for qb in range(4):
                    qts = []
                    for h in (hp, hp + 1):
                        qT_t = wrk.tile([128, 512], F32R, tag="qTs",
                                        name=f"qt_{h}_{qb}")
                        nc.sync.dma_start(
                            qT_t[:],
                            qT_spill[h * 128:(h + 1) * 128,
                                     qb * 512:(qb + 1) * 512])
                        qts.append(qT_t)
                    attention_group([hp, hp + 1], qb, qts)

            # ================= Phase C: o_proj partial =================
            ags = []
            for h in range(QH):
                a = one.tile([128, TOK], F32R, tag=f"hT{h}")
                nc.sync.dma_start(a[:], attnT_spill[h * 128:(h + 1) * 128, :])
                ags.append(a)
            for nb in range(D // 512):
                wo_sb = wrk.tile([128, QH * 512], F32R, tag="w")
                for hc in range(QH):
                    nc.sync.dma_start(
                        wo_sb[:, hc * 512:(hc + 1) * 512], wo[nb, hc])
                for qtb in range(TOK // 128):
                    o_ps = psp.tile([128, 512], F32, tag=["pa", "pb", "aux", "sum"][qtb % 4])
                    for hc in range(QH):
                        nc.tensor.matmul(
                            o_ps[:],
                            ags[hc][:, qtb * 128:(qtb + 1) * 128],
                            wo_sb[:, hc * 512:(hc + 1) * 512],
                            start=(hc == 0), stop=(hc == QH - 1))
                    ot = wrk.tile([128, 512], F32, tag="ot", bufs=4)
                    nc.scalar.copy(ot[:], o_ps[:])
                    nc.scalar.dma_start(
                        out[qtb * 128:(qtb + 1) * 128, nb * 512:(nb + 1) * 512],
                        ot[:])

    nc.compile()
    return nc


def _get_program(variant: str):
    if variant not in _PROGRAMS:
        if variant == "causal":
            _PROGRAMS[variant] = _build_causal()
        else:
            _PROGRAMS[variant] = _build_program(variant)
    return _PROGRAMS[variant]


def _detect_variant(mask: np.ndarray) -> str:
    m = mask.reshape(mask.shape[-2], mask.shape[-1])
    if not m.any():
        return "zero"
    causal = np.where(
        np.tril(np.ones((S, S), dtype=bool)), np.float32(0.0), np.float32(NEG))
    if np.array_equal(m, causal):
        return "causal"
    return "general"


def _tile_w(W, np_dt):  # [4096, C] -> [C//128, 2, 128, 2048]
    C = W.shape[1]
    return np.ascontiguousarray(
        W.reshape(2, 16, 128, C // 128, 128).transpose(3, 0, 2, 1, 4)
        .reshape(C // 128, 2, 128, 16 * 128).astype(np_dt))


def _tile_w2(W, np_dt):  # [4096, C] -> [C//128, 128, 4096] (half-major cols)
    C = W.shape[1]
    return np.ascontiguousarray(
        W.reshape(2, 16, 128, C // 128, 128).transpose(3, 2, 0, 1, 4)
        .reshape(C // 128, 128, 4096).astype(np_dt))


def _kernel_causal(hidden_states, cos, sin, Wq, Wk, Wv, Wo, trace):
    nc = _get_program("causal")

    i = np.arange(128)[:, None]
    j = np.arange(128)[None, :]
    mask_diag = np.where(i <= j, np.float32(0.0),
                         np.float32(NEG / SCALE)).astype(np.float32)
    ones_sq = np.ones((128, 128), dtype=np.float32)

    per_batch = {}
    for b in range(B):
        sT = np.ascontiguousarray(sin[b].T)
        sinTr = np.concatenate([-sT[:64], sT[64:]], axis=0)
        hid = hidden_states[b]  # [2048, 4096]
        hT_t = np.ascontiguousarray(
            hid.reshape(4, 512, 2, 16, 128).transpose(0, 2, 4, 3, 1)
            .reshape(4, 2, 128, 16 * 512).astype(BF16_NP))
        per_batch[b] = (hT_t,
                        np.ascontiguousarray(cos[b].T.astype(BF16_NP)),
                        np.ascontiguousarray(sinTr.astype(BF16_NP)))

    in_maps = []
    for c in range(NCORES):
        b, g = divmod(c, 4)
        hT_t, cosT, sinTr = per_batch[b]
        wo_c = Wo[g * QCOLS:(g + 1) * QCOLS, :]  # [1024, 4096]
        wo_t = np.ascontiguousarray(
            wo_c.reshape(8, 128, 8, 512).transpose(2, 1, 0, 3)
            .reshape(8, 128, 8 * 512).astype(BF16_NP))
        wv_c = Wv[:, g * KCOLS:(g + 1) * KCOLS]  # [4096, 256]
        wvT_t = np.ascontiguousarray(
            wv_c.reshape(32, 128, 256).transpose(1, 0, 2)
            .reshape(128, 32 * 256).astype(BF16_NP))
        im = {
            "hT": hT_t,
            "wq": _tile_w2(Wq[:, g * QCOLS:(g + 1) * QCOLS], BF16_NP),
            "wk": _tile_w2(Wk[:, g * KCOLS:(g + 1) * KCOLS], BF16_NP),
            "wvT": wvT_t,
            "wo": wo_t,
            "cosT": cosT,
            "sinTr": sinTr,
            "ones_sq": ones_sq,
            "mask_diag": mask_diag,
        }
        in_maps.append(im)

    res = run_bass_kernel_spmd(nc, in_maps, core_ids=list(range(NCORES)),
                               trace=trace)
    if trace:
        print(f"HW exec time: {res.exec_time_ns} ns")

    out = np.empty((B, S, D), dtype=np.float32)
    for b in range(B):
        acc = np.zeros((S, D), dtype=np.float64)
        for g in range(4):
            acc += np.asarray(res.results[4 * b + g]["out"], dtype=np.float64)
        out[b] = acc.astype(np.float32)
    return out


def _kernel_legacy(variant, hidden_states, cos, sin, attention_mask,
                   Wq, Wk, Wv, Wo, trace):
    nc = _get_program(variant)

    ident = np.eye(128, dtype=np.float32)
    ones = np.ones((128, 1), dtype=np.float32)

    if variant == "general":
        m = attention_mask.reshape(S, S)
        maskT = np.ascontiguousarray(m.T / np.float32(SCALE))
    else:
        maskT = None

    per_batch = {}
    for b in range(B):
        sT = np.ascontiguousarray(sin[b].T)
        sinTr = np.concatenate([-sT[:64], sT[64:]], axis=0)
        hid = hidden_states[b]  # [2048, 4096]
        hT_t = np.ascontiguousarray(
            hid.reshape(4, 512, 2, 16, 128).transpose(0, 2, 4, 3, 1)
            .reshape(4, 2, 128, 16 * 512))
        per_batch[b] = (hT_t, np.ascontiguousarray(cos[b].T),
                        np.ascontiguousarray(sinTr))

    in_maps = []
    for c in range(NCORES):
        b, g = divmod(c, 4)
        hT_t, cosT, sinTr = per_batch[b]
        wo_c = Wo[g * QCOLS:(g + 1) * QCOLS, :]  # [1024, 4096]
        wo_t = np.ascontiguousarray(
            wo_c.reshape(8, 128, 8, 512).transpose(2, 0, 1, 3))
        im = {
            "hT": hT_t,
            "wq": _tile_w(Wq[:, g * QCOLS:(g + 1) * QCOLS], np.float32),
            "wk": _tile_w(Wk[:, g * KCOLS:(g + 1) * KCOLS], np.float32),
            "wv": _tile_w(Wv[:, g * KCOLS:(g + 1) * KCOLS], np.float32),
            "wo": wo_t,
            "cosT": cosT,
            "sinTr": sinTr,
            "ident": ident,
            "ones": ones,
        }
        if maskT is not None:
            im["maskT"] = maskT
        in_maps.append(im)

    res = run_bass_kernel_spmd(nc, in_maps, core_ids=list(range(NCORES)),
                               trace=trace)
    if trace:
        print(f"HW exec time: {res.exec_time_ns} ns")

    out = np.empty((B, S, D), dtype=np.float32)
    for b in range(B):
        acc = np.zeros((S, D), dtype=np.float64)
        for g in range(4):
            acc += res.results[4 * b + g]["out"]
        out[b] = acc.astype(np.float32)
    return out


def kernel(hidden_states, cos, sin, attention_mask, Wq, Wk, Wv, Wo):
    hidden_states = np.asarray(hidden_states, dtype=np.float32)
    cos = np.asarray(cos, dtype=np.float32)
    sin = np.asarray(sin, dtype=np.float32)
    attention_mask = np.asarray(attention_mask, dtype=np.float32)
    Wq = np.asarray(Wq, dtype=np.float32)
    Wk = np.asarray(Wk, dtype=np.float32)
    Wv = np.asarray(Wv, dtype=np.float32)
    Wo = np.asarray(Wo, dtype=np.float32)

    trace = bool(os.environ.get("KERNEL_TRACE"))
    variant = _detect_variant(attention_mask)
    if variant == "causal":
        return _kernel_causal(hidden_states, cos, sin, Wq, Wk, Wv, Wo, trace)
    return _kernel_legacy(variant, hidden_states, cos, sin, attention_mask,
                          Wq, Wk, Wv, Wo, trace)
